# revision 1
# baseline (speedup 1.0000x reference)
"""Trainium2 Bass kernel for nn_EquivariantInteractionBlock.

Strategy (edge/graph parallel, 8 cores):
- Host: sort edges by dst; split into 8 node-aligned contiguous ranges with
  ~E/8 edges each. Per core, pack edges into "supertiles": <=1024 edges
  covering a window of exactly 128 consecutive dst nodes (padding with
  dummy edges, ew=0). Host also gathers raw x rows by edge_src (pure data
  movement) and pre-swizzles all edge arrays into DMA-friendly layouts.
- Device per supertile:
  * radial MLP: h = silu(rbf@W1+b1) via TensorE (feature-major) + ACT silu
  * per-edge TP weights w = [h;1]@W2' via TensorE (edge-major, per-subtile
    stationary), pathwise PSUM tiles
  * tensor-product products on VectorE (PSUM-source), per-edge scalars
    (cutoff, gate, irrep-norm factors) batched per supertile
  * scatter-add via selection-matrix matmul into a PSUM agg window
    (i-reductions for paths 1/2 ride the scatter as extra columns)
  * node-wise update: norm-divide, msg_linear, gated nonlinearity,
    self-interaction, residual -- small feature-major matmuls + PE transposes
- Each core owns a disjoint node range: no collectives needed; host
  concatenates per-core output rows.
"""

import math
import numpy as np

import concourse.bass as bass
import concourse.mybir as mybir
import concourse.tile as tile
from concourse.bass_utils import run_bass_kernel_spmd
from concourse.masks import make_identity

F32 = mybir.dt.float32
AF = mybir.ActivationFunctionType
OP = mybir.AluOpType

N = 50000
E = 400000
MUL0 = 16
MUL1 = 8
RBF = 8
HID = 64
CUTOFF = 5.0
EPS = 1e-8
INV3 = float(1.0 / np.sqrt(np.float32(3.0)))
APATH = float(1.0 / math.sqrt(MUL0 + MUL1))
NCORE = 8
SUB = 128          # edges per subtile
SPS = 8            # subtiles per supertile
SUPE = SUB * SPS   # 1024 edges per supertile
NPW = 128          # node window per supertile

# P (product/scatter) column layout
C_P1 = 0            # 256: path1 (j16,i16) unreduced
C_P2 = 256          # 128: path2 (j16,i8) unreduced
C_M13 = 384         # 24: m1 path3 (c3,j8) reduced
C_M14 = 408         # 24: m1 path4 (c3,j8) reduced
C_EW = 432          # 1: edge weight (norm channel)
PCOLS = 433


def _host_prep(x, edge_src, edge_dst, edge_sh, edge_rbf, edge_len,
               w_r1, b_r1, w_r2, b_r2, w_g1, b_g1, w_g2, b_g2,
               Wm_s, Wm_v, Wu_s, Wu_v, Ws_s, Ws_v, res_scale):
    order = np.argsort(edge_dst, kind="stable")
    src_s = edge_src[order]
    dst_s = edge_dst[order]
    sh_s = edge_sh[order]
    rbf_s = edge_rbf[order]
    len_s = edge_len[order]

    deg = np.bincount(edge_dst, minlength=N).astype(np.int64)
    cum = np.concatenate([[0], np.cumsum(deg)])

    # node-aligned core splits with ~E/8 edges each
    bounds = [0]
    for k in range(1, NCORE):
        bounds.append(int(np.searchsorted(cum, k * E // NCORE)))
    bounds.append(N)

    cores = []
    for k in range(NCORE):
        n0, n1 = bounds[k], bounds[k + 1]
        sups = []  # (node_base, estart, ecnt)
        nb = n0
        while nb < n1:
            nn = nb
            cnt = 0
            while nn < n1 and nn - nb < NPW and cnt + deg[nn] <= SUPE:
                cnt += int(deg[nn])
                nn += 1
            sups.append((nb, int(cum[nb]), cnt))
            nb = nn
        cores.append((n0, n1, sups))

    nsup = max(len(c[2]) for c in cores)

    # host-transformed weights (shared across cores)
    w1p = np.zeros((9, 128), np.float32)
    w1p[:8, :64] = w_r1
    w1p[:8, 64:] = w_g1
    w1p[8, :64] = b_r1
    w1p[8, 64:] = b_g1

    w2p = np.zeros((65, 576), np.float32)
    wsrc = np.concatenate([w_r2, b_r2[None, :]], axis=0)  # [65, 576]
    # p1: our col j*16+i <- ref col i*16+j
    jj, ii = np.meshgrid(np.arange(16), np.arange(16), indexing="ij")
    w2p[:, (jj * 16 + ii).ravel()] = wsrc[:, (ii * 16 + jj).ravel()]
    # p2: our col 256+j*8+i <- ref col 256+i*16+j  (i in 8, j in 16)
    jj, ii = np.meshgrid(np.arange(16), np.arange(8), indexing="ij")
    w2p[:, (256 + jj * 8 + ii).ravel()] = wsrc[:, (256 + ii * 16 + jj).ravel()]
    # p3: our col 384+j*16+i <- ref col 384+i*8+j  (i in 16, j in 8)
    jj, ii = np.meshgrid(np.arange(8), np.arange(16), indexing="ij")
    w2p[:, (384 + jj * 16 + ii).ravel()] = wsrc[:, (384 + ii * 8 + jj).ravel()]
    # p4: our col 512+j*8+i <- ref col 512+i*8+j  (i in 8, j in 8)
    jj, ii = np.meshgrid(np.arange(8), np.arange(8), indexing="ij")
    w2p[:, (512 + jj * 8 + ii).ravel()] = wsrc[:, (512 + ii * 8 + jj).ravel()]

    wg2 = np.ascontiguousarray(w_g2.astype(np.float32))  # [64,1]

    s0 = 1.0 / math.sqrt(MUL0)
    s1 = 1.0 / math.sqrt(MUL1)
    wms = (Wm_s * s0).astype(np.float32)                      # [16,24]
    # agg1 layout rows (c,j); vv cols (c',j'); wmv[(c,j),(c',j')]=d_cc' Wm_v[j,j']/sqrt8
    wmv = np.zeros((24, 24), np.float32)
    wuv = np.zeros((24, 24), np.float32)
    wsv = np.zeros((24, 24), np.float32)
    for c in range(3):
        for j in range(8):
            for j2 in range(8):
                wmv[c * 8 + j, c * 8 + j2] = Wm_v[j, j2] * s1
                # vgT rows (c,j) -> out rows (j',c') ref order
                wuv[c * 8 + j, j2 * 3 + c] = Wu_v[j, j2] * s1
                # xvT rows are x's natural v order (j,c) j-major
                wsv[j * 3 + c, j2 * 3 + c] = Ws_v[j, j2] * s1
    rep = np.zeros((8, 24), np.float32)
    for c in range(3):
        for j in range(8):
            rep[j, c * 8 + j] = 1.0
    wus = (Wu_s * s0).astype(np.float32)
    wss = (Ws_s * s0).astype(np.float32)
    iotar = np.tile(np.arange(128, dtype=np.float32)[None, :], (128, 1))

    shared = dict(w1p=w1p, w2p=w2p, wg2=wg2, wms=wms, wmv=wmv, rep=rep,
                  wus=wus, wss=wss, wuv=wuv, wsv=wsv, iotar=iotar)

    in_maps = []
    metas = []
    for k in range(NCORE):
        n0, n1, sups = cores[k]
        ns = len(sups)
        idx = np.full((nsup, SUPE), -1, np.int64)
        base_arr = np.full((nsup,), n1, np.int64)
        span_arr = np.zeros((nsup,), np.int64)
        for si, (nb, es, cnt) in enumerate(sups):
            idx[si, :cnt] = np.arange(es, es + cnt)
            base_arr[si] = nb
            span_arr[si] = min(NPW, n1 - nb)
        mask = idx >= 0
        ic = np.clip(idx, 0, E - 1)

        feat = x[src_s[ic]]                       # [nsup, SUPE, 40]
        shp = np.where(mask[..., None], sh_s[ic], 0.0).astype(np.float32)
        lenp = np.where(mask & (len_s[ic] < CUTOFF), len_s[ic], 1.2 * CUTOFF).astype(np.float32)
        rbfp = np.where(mask[..., None], rbf_s[ic], 0.0).astype(np.float32)
        dstl = np.where(mask, dst_s[ic] - base_arr[:, None], 0).astype(np.float32)

        # swizzle [nsup, SUPE, F] -> [nsup, 128, SPS, F]
        def sw(a):
            f = a.shape[-1] if a.ndim == 3 else 1
            return np.ascontiguousarray(
                a.reshape(nsup, SPS, SUB, f).transpose(0, 2, 1, 3)
            ).astype(np.float32)

        scal = np.concatenate(
            [shp, lenp[..., None], dstl[..., None]], axis=-1)   # [nsup,SUPE,6]
        rbft = np.concatenate(
            [rbfp.reshape(nsup * 2, 512, 8).transpose(0, 2, 1),
             np.ones((nsup * 2, 1, 512), np.float32)], axis=1)  # [2nsup,9,512]

        nodes = np.clip(base_arr[:, None] + np.arange(NPW)[None, :], 0, N - 1)
        xown = x[nodes].astype(np.float32)                      # [nsup,128,40]

        m = dict(shared)
        m.update(feat=sw(feat), scal=sw(scal),
                 rbft=np.ascontiguousarray(rbft), xown=np.ascontiguousarray(xown))
        in_maps.append({k2: np.ascontiguousarray(v) for k2, v in m.items()})
        metas.append((n0, n1, base_arr, span_arr, ns))

    return in_maps, metas, nsup, float(b_g2[0]), float(res_scale)


def build_program(nsup, bg2, res, stage=99):
    import concourse.bacc as bacc
    nc = bacc.Bacc("TRN2", target_bir_lowering=False, debug=False,
                   num_devices=NCORE)

    feat_d = nc.dram_tensor("feat", [nsup, 128, SPS, 40], F32, kind="ExternalInput")
    scal_d = nc.dram_tensor("scal", [nsup, 128, SPS, 6], F32, kind="ExternalInput")
    rbft_d = nc.dram_tensor("rbft", [nsup * 2, 9, 512], F32, kind="ExternalInput")
    xown_d = nc.dram_tensor("xown", [nsup, 128, 40], F32, kind="ExternalInput")
    w1p_d = nc.dram_tensor("w1p", [9, 128], F32, kind="ExternalInput")
    w2p_d = nc.dram_tensor("w2p", [65, 576], F32, kind="ExternalInput")
    wg2_d = nc.dram_tensor("wg2", [64, 1], F32, kind="ExternalInput")
    wms_d = nc.dram_tensor("wms", [16, 24], F32, kind="ExternalInput")
    wmv_d = nc.dram_tensor("wmv", [24, 24], F32, kind="ExternalInput")
    rep_d = nc.dram_tensor("rep", [8, 24], F32, kind="ExternalInput")
    wus_d = nc.dram_tensor("wus", [16, 16], F32, kind="ExternalInput")
    wss_d = nc.dram_tensor("wss", [16, 16], F32, kind="ExternalInput")
    wuv_d = nc.dram_tensor("wuv", [24, 24], F32, kind="ExternalInput")
    wsv_d = nc.dram_tensor("wsv", [24, 24], F32, kind="ExternalInput")
    iota_d = nc.dram_tensor("iotar", [128, 128], F32, kind="ExternalInput")
    out_d = nc.dram_tensor("out", [nsup, 128, 40], F32, kind="ExternalOutput")

    with tile.TileContext(nc) as tc:
        with (
            tc.tile_pool(name="const", bufs=1) as cp,
            tc.tile_pool(name="io", bufs=3) as iop,
            tc.tile_pool(name="mid", bufs=2) as mp,
            tc.tile_pool(name="pp", bufs=2) as ppp,
            tc.tile_pool(name="nd", bufs=2) as ndp,
            tc.tile_pool(name="ps1", bufs=1, space="PSUM") as ps1,
            tc.tile_pool(name="ps2", bufs=1, space="PSUM") as ps2,
            tc.tile_pool(name="ps3", bufs=1, space="PSUM") as ps3,
            tc.tile_pool(name="ps4", bufs=1, space="PSUM") as ps4,
            tc.tile_pool(name="psh", bufs=2, space="PSUM") as psh,
            tc.tile_pool(name="psa", bufs=1, space="PSUM") as psa,
        ):
            # constants
            w1p = cp.tile([9, 128], F32, tag="w1p")
            w2p = cp.tile([65, 576], F32, tag="w2p")
            wg2 = cp.tile([128, 1], F32, tag="wg2")
            wms = cp.tile([16, 24], F32, tag="wms")
            wmv = cp.tile([24, 24], F32, tag="wmv")
            rep = cp.tile([8, 24], F32, tag="rep")
            wus = cp.tile([16, 16], F32, tag="wus")
            wss = cp.tile([16, 16], F32, tag="wss")
            wuv = cp.tile([24, 24], F32, tag="wuv")
            wsv = cp.tile([24, 24], F32, tag="wsv")
            iotar = cp.tile([128, 128], F32, tag="iotar")
            ident = cp.tile([128, 128], F32, tag="ident")
            for t, d in [(w1p, w1p_d), (w2p, w2p_d), (wms, wms_d),
                         (wmv, wmv_d), (rep, rep_d), (wus, wus_d), (wss, wss_d),
                         (wuv, wuv_d), (wsv, wsv_d), (iotar, iota_d)]:
                nc.sync.dma_start(out=t[:], in_=d[:])
            nc.sync.dma_start(out=wg2[64:128, :], in_=wg2_d[:])
            make_identity(nc, ident[:])
            cpi2 = cp.tile([128, 1], F32, tag="cpi2")
            cbg2 = cp.tile([128, 1], F32, tag="cbg2")
            nc.gpsimd.memset(cpi2[:], math.pi / 2)
            nc.gpsimd.memset(cbg2[:], bg2)

            for s in range(nsup):
                feats = iop.tile([128, SPS, 40], F32, tag="feat")
                scals = iop.tile([128, SPS, 6], F32, tag="scal")
                nc.sync.dma_start(out=feats[:], in_=feat_d[s])
                nc.sync.dma_start(out=scals[:], in_=scal_d[s])

                # ---- stage 1: hidden layers for both 512-groups ----
                hmsg = []
                hgate = []
                for g in range(2):
                    rbft = iop.tile([9, 512], F32, tag="rbft")
                    nc.sync.dma_start(out=rbft[:], in_=rbft_d[s * 2 + g])
                    hp = psh.tile([128, 512], F32, tag="h")
                    nc.tensor.matmul(out=hp[:], lhsT=w1p[:], rhs=rbft[:],
                                     start=True, stop=True)
                    hm = mp.tile([65, 512], F32, tag=f"hm{g}")
                    hg = mp.tile([128, 512], F32, tag=f"hg{g}")
                    nc.gpsimd.memset(hm[64:65, :], 1.0)
                    nc.scalar.activation(out=hm[0:64, :], in_=hp[0:64, :], func=AF.Silu)
                    nc.scalar.activation(out=hg[64:128, :], in_=hp[64:128, :], func=AF.Silu)
                    hmsg.append(hm)
                    hgate.append(hg)

                if stage < 2:
                    fin0 = ndp.tile([128, 40], F32, tag="fin")
                    nc.vector.tensor_copy(out=fin0[:], in_=feats[:, 0, :])
                    nc.sync.dma_start(out=out_d[s], in_=fin0[:])
                    continue

                # ---- per-edge scalar chain (supertile batch [128, SPS]) ----
                sh0 = scals[:, :, 0:1]
                sh1 = scals[:, :, 1:4]
                lenc = scals[:, :, 4:5]

                sq = mp.tile([128, SPS, 40], F32, tag="sq")
                nc.vector.tensor_tensor(out=sq[:], in0=feats[:], in1=feats[:], op=OP.mult)
                rms = mp.tile([128, 2, SPS], F32, tag="rms")
                nc.vector.reduce_sum(out=rms[:, 0, :], in_=sq[:, :, 0:16], axis=mybir.AxisListType.X)
                nc.vector.reduce_sum(out=rms[:, 1, :], in_=sq[:, :, 16:40], axis=mybir.AxisListType.X)
                nc.vector.tensor_scalar(out=rms[:, 0, :], in0=rms[:, 0, :],
                                        scalar1=1.0 / 16, scalar2=EPS, op0=OP.mult, op1=OP.add)
                nc.vector.tensor_scalar(out=rms[:, 1, :], in0=rms[:, 1, :],
                                        scalar1=1.0 / 8, scalar2=EPS, op0=OP.mult, op1=OP.add)
                inv2 = mp.tile([128, 2, SPS], F32, tag="inv2")
                nc.vector.reciprocal(out=inv2[:], in_=rms[:])
                nc.scalar.activation(out=inv2[:], in_=inv2[:], func=AF.Sqrt)
                invs = inv2[:, 0, :, None]   # [128, SPS, 1]
                invv = inv2[:, 1, :, None]

                st = mp.tile([128, SPS, 16], F32, tag="st")
                vt = mp.tile([128, SPS, 24], F32, tag="vt")
                nc.vector.tensor_tensor(out=st[:], in0=feats[:, :, 0:16],
                                        in1=invs.to_broadcast([128, SPS, 16]), op=OP.mult)
                nc.vector.tensor_tensor(out=vt[:], in0=feats[:, :, 16:40],
                                        in1=invv.to_broadcast([128, SPS, 24]), op=OP.mult)

                cosx = mp.tile([128, SPS], F32, tag="cosx")
                nc.scalar.activation(out=cosx[:], in_=lenc[:, :, 0], func=AF.Sin,
                                     scale=-math.pi / CUTOFF, bias=cpi2[:])
                msk = mp.tile([128, SPS], F32, tag="msk")
                nc.vector.tensor_scalar(out=msk[:], in0=lenc[:, :, 0], scalar1=CUTOFF,
                                        scalar2=None, op0=OP.is_lt)
                cwh = mp.tile([128, SPS], F32, tag="cwh")
                nc.vector.scalar_tensor_tensor(out=cwh[:], in0=cosx[:], scalar=1.0,
                                               in1=msk[:], op0=OP.add, op1=OP.mult)
                gw = mp.tile([128, SPS], F32, tag="gw")
                ew = mp.tile([128, SPS], F32, tag="ew")
                q = mp.tile([128, SPS], F32, tag="q")
                o4 = mp.tile([128, SPS], F32, tag="o4")
                g1s = mp.tile([128, SPS], F32, tag="g1s")
                o3c = mp.tile([128, SPS, 3], F32, tag="o3c")
                s1c = mp.tile([128, SPS, 3], F32, tag="s1c")
                g1 = mp.tile([128, SPS, 16], F32, tag="g1")
                g4 = mp.tile([128, SPS, 24], F32, tag="g4")
                a2p = mp.tile([128, SPS, 8, 3], F32, tag="a2p")
                g2 = mp.tile([128, SPS, 8], F32, tag="g2")

                if stage < 3:
                    fin0 = ndp.tile([128, 40], F32, tag="fin")
                    nc.vector.tensor_copy(out=fin0[:, 0:16], in_=g1[:, 0, :])
                    nc.vector.tensor_copy(out=fin0[:, 16:40], in_=g4[:, 0, :])
                    nc.sync.dma_start(out=out_d[s], in_=fin0[:])
                    continue
                # ---- per 512-group: TP weight matmuls + products ----
                agg = psa.tile([128, PCOLS], F32, tag="agg")
                for g in range(2):
                    p1 = ps1.tile([128, 4, 256], F32, tag="p1")
                    p2 = ps2.tile([128, 4, 128], F32, tag="p2")
                    p3 = ps3.tile([128, 4, 128], F32, tag="p3")
                    p4 = ps4.tile([128, 4, 66], F32, tag="p4")
                    hm = hmsg[g]
                    for tl in range(4):
                        lt = tl * 128
                        lhs = hm[:, lt:lt + 128]
                        nc.tensor.matmul(out=p1[:, tl, :], lhsT=lhs, rhs=w2p[:, 0:256],
                                         start=True, stop=True)
                        nc.tensor.matmul(out=p2[:, tl, :], lhsT=lhs, rhs=w2p[:, 256:384],
                                         start=True, stop=True)
                        nc.tensor.matmul(out=p3[:, tl, :], lhsT=lhs, rhs=w2p[:, 384:512],
                                         start=True, stop=True)
                        nc.tensor.matmul(out=p4[:, tl, 0:64], lhsT=lhs, rhs=w2p[:, 512:576],
                                         start=True, stop=True)
                        nc.tensor.matmul(out=p4[:, tl, 64:65],
                                         lhsT=hgate[g][64:128, lt:lt + 128],
                                         rhs=wg2[64:128, :], start=True, stop=True)

                    sl4 = slice(g * 4, g * 4 + 4)
                    nc.scalar.activation(out=gw[:, sl4], in_=p4[:, :, 65],
                                         func=AF.Sigmoid, bias=cbg2[:])
                    nc.vector.scalar_tensor_tensor(
                        out=ew[:, sl4], in0=gw[:, sl4], scalar=0.5,
                        in1=cwh[:, sl4], op0=OP.mult, op1=OP.mult)
                    nc.vector.tensor_scalar_mul(out=q[:, sl4], in0=ew[:, sl4],
                                                scalar1=APATH)
                    nc.vector.tensor_tensor(out=o4[:, sl4], in0=q[:, sl4],
                                            in1=sh0[:, sl4, 0], op=OP.mult)
                    nc.vector.tensor_tensor(out=g1s[:, sl4], in0=o4[:, sl4],
                                            in1=inv2[:, 0, sl4], op=OP.mult)
                    nc.vector.tensor_tensor(
                        out=o3c[:, sl4, :], in0=sh1[:, sl4, :],
                        in1=q[:, sl4, None].to_broadcast([128, 4, 3]), op=OP.mult)
                    nc.vector.tensor_scalar_mul(out=s1c[:, sl4, :], in0=o3c[:, sl4, :],
                                                scalar1=INV3)
                    nc.vector.tensor_tensor(
                        out=g1[:, sl4, :], in0=st[:, sl4, :],
                        in1=g1s[:, sl4, None].to_broadcast([128, 4, 16]), op=OP.mult)
                    nc.vector.tensor_tensor(
                        out=g4[:, sl4, :], in0=vt[:, sl4, :],
                        in1=o4[:, sl4, None].to_broadcast([128, 4, 24]), op=OP.mult)
                    nc.vector.tensor_tensor(
                        out=a2p[:, sl4], in0=vt[:, sl4, :].rearrange(
                            "p s (i c) -> p s i c", c=3),
                        in1=s1c[:, sl4, None, :].to_broadcast([128, 4, 8, 3]),
                        op=OP.mult)
                    nc.vector.reduce_sum(out=g2[:, sl4], in_=a2p[:, sl4],
                                         axis=mybir.AxisListType.X)
                    P = ppp.tile([128, 4, PCOLS], F32, tag="P")
                    # P1 = p1 * g1 (bcast over j)
                    nc.vector.tensor_tensor(
                        out=P[:, :, C_P1:C_P1 + 256].rearrange("p s (j i) -> p s j i", i=16),
                        in0=p1[:].rearrange("p s (j i) -> p s j i", i=16),
                        in1=g1[:, sl4, None, :].to_broadcast([128, 4, 16, 16]),
                        op=OP.mult)
                    nc.vector.tensor_tensor(
                        out=P[:, :, C_P2:C_P2 + 128].rearrange("p s (j i) -> p s j i", i=8),
                        in0=p2[:].rearrange("p s (j i) -> p s j i", i=8),
                        in1=g2[:, sl4, None, :].to_broadcast([128, 4, 16, 8]),
                        op=OP.mult)
                    # path3: contract i on DVE
                    t3 = ppp.tile([128, 4, 8, 16], F32, tag="t3")
                    nc.vector.tensor_tensor(
                        out=t3[:], in0=p3[:].rearrange("p s (j i) -> p s j i", i=16),
                        in1=st[:, sl4, None, :].to_broadcast([128, 4, 8, 16]), op=OP.mult)
                    u3 = ppp.tile([128, 4, 8], F32, tag="u3")
                    nc.vector.reduce_sum(out=u3[:], in_=t3[:], axis=mybir.AxisListType.X)
                    nc.vector.tensor_tensor(
                        out=P[:, :, C_M13:C_M13 + 24].rearrange("p s (c j) -> p s c j", j=8),
                        in0=u3[:, :, None, :].to_broadcast([128, 4, 3, 8]),
                        in1=o3c[:, sl4, :, None].to_broadcast([128, 4, 3, 8]), op=OP.mult)
                    # path4: products c-expanded then contract i
                    t4 = ppp.tile([128, 4, 3, 64], F32, tag="t4")
                    g4r = g4[:, sl4, :].rearrange("p s (i c) -> p s i c", c=3)
                    for c in range(3):
                        nc.vector.tensor_tensor(
                            out=t4[:, :, c, :].rearrange("p s (j i) -> p s j i", i=8),
                            in0=p4[:, :, 0:64].rearrange("p s (j i) -> p s j i", i=8),
                            in1=g4r[:, :, :, c][:, :, None, :].to_broadcast([128, 4, 8, 8]),
                            op=OP.mult)
                    nc.vector.reduce_sum(
                        out=P[:, :, C_M14:C_M14 + 24],
                        in_=t4[:].rearrange("p s c (j i) -> p s (c j) i", i=8),
                        axis=mybir.AxisListType.X)
                    nc.scalar.copy(out=P[:, :, C_EW], in_=ew[:, sl4])

                    # ---- scatter: selection matmul into agg window ----
                    for tl in range(4 if stage >= 4 else 0):
                        t = g * 4 + tl
                        sel = ppp.tile([128, 128], F32, tag="sel")
                        nc.vector.tensor_tensor(
                            out=sel[:], in0=scals[:, t, 5:6].to_broadcast([128, 128]),
                            in1=iotar[:], op=OP.is_equal)
                        nc.tensor.matmul(out=agg[:], lhsT=sel[:], rhs=P[:, tl, :],
                                         start=(t == 0), stop=(t == SPS - 1))

                if stage < 4:
                    fin0 = ndp.tile([128, 40], F32, tag="fin")
                    nc.vector.tensor_copy(out=fin0[:], in_=P[:, 0, 0:40])
                    nc.sync.dma_start(out=out_d[s], in_=fin0[:])
                    continue
                if stage < 5:
                    fin0 = ndp.tile([128, 40], F32, tag="fin")
                    nc.vector.tensor_copy(out=fin0[:], in_=agg[:, 0:40])
                    nc.sync.dma_start(out=out_d[s], in_=fin0[:])
                    continue
                # ---- node phase ----
                aggs = ndp.tile([128, 40], F32, tag="aggs")
                tmp16 = ndp.tile([128, 16], F32, tag="tmp16")
                nc.vector.reduce_sum(
                    out=aggs[:, 0:16],
                    in_=agg[:, C_P1:C_P1 + 256].rearrange("p (j i) -> p j i", i=16),
                    axis=mybir.AxisListType.X)
                nc.vector.reduce_sum(
                    out=tmp16[:],
                    in_=agg[:, C_P2:C_P2 + 128].rearrange("p (j i) -> p j i", i=8),
                    axis=mybir.AxisListType.X)
                nc.vector.tensor_tensor(out=aggs[:, 0:16], in0=aggs[:, 0:16],
                                        in1=tmp16[:], op=OP.add)
                nc.scalar.copy(out=aggs[:, 16:40], in_=agg[:, C_M13:C_M13 + 24])
                nc.vector.tensor_tensor(out=aggs[:, 16:40], in0=aggs[:, 16:40],
                                        in1=agg[:, C_M14:C_M14 + 24], op=OP.add)
                nrm = ndp.tile([128, 1], F32, tag="nrm")
                nc.vector.tensor_scalar_max(out=nrm[:], in0=agg[:, C_EW, None], scalar1=EPS)
                inv_n = ndp.tile([128, 1], F32, tag="invn")
                nc.vector.reciprocal(out=inv_n[:], in_=nrm[:])
                nc.vector.tensor_tensor(out=aggs[:], in0=aggs[:],
                                        in1=inv_n[:].to_broadcast([128, 40]), op=OP.mult)

                if stage < 6:
                    fin0 = ndp.tile([128, 40], F32, tag="fin")
                    nc.vector.tensor_copy(out=fin0[:], in_=aggs[:])
                    nc.sync.dma_start(out=out_d[s], in_=fin0[:])
                    continue
                # own-node irrep norm
                xo = iop.tile([128, 40], F32, tag="xo")
                nc.sync.dma_start(out=xo[:], in_=xown_d[s])
                xsq = ndp.tile([128, 40], F32, tag="xsq")
                nc.vector.tensor_tensor(out=xsq[:], in0=xo[:], in1=xo[:], op=OP.mult)
                xrm = ndp.tile([128, 2], F32, tag="xrm")
                nc.vector.reduce_sum(out=xrm[:, 0:1], in_=xsq[:, None, 0:16],
                                     axis=mybir.AxisListType.X)
                nc.vector.reduce_sum(out=xrm[:, 1:2], in_=xsq[:, None, 16:40],
                                     axis=mybir.AxisListType.X)
                nc.vector.tensor_scalar(out=xrm[:, 0:1], in0=xrm[:, 0:1],
                                        scalar1=1.0 / 16, scalar2=EPS, op0=OP.mult, op1=OP.add)
                nc.vector.tensor_scalar(out=xrm[:, 1:2], in0=xrm[:, 1:2],
                                        scalar1=1.0 / 8, scalar2=EPS, op0=OP.mult, op1=OP.add)
                nc.vector.reciprocal(out=xrm[:], in_=xrm[:])
                nc.scalar.activation(out=xrm[:], in_=xrm[:], func=AF.Sqrt)
                xns = ndp.tile([128, 40], F32, tag="xns")
                nc.vector.tensor_tensor(out=xns[:, 0:16], in0=xo[:, 0:16],
                                        in1=xrm[:, 0:1].to_broadcast([128, 16]), op=OP.mult)
                nc.vector.tensor_tensor(out=xns[:, 16:40], in0=xo[:, 16:40],
                                        in1=xrm[:, 1:2].to_broadcast([128, 24]), op=OP.mult)

                # transposes to feature-major (separate base-0 s/v tiles)
                def tposed(src_ap, rows, tag):
                    tp = psh.tile([rows, 128], F32, tag="h")
                    dst = ndp.tile([rows, 128], F32, tag=tag)
                    nc.tensor.transpose(out=tp[:], in_=src_ap, identity=ident[:])
                    nc.scalar.copy(out=dst[:], in_=tp[:])
                    return dst

                aggsT_s = tposed(aggs[:, 0:16], 16, "aTs")
                aggsT_v = tposed(aggs[:, 16:40], 24, "aTv")
                xnT_s = tposed(xns[:, 0:16], 16, "xnTs")
                xnT_v = tposed(xns[:, 16:40], 24, "xnTv")
                xoT_s = tposed(xo[:, 0:16], 16, "xoTs")
                xoT_v = tposed(xo[:, 16:40], 24, "xoTv")

                if stage < 7:
                    fin0 = ndp.tile([128, 40], F32, tag="fin")
                    nc.vector.tensor_copy(out=fin0[:], in_=xns[:])
                    nc.sync.dma_start(out=out_d[s], in_=fin0[:])
                    continue
                if stage < 8:
                    fin0 = ndp.tile([128, 40], F32, tag="fin")
                    nc.vector.tensor_copy(out=fin0[0:16, 0:16], in_=aggsT_s[:, 0:16])
                    nc.vector.tensor_copy(out=fin0[0:24, 16:40], in_=xoT_v[:, 0:24])
                    nc.sync.dma_start(out=out_d[s], in_=fin0[:])
                    continue
                scp = psh.tile([16, 128], F32, tag="h")
                nc.tensor.matmul(out=scp[:], lhsT=wms[:, 0:16], rhs=aggsT_s[:],
                                 start=True, stop=True)
                scalT = ndp.tile([16, 128], F32, tag="scalT")
                nc.scalar.activation(out=scalT[:], in_=scp[:], func=AF.Silu)
                gcp = psh.tile([8, 128], F32, tag="h")
                nc.tensor.matmul(out=gcp[:], lhsT=wms[:, 16:24], rhs=aggsT_s[:],
                                 start=True, stop=True)
                gT = ndp.tile([8, 128], F32, tag="gT")
                nc.scalar.activation(out=gT[:], in_=gcp[:], func=AF.Sigmoid)

                vvp = psh.tile([24, 128], F32, tag="h")
                nc.tensor.matmul(out=vvp[:], lhsT=wmv[:], rhs=aggsT_v[:],
                                 start=True, stop=True)
                vvT = ndp.tile([24, 128], F32, tag="vvT")
                nc.scalar.copy(out=vvT[:], in_=vvp[:])
                grp = psh.tile([24, 128], F32, tag="h")
                nc.tensor.matmul(out=grp[:], lhsT=rep[:], rhs=gT[:],
                                 start=True, stop=True)
                vgT = ndp.tile([24, 128], F32, tag="vgT")
                nc.vector.tensor_tensor(out=vgT[:], in0=vvT[:], in1=grp[:], op=OP.mult)

                if stage < 9:
                    fin0 = ndp.tile([128, 40], F32, tag="fin")
                    nc.vector.tensor_copy(out=fin0[0:24, 0:24], in_=vgT[:, 0:24])
                    nc.vector.tensor_copy(out=fin0[0:16, 24:40], in_=scalT[:, 0:16])
                    nc.sync.dma_start(out=out_d[s], in_=fin0[:])
                    continue
                outp_s = psh.tile([16, 128], F32, tag="h")
                nc.tensor.matmul(out=outp_s[:], lhsT=wus[:], rhs=scalT[:],
                                 start=True, stop=False)
                nc.tensor.matmul(out=outp_s[:], lhsT=wss[:], rhs=xnT_s[:],
                                 start=False, stop=True)
                outp_v = psh.tile([24, 128], F32, tag="h")
                nc.tensor.matmul(out=outp_v[:], lhsT=wuv[:], rhs=vgT[:],
                                 start=True, stop=False)
                nc.tensor.matmul(out=outp_v[:], lhsT=wsv[:], rhs=xnT_v[:],
                                 start=False, stop=True)
                finT_s = ndp.tile([16, 128], F32, tag="finTs")
                nc.vector.scalar_tensor_tensor(out=finT_s[:], in0=outp_s[:], scalar=res,
                                               in1=xoT_s[:], op0=OP.mult, op1=OP.add)
                finT_v = ndp.tile([24, 128], F32, tag="finTv")
                nc.vector.scalar_tensor_tensor(out=finT_v[:], in0=outp_v[:], scalar=res,
                                               in1=xoT_v[:], op0=OP.mult, op1=OP.add)
                fin = ndp.tile([128, 40], F32, tag="fin")
                fps = psh.tile([128, 16], F32, tag="h")
                nc.tensor.transpose(out=fps[:], in_=finT_s[:], identity=ident[0:16, 0:16])
                nc.scalar.copy(out=fin[:, 0:16], in_=fps[:])
                fpv = psh.tile([128, 24], F32, tag="h")
                nc.tensor.transpose(out=fpv[:], in_=finT_v[:], identity=ident[0:24, 0:24])
                nc.scalar.copy(out=fin[:, 16:40], in_=fpv[:])
                nc.sync.dma_start(out=out_d[s], in_=fin[:])

    nc.compile()
    return nc


_CACHE = {}


def kernel(**inputs):
    in_maps, metas, nsup, bg2, res = _host_prep(**inputs)
    key = (nsup, bg2, res)
    if key not in _CACHE:
        _CACHE[key] = build_program(nsup, bg2, res)
    nc = _CACHE[key]
    r = run_bass_kernel_spmd(nc, in_maps, list(range(NCORE)))
    out = np.zeros((N, 40), np.float32)
    for k in range(NCORE):
        n0, n1, base_arr, span_arr, ns = metas[k]
        ob = r.results[k]["out"]
        for si in range(ns):
            sp = int(span_arr[si])
            if sp > 0:
                b = int(base_arr[si])
                out[b:b + sp] = ob[si, :sp]
    return out



# revision 15
# speedup vs baseline: 2.1382x; 2.1382x over previous
"""Trainium2 Bass kernel for nn_EquivariantInteractionBlock.

Strategy (edge/graph parallel, 8 cores):
- Host: sort edges by dst; split into 8 node-aligned contiguous ranges with
  ~E/8 edges each. Per core, pack edges into supertiles: <=1024 edges
  covering a window of <=128 consecutive dst nodes. Host gathers raw x rows
  by edge_src, precomputes the cosine cutoff, builds one-hot scatter
  matrices, and pre-swizzles everything into DMA-friendly bf16 layouts.
- Device per supertile (all matmuls bf16, fp32 PSUM accumulate):
  * radial MLP hidden: h = silu(rbf@W1) via one matmul + one silu per
    512-edge group (msg+gate hidden together, feature-major)
  * per-edge TP weights + gate logit: per 128-edge subtile one stationary
    load (h slice) and two matmuls streaming 512+66 weight columns
  * sigmoid via tanh (same ACT table set as silu -> no table reloads),
    rsqrt for RMS norms via DVE Newton iteration
  * tensor-product products on VectorE, i-reductions for paths 1/2 ride
    the scatter matmul as extra columns
  * scatter-add via host-built one-hot selection matrices (bf16 matmul)
  * node phase: normalize, two packed PE transposes, small accumulating
    matmuls for msg/update/self linears, residual in fp32
- Each core owns a disjoint node range: no collectives; host concatenates
  per-core output rows.
"""

import math
import numpy as np
import ml_dtypes

import concourse.bass as bass
import concourse.mybir as mybir
import concourse.tile as tile
from concourse.bass_utils import run_bass_kernel_spmd
from concourse.masks import make_identity

F32 = mybir.dt.float32
BF16 = mybir.dt.bfloat16
AF = mybir.ActivationFunctionType
OP = mybir.AluOpType
BF = ml_dtypes.bfloat16

N = 50000
E = 400000
MUL0 = 16
MUL1 = 8
RBF = 8
HID = 64
CUTOFF = 5.0
EPS = 1e-8
INV3 = float(1.0 / np.sqrt(np.float32(3.0)))
APATH = float(1.0 / math.sqrt(MUL0 + MUL1))
NCORE = 8
SUB = 128          # edges per subtile
SPS = 8            # subtiles per supertile
SUPE = SUB * SPS   # 1024 edges per supertile
NPW = 128          # node window per supertile

# P (product/scatter) column layout
C_P12 = 0           # 384: (j16 x [i16 p1 | i8 p2]) unreduced
C_M13 = 384         # 24: m1 path3 (c3,j8) reduced
C_M14 = 408         # 24: m1 path4 (c3,j8) reduced
C_EW = 432          # 1: edge weight (norm channel)
PCOLS = 433


def _host_prep(x, edge_src, edge_dst, edge_sh, edge_rbf, edge_len,
               w_r1, b_r1, w_r2, b_r2, w_g1, b_g1, w_g2, b_g2,
               Wm_s, Wm_v, Wu_s, Wu_v, Ws_s, Ws_v, res_scale):
    order = np.argsort(edge_dst, kind="stable")
    src_s = edge_src[order]
    dst_s = edge_dst[order]
    sh_s = edge_sh[order]
    rbf_s = edge_rbf[order]
    len_s = edge_len[order]

    deg = np.bincount(edge_dst, minlength=N).astype(np.int64)
    cum = np.concatenate([[0], np.cumsum(deg)])

    bounds = [0]
    for k in range(1, NCORE):
        bounds.append(int(np.searchsorted(cum, k * E // NCORE)))
    bounds.append(N)

    cores = []
    for k in range(NCORE):
        n0, n1 = bounds[k], bounds[k + 1]
        sups = []  # (node_base, estart, ecnt)
        nb = n0
        while nb < n1:
            nn = nb
            cnt = 0
            while nn < n1 and nn - nb < NPW and cnt + deg[nn] <= SUPE:
                cnt += int(deg[nn])
                nn += 1
            sups.append((nb, int(cum[nb]), cnt))
            nb = nn
        cores.append((n0, n1, sups))

    nsup = max(len(c[2]) for c in cores)

    # ---- host-transformed weights (shared across cores) ----
    w1p = np.zeros((9, 128), np.float32)
    w1p[:8, :64] = w_r1
    w1p[:8, 64:] = w_g1
    w1p[8, :64] = b_r1
    w1p[8, 64:] = b_g1

    # w2e [128, 578]: rows 0:64 = w_r2 (reordered cols), rows 64:128 zero
    # except gate col. cols: 0:384 interleaved (j16 x [i16 p1 | i8 p2]),
    # 384:512 p3 (j8,i16), 512:576 p4 (j8,i8), 576 gate, 577 pad
    w2e = np.zeros((128, 578), np.float32)
    wsrc = w_r2.astype(np.float32)  # [64, 576]
    # p1: our col j*24+i <- ref col i*16+j (i16, j16)
    jj, ii = np.meshgrid(np.arange(16), np.arange(16), indexing="ij")
    w2e[:64, (jj * 24 + ii).ravel()] = wsrc[:, (ii * 16 + jj).ravel()]
    # p2: our col j*24+16+i <- ref col 256+i*16+j (i8, j16)
    jj, ii = np.meshgrid(np.arange(16), np.arange(8), indexing="ij")
    w2e[:64, (jj * 24 + 16 + ii).ravel()] = wsrc[:, (256 + ii * 16 + jj).ravel()]
    # p3: our col 384+j*16+i <- ref col 384+i*8+j (i16, j8)
    jj, ii = np.meshgrid(np.arange(8), np.arange(16), indexing="ij")
    w2e[:64, (384 + jj * 16 + ii).ravel()] = wsrc[:, (384 + ii * 8 + jj).ravel()]
    # p4: our col 512+j*8+i <- ref col 512+i*8+j (i8, j8)
    jj, ii = np.meshgrid(np.arange(8), np.arange(8), indexing="ij")
    w2e[:64, (512 + jj * 8 + ii).ravel()] = wsrc[:, (512 + ii * 8 + jj).ravel()]
    w2e[64:128, 576] = w_g2[:, 0]

    # b_r2 row, same column order (only used when b_r2 != 0)
    br2e = np.zeros((1, 578), np.float32)
    bsrc = b_r2.astype(np.float32)
    jj, ii = np.meshgrid(np.arange(16), np.arange(16), indexing="ij")
    br2e[0, (jj * 24 + ii).ravel()] = bsrc[(ii * 16 + jj).ravel()]
    jj, ii = np.meshgrid(np.arange(16), np.arange(8), indexing="ij")
    br2e[0, (jj * 24 + 16 + ii).ravel()] = bsrc[(256 + ii * 16 + jj).ravel()]
    jj, ii = np.meshgrid(np.arange(8), np.arange(16), indexing="ij")
    br2e[0, (384 + jj * 16 + ii).ravel()] = bsrc[(384 + ii * 8 + jj).ravel()]
    jj, ii = np.meshgrid(np.arange(8), np.arange(8), indexing="ij")
    br2e[0, (512 + jj * 8 + ii).ravel()] = bsrc[(512 + ii * 8 + jj).ravel()]
    use_bias = bool(np.any(b_r2 != 0.0))

    s0 = 1.0 / math.sqrt(MUL0)
    s1 = 1.0 / math.sqrt(MUL1)
    wms = (Wm_s * s0).astype(np.float32)                      # [16,24]
    wmv = np.zeros((24, 24), np.float32)
    wuv = np.zeros((24, 24), np.float32)
    wsv = np.zeros((24, 24), np.float32)
    for c in range(3):
        for j in range(8):
            for j2 in range(8):
                wmv[c * 8 + j, c * 8 + j2] = Wm_v[j, j2] * s1
                wuv[c * 8 + j, j2 * 3 + c] = Wu_v[j, j2] * s1
                wsv[j * 3 + c, j2 * 3 + c] = Ws_v[j, j2] * s1
    wus = (Wu_s * s0).astype(np.float32)
    wss = (Ws_s * s0).astype(np.float32)
    rep = np.zeros((8, 24), np.float32)
    for c in range(3):
        for j in range(8):
            rep[j, c * 8 + j] = 1.0

    shared = dict(
        w1p=w1p.astype(BF), w2e=w2e.astype(BF), br2e=br2e.astype(BF),
        wms=wms.astype(BF), wmv=wmv.astype(BF), rep=rep.astype(BF),
        wus=wus.astype(BF), wss=wss.astype(BF),
        wuv=wuv.astype(BF), wsv=wsv.astype(BF))

    in_maps = []
    metas = []
    for k in range(NCORE):
        n0, n1, sups = cores[k]
        ns = len(sups)
        idx = np.full((nsup, SUPE), -1, np.int64)
        base_arr = np.full((nsup,), n1, np.int64)
        span_arr = np.zeros((nsup,), np.int64)
        for si, (nb, es, cnt) in enumerate(sups):
            idx[si, :cnt] = np.arange(es, es + cnt)
            base_arr[si] = nb
            span_arr[si] = min(NPW, n1 - nb)
        mask = idx >= 0
        ic = np.clip(idx, 0, E - 1)

        feat = x[src_s[ic]]                                    # [nsup,SUPE,40]
        shp = sh_s[ic].astype(np.float32)
        lenp = len_s[ic].astype(np.float32)
        cw = 0.5 * (np.cos(np.pi * lenp / CUTOFF) + 1.0) * (lenp < CUTOFF)
        cwh = np.where(mask, 0.5 * cw, 0.0).astype(np.float32)  # [nsup,SUPE]
        rbfp = np.where(mask[..., None], rbf_s[ic], 0.0).astype(np.float32)
        dstl = np.where(mask, dst_s[ic] - base_arr[:, None], 0).astype(np.int64)

        # scal cols: sh0*APATH, sh1*APATH (3), cwh
        scal = np.concatenate(
            [APATH * shp[..., 0:1], APATH * shp[..., 1:4], cwh[..., None]],
            axis=-1).astype(np.float32)                         # [nsup,SUPE,5]

        # swizzle [nsup, SUPE, F] -> [nsup, 128, SPS, F]
        def sw(a, dt):
            f = a.shape[-1]
            return np.ascontiguousarray(
                a.reshape(nsup, SPS, SUB, f).transpose(0, 2, 1, 3)).astype(dt)

        rbft = np.concatenate(
            [rbfp.reshape(nsup * 2, 512, 8).transpose(0, 2, 1),
             np.ones((nsup * 2, 1, 512), np.float32)], axis=1)  # [2nsup,9,512]

        # one-hot scatter matrices [nsup, SPS, SUB, NPW] -> [nsup,128,SPS*128]
        sel = np.zeros((nsup, SPS, SUB, NPW), np.float32)
        si_i, e_i = np.nonzero(mask)
        t_i = e_i // SUB
        p_i = e_i % SUB
        sel[si_i, t_i, p_i, dstl[si_i, e_i]] = 1.0
        sel = np.ascontiguousarray(
            sel.transpose(0, 2, 1, 3).reshape(nsup, SUB, SPS * NPW)).astype(BF)

        nodes = np.clip(base_arr[:, None] + np.arange(NPW)[None, :], 0, N - 1)
        xown = x[nodes].astype(np.float32)                      # [nsup,128,40]

        m = dict(shared)
        m.update(feat=sw(feat, BF), scal=sw(scal, np.float32),
                 rbft=np.ascontiguousarray(rbft).astype(BF), sel=sel,
                 xown=np.ascontiguousarray(xown))
        in_maps.append(m)
        metas.append((n0, n1, base_arr, span_arr, ns))

    return in_maps, metas, nsup, float(b_g2[0]), float(res_scale), use_bias


def _newton_rsqrt(nc, y, r, rh, w, msq):
    """y = 1/sqrt(msq) (all args APs of equal shape; r/rh/w scratch).
    msq in [0.05, 10] roughly; r = 1/msq; y = sqrt(r) by Newton."""
    nc.vector.reciprocal(out=r, in_=msq)
    nc.vector.tensor_scalar_mul(out=rh, in0=r, scalar1=0.5)
    nc.vector.tensor_scalar(out=y, in0=r, scalar1=0.5, scalar2=0.5,
                            op0=OP.mult, op1=OP.add)
    for _ in range(3):
        nc.vector.reciprocal(out=w, in_=y)
        nc.vector.tensor_tensor(out=w, in0=w, in1=rh, op=OP.mult)
        nc.vector.scalar_tensor_tensor(out=y, in0=y, scalar=0.5,
                                       in1=w, op0=OP.mult, op1=OP.add)


def build_program(nsup, bg2, res, use_bias):
    import concourse.bacc as bacc
    nc = bacc.Bacc("TRN2", target_bir_lowering=False, debug=False,
                   num_devices=NCORE)

    feat_d = nc.dram_tensor("feat", [nsup, 128, SPS, 40], BF16, kind="ExternalInput")
    scal_d = nc.dram_tensor("scal", [nsup, 128, SPS, 5], F32, kind="ExternalInput")
    rbft_d = nc.dram_tensor("rbft", [nsup * 2, 9, 512], BF16, kind="ExternalInput")
    sel_d = nc.dram_tensor("sel", [nsup, 128, SPS * 128], BF16, kind="ExternalInput")
    xown_d = nc.dram_tensor("xown", [nsup, 128, 40], F32, kind="ExternalInput")
    w1p_d = nc.dram_tensor("w1p", [9, 128], BF16, kind="ExternalInput")
    w2e_d = nc.dram_tensor("w2e", [128, 578], BF16, kind="ExternalInput")
    br2e_d = nc.dram_tensor("br2e", [1, 578], BF16, kind="ExternalInput")
    wms_d = nc.dram_tensor("wms", [16, 24], BF16, kind="ExternalInput")
    wmv_d = nc.dram_tensor("wmv", [24, 24], BF16, kind="ExternalInput")
    rep_d = nc.dram_tensor("rep", [8, 24], BF16, kind="ExternalInput")
    wus_d = nc.dram_tensor("wus", [16, 16], BF16, kind="ExternalInput")
    wss_d = nc.dram_tensor("wss", [16, 16], BF16, kind="ExternalInput")
    wuv_d = nc.dram_tensor("wuv", [24, 24], BF16, kind="ExternalInput")
    wsv_d = nc.dram_tensor("wsv", [24, 24], BF16, kind="ExternalInput")
    out_d = nc.dram_tensor("out", [nsup, 128, 40], F32, kind="ExternalOutput")

    with tile.TileContext(nc) as tc:
        with (
            tc.tile_pool(name="const", bufs=1) as cp,
            tc.tile_pool(name="io", bufs=3) as iop,
            tc.tile_pool(name="mid", bufs=2) as mp,
            tc.tile_pool(name="pp", bufs=2) as ppp,
            tc.tile_pool(name="nd", bufs=2) as ndp,
            tc.tile_pool(name="psh", bufs=2, space="PSUM") as psH,
            tc.tile_pool(name="psw0", bufs=1, space="PSUM") as psW0,
            tc.tile_pool(name="psw1", bufs=1, space="PSUM") as psW1,
            tc.tile_pool(name="psa", bufs=1, space="PSUM") as psA,
        ):
            w1p = cp.tile([9, 128], BF16, tag="w1p")
            w2e = cp.tile([128, 578], BF16, tag="w2e")
            br2e = cp.tile([1, 578], BF16, tag="br2e")
            wms = cp.tile([16, 24], BF16, tag="wms")
            wmv = cp.tile([24, 24], BF16, tag="wmv")
            rep = cp.tile([8, 24], BF16, tag="rep")
            wus = cp.tile([16, 16], BF16, tag="wus")
            wss = cp.tile([16, 16], BF16, tag="wss")
            wuv = cp.tile([24, 24], BF16, tag="wuv")
            wsv = cp.tile([24, 24], BF16, tag="wsv")
            ident = cp.tile([128, 128], F32, tag="ident")
            for t, d in [(w1p, w1p_d), (w2e, w2e_d), (br2e, br2e_d),
                         (wms, wms_d), (wmv, wmv_d), (rep, rep_d),
                         (wus, wus_d), (wss, wss_d), (wuv, wuv_d),
                         (wsv, wsv_d)]:
                nc.sync.dma_start(out=t[:], in_=d[:])
            make_identity(nc, ident[:])
            cbg2h = cp.tile([128, 1], F32, tag="cbg2h")
            nc.gpsimd.memset(cbg2h[:], 0.5 * bg2)
            onesr = cp.tile([1, 128], BF16, tag="onesr")
            nc.gpsimd.memset(onesr[:], 1.0)

            for s in range(nsup):
                feats = iop.tile([128, SPS, 40], BF16, tag="feat")
                scals = iop.tile([128, SPS, 5], F32, tag="scal")
                selt = iop.tile([128, SPS, 128], BF16, tag="sel")
                xo = iop.tile([128, 40], F32, tag="xo")
                nc.sync.dma_start(out=feats[:], in_=feat_d[s])
                nc.sync.dma_start(out=scals[:], in_=scal_d[s])
                nc.sync.dma_start(out=selt[:], in_=sel_d[s])
                nc.sync.dma_start(out=xo[:], in_=xown_d[s])

                # ---- joint RMS factors: edge (s,v per 8 subtiles) + node ----
                sq = mp.tile([128, SPS, 40], F32, tag="sq")
                nc.vector.tensor_tensor(out=sq[:], in0=feats[:], in1=feats[:],
                                        op=OP.mult)
                xsq = mp.tile([128, 40], F32, tag="xsq")
                nc.vector.tensor_tensor(out=xsq[:], in0=xo[:], in1=xo[:],
                                        op=OP.mult)
                # rows: 0 edge-s, 1 edge-v, 2 node ([s, v] in cols 0:2)
                ms = mp.tile([128, 3, SPS], F32, tag="ms")
                nc.vector.memset(ms[:, 2, 2:SPS], 1.0)
                nc.vector.reduce_sum(out=ms[:, 0, :], in_=sq[:, :, 0:16],
                                     axis=mybir.AxisListType.X)
                nc.vector.reduce_sum(out=ms[:, 1, :], in_=sq[:, :, 16:40],
                                     axis=mybir.AxisListType.X)
                nc.vector.reduce_sum(out=ms[:, 2, 0:1], in_=xsq[:, None, 0:16],
                                     axis=mybir.AxisListType.X)
                nc.vector.reduce_sum(out=ms[:, 2, 1:2], in_=xsq[:, None, 16:40],
                                     axis=mybir.AxisListType.X)
                nc.vector.tensor_scalar(out=ms[:, 0, :], in0=ms[:, 0, :],
                                        scalar1=1.0 / 16, scalar2=EPS,
                                        op0=OP.mult, op1=OP.add)
                nc.vector.tensor_scalar(out=ms[:, 1, :], in0=ms[:, 1, :],
                                        scalar1=1.0 / 8, scalar2=EPS,
                                        op0=OP.mult, op1=OP.add)
                nc.vector.tensor_scalar(out=ms[:, 2, 0:1], in0=ms[:, 2, 0:1],
                                        scalar1=1.0 / 16, scalar2=EPS,
                                        op0=OP.mult, op1=OP.add)
                nc.vector.tensor_scalar(out=ms[:, 2, 1:2], in0=ms[:, 2, 1:2],
                                        scalar1=1.0 / 8, scalar2=EPS,
                                        op0=OP.mult, op1=OP.add)
                inv = mp.tile([128, 3, SPS], F32, tag="inv")
                nr = mp.tile([128, 3, SPS], F32, tag="nr")
                nrh = mp.tile([128, 3, SPS], F32, tag="nrh")
                nw = mp.tile([128, 3, SPS], F32, tag="nw")
                _newton_rsqrt(nc, inv[:], nr[:], nrh[:], nw[:], ms[:])
                # inv rows: 0 = edge-s, 1 = edge-v, 2 = [node-s, node-v, ...]

                st = mp.tile([128, SPS, 16], BF16, tag="st")
                vt = mp.tile([128, SPS, 24], BF16, tag="vt")
                nc.vector.tensor_tensor(
                    out=st[:], in0=feats[:, :, 0:16],
                    in1=inv[:, 0, :, None].to_broadcast([128, SPS, 16]),
                    op=OP.mult)
                nc.vector.tensor_tensor(
                    out=vt[:], in0=feats[:, :, 16:40],
                    in1=inv[:, 1, :, None].to_broadcast([128, SPS, 24]),
                    op=OP.mult)

                # ---- radial MLP hidden for both groups ----
                hsil = []
                for g in range(2):
                    rbft = iop.tile([9, 512], BF16, tag="rbft")
                    nc.sync.dma_start(out=rbft[:], in_=rbft_d[s * 2 + g])
                    hp = psH.tile([128, 512], F32, tag="h")
                    nc.tensor.matmul(out=hp[:], lhsT=w1p[:], rhs=rbft[:],
                                     start=True, stop=True)
                    hs = mp.tile([128, 512], BF16, tag=f"hs{g}")
                    nc.scalar.activation(out=hs[:], in_=hp[:], func=AF.Silu)
                    hsil.append(hs)

                agg = psA.tile([128, PCOLS], F32, tag="agg")
                for g in range(2):
                    sl4 = slice(g * 4, g * 4 + 4)
                    pw0 = psW0.tile([128, 4, 512], F32, tag="pw0")
                    pw1 = psW1.tile([128, 4, 66], F32, tag="pw1")
                    for tl in range(4):
                        lhs = hsil[g][:, tl * 128:(tl + 1) * 128]
                        if use_bias:
                            nc.tensor.matmul(out=pw0[:, tl, :], lhsT=onesr[:],
                                             rhs=br2e[:, 0:512],
                                             start=True, stop=False)
                            nc.tensor.matmul(out=pw1[:, tl, :], lhsT=onesr[:],
                                             rhs=br2e[:, 512:578],
                                             start=True, stop=False)
                        nc.tensor.matmul(out=pw0[:, tl, :], lhsT=lhs,
                                         rhs=w2e[:, 0:512],
                                         start=not use_bias, stop=True)
                        nc.tensor.matmul(out=pw1[:, tl, :], lhsT=lhs,
                                         rhs=w2e[:, 512:578],
                                         start=not use_bias, stop=True)

                    # ---- per-edge scalar chain (group batch [128,4]) ----
                    gwt = mp.tile([128, 4], F32, tag="gwt")
                    nc.scalar.activation(out=gwt[:], in_=pw1[:, :, 64],
                                         func=AF.Tanh, scale=0.5, bias=cbg2h[:])
                    ew = mp.tile([128, 4], F32, tag="ew")
                    nc.vector.scalar_tensor_tensor(
                        out=ew[:], in0=gwt[:], scalar=1.0,
                        in1=scals[:, sl4, 4], op0=OP.add, op1=OP.mult)
                    o4 = mp.tile([128, 4], BF16, tag="o4")
                    nc.vector.tensor_tensor(out=o4[:], in0=ew[:],
                                            in1=scals[:, sl4, 0], op=OP.mult)
                    o3c = mp.tile([128, 4, 3], BF16, tag="o3c")
                    nc.vector.tensor_tensor(
                        out=o3c[:], in0=scals[:, sl4, 1:4],
                        in1=ew[:, :, None].to_broadcast([128, 4, 3]), op=OP.mult)
                    s1c = mp.tile([128, 4, 3], BF16, tag="s1c")
                    nc.vector.tensor_scalar_mul(out=s1c[:], in0=o3c[:],
                                                scalar1=INV3)
                    i4 = mp.tile([128, 4], BF16, tag="i4")
                    nc.vector.tensor_tensor(out=i4[:], in0=o4[:],
                                            in1=inv[:, 0, sl4], op=OP.mult)
                    g1 = mp.tile([128, 4, 16], BF16, tag="g1")
                    nc.vector.tensor_tensor(
                        out=g1[:], in0=feats[:, sl4, 0:16],
                        in1=i4[:, :, None].to_broadcast([128, 4, 16]), op=OP.mult)
                    g4 = mp.tile([128, 4, 24], BF16, tag="g4")
                    nc.vector.tensor_tensor(
                        out=g4[:], in0=vt[:, sl4, :],
                        in1=o4[:, :, None].to_broadcast([128, 4, 24]), op=OP.mult)
                    a2 = ppp.tile([128, 4, 8, 3], BF16, tag="a2")
                    nc.vector.tensor_tensor(
                        out=a2[:],
                        in0=vt[:, sl4, :].rearrange("p s (i c) -> p s i c", c=3),
                        in1=s1c[:, :, None, :].to_broadcast([128, 4, 8, 3]),
                        op=OP.mult)
                    g2 = mp.tile([128, 4, 8], F32, tag="g2")
                    nc.vector.reduce_sum(out=g2[:], in_=a2[:],
                                         axis=mybir.AxisListType.X)

                    # ---- products ----
                    P = ppp.tile([128, 4, PCOLS], BF16, tag="P")
                    p12 = pw0[:, :, 0:384].rearrange("p s (j i) -> p s j i", i=24)
                    nc.vector.tensor_tensor(
                        out=P[:, :, 0:384].rearrange("p s (j i) -> p s j i", i=24)[:, :, :, 0:16],
                        in0=p12[:, :, :, 0:16],
                        in1=g1[:, :, None, :].to_broadcast([128, 4, 16, 16]),
                        op=OP.mult)
                    nc.vector.tensor_tensor(
                        out=P[:, :, 0:384].rearrange("p s (j i) -> p s j i", i=24)[:, :, :, 16:24],
                        in0=p12[:, :, :, 16:24],
                        in1=g2[:, :, None, :].to_broadcast([128, 4, 16, 8]),
                        op=OP.mult)
                    t3 = ppp.tile([128, 4, 8, 16], BF16, tag="t3")
                    nc.vector.tensor_tensor(
                        out=t3[:],
                        in0=pw0[:, :, 384:512].rearrange("p s (j i) -> p s j i", i=16),
                        in1=st[:, sl4, None, :].to_broadcast([128, 4, 8, 16]),
                        op=OP.mult)
                    u3 = mp.tile([128, 4, 8], F32, tag="u3")
                    nc.vector.reduce_sum(out=u3[:], in_=t3[:],
                                         axis=mybir.AxisListType.X)
                    nc.vector.tensor_tensor(
                        out=P[:, :, C_M13:C_M13 + 24].rearrange(
                            "p s (c j) -> p s c j", j=8),
                        in0=u3[:, :, None, :].to_broadcast([128, 4, 3, 8]),
                        in1=o3c[:, :, :, None].to_broadcast([128, 4, 3, 8]),
                        op=OP.mult)
                    t4 = ppp.tile([128, 4, 3, 64], BF16, tag="t4")
                    g4r = g4[:].rearrange("p s (i c) -> p s i c", c=3)
                    for c in range(3):
                        nc.vector.tensor_tensor(
                            out=t4[:, :, c, :].rearrange(
                                "p s (j i) -> p s j i", i=8),
                            in0=pw1[:, :, 0:64].rearrange(
                                "p s (j i) -> p s j i", i=8),
                            in1=g4r[:, :, :, c][:, :, None, :].to_broadcast(
                                [128, 4, 8, 8]),
                            op=OP.mult)
                    with nc.allow_low_precision(reason="8-term bf16 sum"):
                        nc.vector.reduce_sum(
                            out=P[:, :, C_M14:C_M14 + 24],
                            in_=t4[:].rearrange("p s c (j i) -> p s (c j) i", i=8),
                            axis=mybir.AxisListType.X)
                    nc.vector.tensor_copy(out=P[:, :, C_EW], in_=ew[:])

                    # ---- scatter via one-hot matmuls ----
                    for tl in range(4):
                        t = g * 4 + tl
                        nc.tensor.matmul(out=agg[:], lhsT=selt[:, t, :],
                                         rhs=P[:, tl, :],
                                         start=(t == 0), stop=(t == SPS - 1))

                # ---- node phase ----
                m0 = ndp.tile([128, 16], F32, tag="m0")
                nc.vector.reduce_sum(
                    out=m0[:],
                    in_=agg[:, 0:384].rearrange("p (j i) -> p j i", i=24),
                    axis=mybir.AxisListType.X)
                v1 = ndp.tile([128, 24], F32, tag="v1")
                nc.vector.reduce_sum(
                    out=v1[:],
                    in_=agg[:, C_M13:C_M13 + 48].rearrange(
                        "p (a b) -> p b a", b=24),
                    axis=mybir.AxisListType.X)
                nrm = ndp.tile([128, 1], F32, tag="nrm")
                nc.vector.tensor_scalar_max(out=nrm[:], in0=agg[:, C_EW, None],
                                            scalar1=EPS)
                rinv = ndp.tile([128, 1], F32, tag="rinv")
                nc.vector.reciprocal(out=rinv[:], in_=nrm[:])

                cat_s = ndp.tile([128, 32], F32, tag="cat_s")
                cat_v = ndp.tile([128, 48], F32, tag="cat_v")
                nc.vector.tensor_tensor(
                    out=cat_s[:, 0:16], in0=m0[:],
                    in1=rinv[:].to_broadcast([128, 16]), op=OP.mult)
                nc.vector.tensor_tensor(
                    out=cat_v[:, 0:24], in0=v1[:],
                    in1=rinv[:].to_broadcast([128, 24]), op=OP.mult)
                nc.vector.tensor_tensor(
                    out=cat_s[:, 16:32], in0=xo[:, 0:16],
                    in1=inv[:, 2, 0:1].to_broadcast([128, 16]), op=OP.mult)
                nc.vector.tensor_tensor(
                    out=cat_v[:, 24:48], in0=xo[:, 16:40],
                    in1=inv[:, 2, 1:2].to_broadcast([128, 24]), op=OP.mult)

                def tposed(src_ap, rows, tag):
                    tp = psH.tile([rows, 128], F32, tag="h")
                    dst = ndp.tile([rows, 128], BF16, tag=tag)
                    nc.tensor.transpose(out=tp[:], in_=src_ap, identity=ident[:])
                    nc.scalar.copy(out=dst[:], in_=tp[:])
                    return dst

                aggT_s = tposed(cat_s[:, 0:16], 16, "aTs")
                xnT_s = tposed(cat_s[:, 16:32], 16, "xnTs")
                aggT_v = tposed(cat_v[:, 0:24], 24, "aTv")
                xnT_v = tposed(cat_v[:, 24:48], 24, "xnTv")

                scp = psH.tile([16, 128], F32, tag="h")
                nc.tensor.matmul(out=scp[:], lhsT=wms[:, 0:16], rhs=aggT_s[:],
                                 start=True, stop=True)
                scalT = ndp.tile([16, 128], BF16, tag="scalT")
                nc.scalar.activation(out=scalT[:], in_=scp[:], func=AF.Silu)
                gcp = psH.tile([8, 128], F32, tag="h")
                nc.tensor.matmul(out=gcp[:], lhsT=wms[:, 16:24], rhs=aggT_s[:],
                                 start=True, stop=True)
                gT = ndp.tile([8, 128], BF16, tag="gT")
                nc.scalar.activation(out=gT[:], in_=gcp[:], func=AF.Tanh,
                                     scale=0.5)
                nc.vector.tensor_scalar(out=gT[:], in0=gT[:], scalar1=0.5,
                                        scalar2=0.5, op0=OP.mult, op1=OP.add)

                vvp = psH.tile([24, 128], F32, tag="h")
                nc.tensor.matmul(out=vvp[:], lhsT=wmv[:], rhs=aggT_v[:],
                                 start=True, stop=True)
                grp = psH.tile([24, 128], F32, tag="h")
                nc.tensor.matmul(out=grp[:], lhsT=rep[:], rhs=gT[:],
                                 start=True, stop=True)
                vvc = ndp.tile([24, 128], BF16, tag="vvc")
                nc.scalar.copy(out=vvc[:], in_=vvp[:])
                vgT = ndp.tile([24, 128], BF16, tag="vgT")
                nc.vector.tensor_tensor(out=vgT[:], in0=vvc[:], in1=grp[:],
                                        op=OP.mult)

                osp = psH.tile([16, 128], F32, tag="h")
                nc.tensor.matmul(out=osp[:], lhsT=wus[:], rhs=scalT[:],
                                 start=True, stop=False)
                nc.tensor.matmul(out=osp[:], lhsT=wss[:], rhs=xnT_s[:],
                                 start=False, stop=True)
                ovp = psH.tile([24, 128], F32, tag="h")
                nc.tensor.matmul(out=ovp[:], lhsT=wuv[:], rhs=vgT[:],
                                 start=True, stop=False)
                nc.tensor.matmul(out=ovp[:], lhsT=wsv[:], rhs=xnT_v[:],
                                 start=False, stop=True)

                fTs = ndp.tile([16, 128], F32, tag="fTs")
                nc.vector.tensor_scalar_mul(out=fTs[:], in0=osp[:], scalar1=res)
                fTv = ndp.tile([24, 128], F32, tag="fTv")
                nc.vector.tensor_scalar_mul(out=fTv[:], in0=ovp[:], scalar1=res)
                fps = psH.tile([128, 16], F32, tag="h")
                nc.tensor.transpose(out=fps[:], in_=fTs[:],
                                    identity=ident[0:16, 0:16])
                fpv = psH.tile([128, 24], F32, tag="h")
                nc.tensor.transpose(out=fpv[:], in_=fTv[:],
                                    identity=ident[0:24, 0:24])
                outt = ndp.tile([128, 40], F32, tag="outt")
                nc.vector.tensor_tensor(out=outt[:, 0:16], in0=xo[:, 0:16],
                                        in1=fps[:], op=OP.add)
                nc.vector.tensor_tensor(out=outt[:, 16:40], in0=xo[:, 16:40],
                                        in1=fpv[:], op=OP.add)
                nc.sync.dma_start(out=out_d[s], in_=outt[:])

    nc.compile()
    return nc


_CACHE = {}


def kernel(**inputs):
    in_maps, metas, nsup, bg2, res, use_bias = _host_prep(**inputs)
    key = (nsup, bg2, res, use_bias)
    if key not in _CACHE:
        _CACHE[key] = build_program(nsup, bg2, res, use_bias)
    nc = _CACHE[key]
    r = run_bass_kernel_spmd(nc, in_maps, list(range(NCORE)))
    out = np.zeros((N, 40), np.float32)
    for k in range(NCORE):
        n0, n1, base_arr, span_arr, ns = metas[k]
        ob = r.results[k]["out"]
        for si in range(ns):
            sp = int(span_arr[si])
            if sp > 0:
                b = int(base_arr[si])
                out[b:b + sp] = ob[si, :sp]
    return out


# revision 28
# speedup vs baseline: 2.1984x; 1.0282x over previous
"""Trainium2 Bass kernel for nn_EquivariantInteractionBlock.

Strategy (edge/graph parallel, 8 cores):
- Host: sort edges by dst; split into 8 node-aligned contiguous ranges with
  ~E/8 edges each. Per core, pack edges into supertiles: <=1024 edges
  covering a window of <=128 consecutive dst nodes. Host gathers raw x rows
  by edge_src, precomputes the cosine cutoff, builds one-hot scatter
  matrices, and pre-swizzles everything into DMA-friendly bf16 layouts.
- Device per supertile (all matmuls bf16, fp32 PSUM accumulate):
  * radial MLP hidden: h = silu(rbf@W1) via one matmul + one silu per
    512-edge group (msg+gate hidden together, feature-major)
  * per-edge TP weights + gate logit: per 128-edge subtile one stationary
    load (h slice) and two matmuls streaming 512+66 weight columns
  * sigmoid via tanh (same ACT table set as silu -> no table reloads),
    rsqrt for RMS norms via DVE Newton iteration
  * tensor-product products on VectorE, i-reductions for paths 1/2 ride
    the scatter matmul as extra columns
  * scatter-add via host-built one-hot selection matrices (bf16 matmul)
  * node phase: normalize, two packed PE transposes, small accumulating
    matmuls for msg/update/self linears, residual in fp32
- Each core owns a disjoint node range: no collectives; host concatenates
  per-core output rows.
"""

import math
import numpy as np
import ml_dtypes

import concourse.bass as bass
import concourse.mybir as mybir
import concourse.tile as tile
from concourse.bass_utils import run_bass_kernel_spmd
from concourse.masks import make_identity

F32 = mybir.dt.float32
BF16 = mybir.dt.bfloat16
AF = mybir.ActivationFunctionType
OP = mybir.AluOpType
BF = ml_dtypes.bfloat16

N = 50000
E = 400000
MUL0 = 16
MUL1 = 8
RBF = 8
HID = 64
CUTOFF = 5.0
EPS = 1e-8
INV3 = float(1.0 / np.sqrt(np.float32(3.0)))
APATH = float(1.0 / math.sqrt(MUL0 + MUL1))
NCORE = 8
SUB = 128          # edges per subtile
SPS = 8            # subtiles per supertile
SUPE = SUB * SPS   # 1024 edges per supertile
NPW = 128          # node window per supertile

# P (product/scatter) column layout
C_P12 = 0           # 384: (j16 x [i16 p1 | i8 p2]) unreduced
C_M13 = 384         # 24: m1 path3 (c3,j8) reduced
C_M14 = 408         # 24: m1 path4 (c3,j8) reduced
C_EW = 432          # 1: edge weight (norm channel)
PCOLS = 433

# irrep-norm scale folding: device computes rsqrt(sum of squares); the
# 1/sqrt(mean) = sqrt(16) (s) / sqrt(8) (v) factors are folded into weights
FS = 4.0
FV = float(np.sqrt(8.0))


def _host_prep(x, edge_src, edge_dst, edge_sh, edge_rbf, edge_len,
               w_r1, b_r1, w_r2, b_r2, w_g1, b_g1, w_g2, b_g2,
               Wm_s, Wm_v, Wu_s, Wu_v, Ws_s, Ws_v, res_scale):
    order = np.argsort(edge_dst, kind="stable")
    src_s = edge_src[order]
    dst_s = edge_dst[order]
    sh_s = edge_sh[order]
    rbf_s = edge_rbf[order]
    len_s = edge_len[order]

    deg = np.bincount(edge_dst, minlength=N).astype(np.int64)
    cum = np.concatenate([[0], np.cumsum(deg)])

    bounds = [0]
    for k in range(1, NCORE):
        bounds.append(int(np.searchsorted(cum, k * E // NCORE)))
    bounds.append(N)

    cores = []
    for k in range(NCORE):
        n0, n1 = bounds[k], bounds[k + 1]
        sups = []  # (node_base, estart, ecnt)
        nb = n0
        while nb < n1:
            nn = nb
            cnt = 0
            while nn < n1 and nn - nb < NPW and cnt + deg[nn] <= SUPE:
                cnt += int(deg[nn])
                nn += 1
            sups.append((nb, int(cum[nb]), cnt))
            nb = nn
        cores.append((n0, n1, sups))

    nsup = max(len(c[2]) for c in cores)

    # ---- host-transformed weights (shared across cores) ----
    w1p = np.zeros((9, 128), np.float32)
    w1p[:8, :64] = w_r1
    w1p[:8, 64:] = w_g1
    w1p[8, :64] = b_r1
    w1p[8, 64:] = b_g1

    # w2e [128, 578]: rows 0:64 = w_r2 (reordered cols), rows 64:128 zero
    # except gate col. cols: 0:384 interleaved (j16 x [i16 p1 | i8 p2]),
    # 384:512 p3 (j8,i16), 512:576 p4 (j8,i8), 576 gate, 577 pad
    # block scales fold the 1/sqrt(mean)-vs-rsqrt(sum) factors: paths
    # contracting normalized s get FS, normalized v get FV
    w2e = np.zeros((128, 578), np.float32)
    wsrc = w_r2.astype(np.float32)  # [64, 576]
    # p1: our col j*24+i <- ref col i*16+j (i16, j16)
    jj, ii = np.meshgrid(np.arange(16), np.arange(16), indexing="ij")
    w2e[:64, (jj * 24 + ii).ravel()] = FS * wsrc[:, (ii * 16 + jj).ravel()]
    # p2: our col j*24+16+i <- ref col 256+i*16+j (i8, j16)
    jj, ii = np.meshgrid(np.arange(16), np.arange(8), indexing="ij")
    w2e[:64, (jj * 24 + 16 + ii).ravel()] = FV * wsrc[:, (256 + ii * 16 + jj).ravel()]
    # p3: our col 384+j*16+i <- ref col 384+i*8+j (i16, j8)
    jj, ii = np.meshgrid(np.arange(8), np.arange(16), indexing="ij")
    w2e[:64, (384 + jj * 16 + ii).ravel()] = FS * wsrc[:, (384 + ii * 8 + jj).ravel()]
    # p4: our col 512+j*8+i <- ref col 512+i*8+j (i8, j8)
    jj, ii = np.meshgrid(np.arange(8), np.arange(8), indexing="ij")
    w2e[:64, (512 + jj * 8 + ii).ravel()] = FV * wsrc[:, (512 + ii * 8 + jj).ravel()]
    w2e[64:128, 576] = w_g2[:, 0]

    # b_r2 row, same column order and scales (only used when b_r2 != 0)
    br2e = np.zeros((1, 578), np.float32)
    bsrc = b_r2.astype(np.float32)
    jj, ii = np.meshgrid(np.arange(16), np.arange(16), indexing="ij")
    br2e[0, (jj * 24 + ii).ravel()] = FS * bsrc[(ii * 16 + jj).ravel()]
    jj, ii = np.meshgrid(np.arange(16), np.arange(8), indexing="ij")
    br2e[0, (jj * 24 + 16 + ii).ravel()] = FV * bsrc[(256 + ii * 16 + jj).ravel()]
    jj, ii = np.meshgrid(np.arange(8), np.arange(16), indexing="ij")
    br2e[0, (384 + jj * 16 + ii).ravel()] = FS * bsrc[(384 + ii * 8 + jj).ravel()]
    jj, ii = np.meshgrid(np.arange(8), np.arange(8), indexing="ij")
    br2e[0, (512 + jj * 8 + ii).ravel()] = FV * bsrc[(512 + ii * 8 + jj).ravel()]
    use_bias = bool(np.any(b_r2 != 0.0))

    s0 = 1.0 / math.sqrt(MUL0)
    s1 = 1.0 / math.sqrt(MUL1)
    wms = (Wm_s * s0).astype(np.float32)                      # [16,24]
    wmv = np.zeros((24, 24), np.float32)
    wuv = np.zeros((24, 24), np.float32)
    wsv = np.zeros((24, 24), np.float32)
    for c in range(3):
        for j in range(8):
            for j2 in range(8):
                wmv[c * 8 + j, c * 8 + j2] = Wm_v[j, j2] * s1
                wuv[c * 8 + j, j2 * 3 + c] = Wu_v[j, j2] * s1
                wsv[j * 3 + c, j2 * 3 + c] = Ws_v[j, j2] * s1 * FV
    wus = (Wu_s * s0).astype(np.float32)
    wss = (Ws_s * s0 * FS).astype(np.float32)
    rep = np.zeros((8, 24), np.float32)
    for c in range(3):
        for j in range(8):
            rep[j, c * 8 + j] = 1.0

    shared = dict(
        w1p=w1p.astype(BF), w2e=w2e.astype(BF), br2e=br2e.astype(BF),
        wms=wms.astype(BF), wmv=wmv.astype(BF), rep=rep.astype(BF),
        wus=wus.astype(BF), wss=wss.astype(BF),
        wuv=wuv.astype(BF), wsv=wsv.astype(BF))

    in_maps = []
    metas = []
    for k in range(NCORE):
        n0, n1, sups = cores[k]
        ns = len(sups)
        idx = np.full((nsup, SUPE), -1, np.int64)
        base_arr = np.full((nsup,), n1, np.int64)
        span_arr = np.zeros((nsup,), np.int64)
        for si, (nb, es, cnt) in enumerate(sups):
            idx[si, :cnt] = np.arange(es, es + cnt)
            base_arr[si] = nb
            span_arr[si] = min(NPW, n1 - nb)
        mask = idx >= 0
        ic = np.clip(idx, 0, E - 1)

        feat = x[src_s[ic]]                                    # [nsup,SUPE,40]
        shp = sh_s[ic].astype(np.float32)
        lenp = len_s[ic].astype(np.float32)
        cw = 0.5 * (np.cos(np.pi * lenp / CUTOFF) + 1.0) * (lenp < CUTOFF)
        cwh = np.where(mask, 0.5 * cw, 0.0).astype(np.float32)  # [nsup,SUPE]
        rbfp = np.where(mask[..., None], rbf_s[ic], 0.0).astype(np.float32)
        dstl = np.where(mask, dst_s[ic] - base_arr[:, None], 0).astype(np.int64)

        # scal cols: sh0*APATH, sh1*APATH (3), sh1*APATH*INV3 (3), cwh
        scal = np.concatenate(
            [APATH * shp[..., 0:1], APATH * shp[..., 1:4],
             (APATH * INV3) * shp[..., 1:4], cwh[..., None]],
            axis=-1).astype(np.float32)                         # [nsup,SUPE,8]

        # swizzle [nsup, SUPE, F] -> [nsup, 128, SPS, F]
        def sw(a, dt):
            f = a.shape[-1]
            return np.ascontiguousarray(
                a.reshape(nsup, SPS, SUB, f).transpose(0, 2, 1, 3)).astype(dt)

        rbft = np.concatenate(
            [rbfp.reshape(nsup * 2, 512, 8).transpose(0, 2, 1),
             np.ones((nsup * 2, 1, 512), np.float32)], axis=1)  # [2nsup,9,512]

        # one-hot scatter matrices [nsup, SPS, SUB, NPW] -> [nsup,128,SPS*128]
        sel = np.zeros((nsup, SPS, SUB, NPW), np.float32)
        si_i, e_i = np.nonzero(mask)
        t_i = e_i // SUB
        p_i = e_i % SUB
        sel[si_i, t_i, p_i, dstl[si_i, e_i]] = 1.0
        sel = np.ascontiguousarray(
            sel.transpose(0, 2, 1, 3).reshape(nsup, SUB, SPS * NPW)).astype(BF)

        nodes = np.clip(base_arr[:, None] + np.arange(NPW)[None, :], 0, N - 1)
        xown = x[nodes].astype(np.float32)                      # [nsup,128,40]

        m = dict(shared)
        m.update(feat=sw(feat, BF), scal=sw(scal, np.float32),
                 rbft=np.ascontiguousarray(rbft).astype(BF), sel=sel,
                 xown=np.ascontiguousarray(xown))
        in_maps.append(m)
        metas.append((n0, n1, base_arr, span_arr, ns))

    return in_maps, metas, nsup, float(b_g2[0]), float(res_scale), use_bias


def _newton_rsqrt(nc, y, r, rh, w, msq):
    """y = 1/sqrt(msq) (all args APs of equal shape; r/rh/w scratch).
    msq is a sum of >=1 squared N(0,1) draws (roughly [1, 64]);
    r = 1/msq in ~[0.015, 1]; y = sqrt(r) by Heron from y0 = r + 0.25."""
    nc.vector.reciprocal(out=r, in_=msq)
    nc.vector.tensor_scalar_mul(out=rh, in0=r, scalar1=0.5)
    nc.vector.tensor_scalar(out=y, in0=r, scalar1=0.25, scalar2=None,
                            op0=OP.add)
    for _ in range(3):
        nc.vector.reciprocal(out=w, in_=y)
        nc.vector.tensor_tensor(out=w, in0=w, in1=rh, op=OP.mult)
        nc.vector.scalar_tensor_tensor(out=y, in0=y, scalar=0.5,
                                       in1=w, op0=OP.mult, op1=OP.add)


def build_program(nsup, bg2, res, use_bias):
    import concourse.bacc as bacc
    nc = bacc.Bacc("TRN2", target_bir_lowering=False, debug=False,
                   num_devices=NCORE)

    feat_d = nc.dram_tensor("feat", [nsup, 128, SPS, 40], BF16, kind="ExternalInput")
    scal_d = nc.dram_tensor("scal", [nsup, 128, SPS, 8], F32, kind="ExternalInput")
    rbft_d = nc.dram_tensor("rbft", [nsup * 2, 9, 512], BF16, kind="ExternalInput")
    sel_d = nc.dram_tensor("sel", [nsup, 128, SPS * 128], BF16, kind="ExternalInput")
    xown_d = nc.dram_tensor("xown", [nsup, 128, 40], F32, kind="ExternalInput")
    w1p_d = nc.dram_tensor("w1p", [9, 128], BF16, kind="ExternalInput")
    w2e_d = nc.dram_tensor("w2e", [128, 578], BF16, kind="ExternalInput")
    br2e_d = nc.dram_tensor("br2e", [1, 578], BF16, kind="ExternalInput")
    wms_d = nc.dram_tensor("wms", [16, 24], BF16, kind="ExternalInput")
    wmv_d = nc.dram_tensor("wmv", [24, 24], BF16, kind="ExternalInput")
    rep_d = nc.dram_tensor("rep", [8, 24], BF16, kind="ExternalInput")
    wus_d = nc.dram_tensor("wus", [16, 16], BF16, kind="ExternalInput")
    wss_d = nc.dram_tensor("wss", [16, 16], BF16, kind="ExternalInput")
    wuv_d = nc.dram_tensor("wuv", [24, 24], BF16, kind="ExternalInput")
    wsv_d = nc.dram_tensor("wsv", [24, 24], BF16, kind="ExternalInput")
    out_d = nc.dram_tensor("out", [nsup, 128, 40], F32, kind="ExternalOutput")

    with tile.TileContext(nc) as tc:
        with (
            tc.tile_pool(name="const", bufs=1) as cp,
            tc.tile_pool(name="io", bufs=3) as iop,
            tc.tile_pool(name="mid", bufs=2) as mp,
            tc.tile_pool(name="pp", bufs=2) as ppp,
            tc.tile_pool(name="nd", bufs=2) as ndp,
            tc.tile_pool(name="psh", bufs=2, space="PSUM") as psH,
            tc.tile_pool(name="psw0", bufs=1, space="PSUM") as psW0,
            tc.tile_pool(name="psw1", bufs=1, space="PSUM") as psW1,
            tc.tile_pool(name="psa", bufs=1, space="PSUM") as psA,
        ):
            w1p = cp.tile([9, 128], BF16, tag="w1p")
            w2e = cp.tile([128, 578], BF16, tag="w2e")
            br2e = cp.tile([1, 578], BF16, tag="br2e")
            wms = cp.tile([16, 24], BF16, tag="wms")
            wmv = cp.tile([24, 24], BF16, tag="wmv")
            rep = cp.tile([8, 24], BF16, tag="rep")
            wus = cp.tile([16, 16], BF16, tag="wus")
            wss = cp.tile([16, 16], BF16, tag="wss")
            wuv = cp.tile([24, 24], BF16, tag="wuv")
            wsv = cp.tile([24, 24], BF16, tag="wsv")
            ident = cp.tile([128, 128], F32, tag="ident")
            for t, d in [(w1p, w1p_d), (w2e, w2e_d), (br2e, br2e_d),
                         (wms, wms_d), (wmv, wmv_d), (rep, rep_d),
                         (wus, wus_d), (wss, wss_d), (wuv, wuv_d),
                         (wsv, wsv_d)]:
                nc.sync.dma_start(out=t[:], in_=d[:])
            make_identity(nc, ident[:])
            cbg2h = cp.tile([128, 1], F32, tag="cbg2h")
            nc.gpsimd.memset(cbg2h[:], 0.5 * bg2)
            onesr = cp.tile([1, 128], BF16, tag="onesr")
            nc.gpsimd.memset(onesr[:], 1.0)

            for s in range(nsup):
                feats = iop.tile([128, SPS, 40], BF16, tag="feat")
                scals = iop.tile([128, SPS, 8], F32, tag="scal")
                selt = iop.tile([128, SPS, 128], BF16, tag="sel")
                xo = iop.tile([128, 40], F32, tag="xo")
                nc.sync.dma_start(out=feats[:], in_=feat_d[s])
                nc.sync.dma_start(out=scals[:], in_=scal_d[s])
                nc.sync.dma_start(out=selt[:], in_=sel_d[s])
                nc.sync.dma_start(out=xo[:], in_=xown_d[s])

                # ---- joint RMS factors (raw sums of squares; mean-scales are
                # folded into w2e/wss/wsv on host) ----
                sq = mp.tile([128, SPS, 40], F32, tag="sq")
                nc.gpsimd.tensor_tensor(out=sq[:], in0=feats[:], in1=feats[:],
                                        op=OP.mult)
                xsq = mp.tile([128, 40], F32, tag="xsq")
                nc.gpsimd.tensor_tensor(out=xsq[:], in0=xo[:], in1=xo[:],
                                        op=OP.mult)
                # rows: 0 edge-s, 1 edge-v, 2 node ([s, v] in cols 0:2)
                ms = mp.tile([128, 3, SPS], F32, tag="ms")
                nc.vector.memset(ms[:, 2, 2:SPS], 1.0)
                nc.vector.reduce_sum(out=ms[:, 0, :], in_=sq[:, :, 0:16],
                                     axis=mybir.AxisListType.X)
                nc.vector.reduce_sum(out=ms[:, 1, :], in_=sq[:, :, 16:40],
                                     axis=mybir.AxisListType.X)
                nc.vector.reduce_sum(out=ms[:, 2, 0:1], in_=xsq[:, None, 0:16],
                                     axis=mybir.AxisListType.X)
                nc.vector.reduce_sum(out=ms[:, 2, 1:2], in_=xsq[:, None, 16:40],
                                     axis=mybir.AxisListType.X)
                inv = mp.tile([128, 3, SPS], F32, tag="inv")
                nr = mp.tile([128, 3, SPS], F32, tag="nr")
                nrh = mp.tile([128, 3, SPS], F32, tag="nrh")
                nw = mp.tile([128, 3, SPS], F32, tag="nw")
                _newton_rsqrt(nc, inv[:], nr[:], nrh[:], nw[:], ms[:])
                # inv rows: 0 = edge-s, 1 = edge-v, 2 = [node-s, node-v, ...]

                st = mp.tile([128, SPS, 16], BF16, tag="st")
                vt = mp.tile([128, SPS, 24], BF16, tag="vt")
                nc.gpsimd.tensor_tensor(
                    out=st[:], in0=feats[:, :, 0:16],
                    in1=inv[:, 0, :, None].to_broadcast([128, SPS, 16]),
                    op=OP.mult)
                nc.gpsimd.tensor_tensor(
                    out=vt[:], in0=feats[:, :, 16:40],
                    in1=inv[:, 1, :, None].to_broadcast([128, SPS, 24]),
                    op=OP.mult)

                # ---- radial MLP hidden for both groups ----
                hsil = []
                for g in range(2):
                    rbft = iop.tile([9, 512], BF16, tag="rbft")
                    nc.sync.dma_start(out=rbft[:], in_=rbft_d[s * 2 + g])
                    hp = psH.tile([128, 512], F32, tag="h")
                    nc.tensor.matmul(out=hp[:], lhsT=w1p[:], rhs=rbft[:],
                                     start=True, stop=True)
                    hs = mp.tile([128, 512], BF16, tag=f"hs{g}")
                    nc.scalar.activation(out=hs[:], in_=hp[:], func=AF.Silu)
                    hsil.append(hs)

                # supertile-wide chain/product tiles
                gw8 = mp.tile([128, SPS], F32, tag="gw8")
                o4 = mp.tile([128, SPS], BF16, tag="o4")
                o3cs = mp.tile([128, SPS, 6], BF16, tag="o3cs")
                i4 = mp.tile([128, SPS], BF16, tag="i4")
                g12 = mp.tile([128, SPS, 24], BF16, tag="g12")
                g4 = mp.tile([128, SPS, 24], BF16, tag="g4")
                u3 = mp.tile([128, SPS, 8], BF16, tag="u3")
                a2 = ppp.tile([128, SPS, 8, 3], BF16, tag="a2")
                t3d = ppp.tile([128, SPS, 8, 16], BF16, tag="t3")
                t4d = ppp.tile([128, SPS, 3, 64], BF16, tag="t4")
                P = ppp.tile([128, SPS, PCOLS], BF16, tag="P")

                agg = psA.tile([128, PCOLS], F32, tag="agg")
                for g in range(2):
                    sl4 = slice(g * 4, g * 4 + 4)
                    pw0 = psW0.tile([128, 4, 512], F32, tag="pw0")
                    pw1 = psW1.tile([128, 4, 66], F32, tag="pw1")
                    for tl in range(4):
                        lhs = hsil[g][:, tl * 128:(tl + 1) * 128]
                        if use_bias:
                            nc.tensor.matmul(out=pw0[:, tl, :], lhsT=onesr[:],
                                             rhs=br2e[:, 0:512],
                                             start=True, stop=False)
                            nc.tensor.matmul(out=pw1[:, tl, :], lhsT=onesr[:],
                                             rhs=br2e[:, 512:578],
                                             start=True, stop=False)
                        nc.tensor.matmul(out=pw0[:, tl, :], lhsT=lhs,
                                         rhs=w2e[:, 0:512],
                                         start=not use_bias, stop=True)
                        nc.tensor.matmul(out=pw1[:, tl, :], lhsT=lhs,
                                         rhs=w2e[:, 512:578],
                                         start=not use_bias, stop=True)

                    # ---- per-edge scalar chain (gpsimd; group batch) ----
                    nc.scalar.activation(out=gw8[:, sl4], in_=pw1[:, :, 64],
                                         func=AF.Tanh, scale=0.5, bias=cbg2h[:])
                    # ew = (tanh+1)*cwh, written straight into P's norm col
                    nc.vector.scalar_tensor_tensor(
                        out=P[:, sl4, C_EW], in0=gw8[:, sl4], scalar=1.0,
                        in1=scals[:, sl4, 7], op0=OP.add, op1=OP.mult)
                    ew = P[:, sl4, C_EW]
                    nc.gpsimd.tensor_tensor(out=o4[:, sl4], in0=ew,
                                            in1=scals[:, sl4, 0], op=OP.mult)
                    nc.gpsimd.tensor_tensor(
                        out=o3cs[:, sl4, :], in0=scals[:, sl4, 1:7],
                        in1=ew[:, :, None].to_broadcast([128, 4, 6]), op=OP.mult)
                    nc.gpsimd.tensor_tensor(out=i4[:, sl4], in0=o4[:, sl4],
                                            in1=inv[:, 0, sl4], op=OP.mult)
                    nc.gpsimd.tensor_tensor(
                        out=g12[:, sl4, 0:16], in0=feats[:, sl4, 0:16],
                        in1=i4[:, sl4, None].to_broadcast([128, 4, 16]),
                        op=OP.mult)
                    nc.gpsimd.tensor_tensor(
                        out=g4[:, sl4, :], in0=vt[:, sl4, :],
                        in1=o4[:, sl4, None].to_broadcast([128, 4, 24]),
                        op=OP.mult)
                    nc.gpsimd.tensor_tensor(
                        out=a2[:, sl4],
                        in0=vt[:, sl4, :].rearrange("p s (i c) -> p s i c", c=3),
                        in1=o3cs[:, sl4, None, 3:6].to_broadcast([128, 4, 8, 3]),
                        op=OP.mult)
                    with nc.allow_low_precision(reason="3-term bf16 sum"):
                        nc.vector.reduce_sum(out=g12[:, sl4, 16:24],
                                             in_=a2[:, sl4],
                                             axis=mybir.AxisListType.X)

                    # ---- products (DVE) ----
                    nc.vector.tensor_tensor(
                        out=P[:, sl4, 0:384].rearrange(
                            "p s (j i) -> p s j i", i=24),
                        in0=pw0[:, :, 0:384].rearrange(
                            "p s (j i) -> p s j i", i=24),
                        in1=g12[:, sl4, None, :].to_broadcast([128, 4, 16, 24]),
                        op=OP.mult)
                    nc.vector.tensor_tensor(
                        out=t3d[:, sl4],
                        in0=pw0[:, :, 384:512].rearrange(
                            "p s (j i) -> p s j i", i=16),
                        in1=st[:, sl4, None, :].to_broadcast([128, 4, 8, 16]),
                        op=OP.mult)
                    with nc.allow_low_precision(reason="16-term bf16 sum"):
                        nc.vector.reduce_sum(out=u3[:, sl4], in_=t3d[:, sl4],
                                             axis=mybir.AxisListType.X)
                    g4r = g4[:, sl4, :].rearrange("p s (i c) -> p s i c", c=3)
                    for c in range(3):
                        nc.vector.tensor_tensor(
                            out=t4d[:, sl4, c, :].rearrange(
                                "p s (j i) -> p s j i", i=8),
                            in0=pw1[:, :, 0:64].rearrange(
                                "p s (j i) -> p s j i", i=8),
                            in1=g4r[:, :, :, c][:, :, None, :].to_broadcast(
                                [128, 4, 8, 8]),
                            op=OP.mult)

                # ---- path3/4 outputs (supertile batch) + scatter ----
                nc.vector.tensor_tensor(
                    out=P[:, :, C_M13:C_M13 + 24].rearrange(
                        "p s (c j) -> p s c j", j=8),
                    in0=u3[:, :, None, :].to_broadcast([128, SPS, 3, 8]),
                    in1=o3cs[:, :, 0:3, None].to_broadcast([128, SPS, 3, 8]),
                    op=OP.mult)
                with nc.allow_low_precision(reason="8-term bf16 sum"):
                    nc.vector.reduce_sum(
                        out=P[:, :, C_M14:C_M14 + 24],
                        in_=t4d[:].rearrange("p s c (j i) -> p s (c j) i", i=8),
                        axis=mybir.AxisListType.X)
                for t in range(SPS):
                    nc.tensor.matmul(out=agg[:], lhsT=selt[:, t, :],
                                     rhs=P[:, t, :],
                                     start=(t == 0), stop=(t == SPS - 1))

                # ---- node phase ----
                m0 = ndp.tile([128, 16], F32, tag="m0")
                nc.vector.reduce_sum(
                    out=m0[:],
                    in_=agg[:, 0:384].rearrange("p (j i) -> p j i", i=24),
                    axis=mybir.AxisListType.X)
                v1 = ndp.tile([128, 24], F32, tag="v1")
                nc.vector.reduce_sum(
                    out=v1[:],
                    in_=agg[:, C_M13:C_M13 + 48].rearrange(
                        "p (a b) -> p b a", b=24),
                    axis=mybir.AxisListType.X)
                nrm = ndp.tile([128, 1], F32, tag="nrm")
                nc.vector.tensor_scalar_max(out=nrm[:], in0=agg[:, C_EW, None],
                                            scalar1=EPS)
                rinv = ndp.tile([128, 1], F32, tag="rinv")
                nc.vector.reciprocal(out=rinv[:], in_=nrm[:])

                cat_s = ndp.tile([128, 32], F32, tag="cat_s")
                cat_v = ndp.tile([128, 48], F32, tag="cat_v")
                nc.gpsimd.tensor_tensor(
                    out=cat_s[:, 0:16], in0=m0[:],
                    in1=rinv[:].to_broadcast([128, 16]), op=OP.mult)
                nc.gpsimd.tensor_tensor(
                    out=cat_v[:, 0:24], in0=v1[:],
                    in1=rinv[:].to_broadcast([128, 24]), op=OP.mult)
                nc.gpsimd.tensor_tensor(
                    out=cat_s[:, 16:32], in0=xo[:, 0:16],
                    in1=inv[:, 2, 0:1].to_broadcast([128, 16]), op=OP.mult)
                nc.gpsimd.tensor_tensor(
                    out=cat_v[:, 24:48], in0=xo[:, 16:40],
                    in1=inv[:, 2, 1:2].to_broadcast([128, 24]), op=OP.mult)

                def tposed(src_ap, rows, tag):
                    tp = psH.tile([rows, 128], F32, tag="h")
                    dst = ndp.tile([rows, 128], BF16, tag=tag)
                    nc.tensor.transpose(out=tp[:], in_=src_ap, identity=ident[:])
                    nc.scalar.copy(out=dst[:], in_=tp[:])
                    return dst

                aggT_s = tposed(cat_s[:, 0:16], 16, "aTs")
                xnT_s = tposed(cat_s[:, 16:32], 16, "xnTs")
                aggT_v = tposed(cat_v[:, 0:24], 24, "aTv")
                xnT_v = tposed(cat_v[:, 24:48], 24, "xnTv")

                scp = psH.tile([16, 128], F32, tag="h")
                nc.tensor.matmul(out=scp[:], lhsT=wms[:, 0:16], rhs=aggT_s[:],
                                 start=True, stop=True)
                scalT = ndp.tile([16, 128], BF16, tag="scalT")
                nc.scalar.activation(out=scalT[:], in_=scp[:], func=AF.Silu)
                gcp = psH.tile([8, 128], F32, tag="h")
                nc.tensor.matmul(out=gcp[:], lhsT=wms[:, 16:24], rhs=aggT_s[:],
                                 start=True, stop=True)
                gT = ndp.tile([8, 128], BF16, tag="gT")
                nc.scalar.activation(out=gT[:], in_=gcp[:], func=AF.Tanh,
                                     scale=0.5)
                nc.vector.tensor_scalar(out=gT[:], in0=gT[:], scalar1=0.5,
                                        scalar2=0.5, op0=OP.mult, op1=OP.add)

                vvp = psH.tile([24, 128], F32, tag="h")
                nc.tensor.matmul(out=vvp[:], lhsT=wmv[:], rhs=aggT_v[:],
                                 start=True, stop=True)
                grp = psH.tile([24, 128], F32, tag="h")
                nc.tensor.matmul(out=grp[:], lhsT=rep[:], rhs=gT[:],
                                 start=True, stop=True)
                vvc = ndp.tile([24, 128], BF16, tag="vvc")
                nc.scalar.copy(out=vvc[:], in_=vvp[:])
                vgT = ndp.tile([24, 128], BF16, tag="vgT")
                nc.vector.tensor_tensor(out=vgT[:], in0=vvc[:], in1=grp[:],
                                        op=OP.mult)

                osp = psH.tile([16, 128], F32, tag="h")
                nc.tensor.matmul(out=osp[:], lhsT=wus[:], rhs=scalT[:],
                                 start=True, stop=False)
                nc.tensor.matmul(out=osp[:], lhsT=wss[:], rhs=xnT_s[:],
                                 start=False, stop=True)
                ovp = psH.tile([24, 128], F32, tag="h")
                nc.tensor.matmul(out=ovp[:], lhsT=wuv[:], rhs=vgT[:],
                                 start=True, stop=False)
                nc.tensor.matmul(out=ovp[:], lhsT=wsv[:], rhs=xnT_v[:],
                                 start=False, stop=True)

                fTs = ndp.tile([16, 128], F32, tag="fTs")
                nc.vector.tensor_scalar_mul(out=fTs[:], in0=osp[:], scalar1=res)
                fTv = ndp.tile([24, 128], F32, tag="fTv")
                nc.vector.tensor_scalar_mul(out=fTv[:], in0=ovp[:], scalar1=res)
                fps = psH.tile([128, 16], F32, tag="h")
                nc.tensor.transpose(out=fps[:], in_=fTs[:],
                                    identity=ident[0:16, 0:16])
                fpv = psH.tile([128, 24], F32, tag="h")
                nc.tensor.transpose(out=fpv[:], in_=fTv[:],
                                    identity=ident[0:24, 0:24])
                outt = ndp.tile([128, 40], F32, tag="outt")
                nc.vector.tensor_tensor(out=outt[:, 0:16], in0=xo[:, 0:16],
                                        in1=fps[:], op=OP.add)
                nc.vector.tensor_tensor(out=outt[:, 16:40], in0=xo[:, 16:40],
                                        in1=fpv[:], op=OP.add)
                nc.sync.dma_start(out=out_d[s], in_=outt[:])

    nc.compile()
    return nc


_CACHE = {}


def kernel(**inputs):
    in_maps, metas, nsup, bg2, res, use_bias = _host_prep(**inputs)
    key = (nsup, bg2, res, use_bias)
    if key not in _CACHE:
        _CACHE[key] = build_program(nsup, bg2, res, use_bias)
    nc = _CACHE[key]
    r = run_bass_kernel_spmd(nc, in_maps, list(range(NCORE)))
    out = np.zeros((N, 40), np.float32)
    for k in range(NCORE):
        n0, n1, base_arr, span_arr, ns = metas[k]
        ob = r.results[k]["out"]
        for si in range(ns):
            sp = int(span_arr[si])
            if sp > 0:
                b = int(base_arr[si])
                out[b:b + sp] = ob[si, :sp]
    return out


# revision 31
# speedup vs baseline: 2.3480x; 1.0680x over previous
"""Trainium2 Bass kernel for nn_EquivariantInteractionBlock.

Strategy (edge/graph parallel, 8 cores):
- Host: sort edges by dst; split into 8 node-aligned contiguous ranges with
  ~E/8 edges each. Per core, pack edges into supertiles: <=1024 edges
  covering a window of <=128 consecutive dst nodes. Host gathers raw x rows
  by edge_src, precomputes the cosine cutoff, builds one-hot scatter
  matrices, and pre-swizzles everything into DMA-friendly bf16 layouts.
- Device per supertile (all matmuls bf16, fp32 PSUM accumulate):
  * radial MLP hidden: h = silu(rbf@W1) via one matmul + one silu per
    512-edge group (msg+gate hidden together, feature-major)
  * per-edge TP weights + gate logit: per 128-edge subtile one stationary
    load (h slice) and two matmuls streaming 512+66 weight columns
  * sigmoid via tanh (same ACT table set as silu -> no table reloads),
    rsqrt for RMS norms via DVE Newton iteration
  * tensor-product products on VectorE, i-reductions for paths 1/2 ride
    the scatter matmul as extra columns
  * scatter-add via host-built one-hot selection matrices (bf16 matmul)
  * node phase: normalize, two packed PE transposes, small accumulating
    matmuls for msg/update/self linears, residual in fp32
- Each core owns a disjoint node range: no collectives; host concatenates
  per-core output rows.
"""

import math
import numpy as np
import ml_dtypes

import concourse.bass as bass
import concourse.mybir as mybir
import concourse.tile as tile
from concourse.bass_utils import run_bass_kernel_spmd
from concourse.masks import make_identity

F32 = mybir.dt.float32
BF16 = mybir.dt.bfloat16
AF = mybir.ActivationFunctionType
OP = mybir.AluOpType
BF = ml_dtypes.bfloat16

N = 50000
E = 400000
MUL0 = 16
MUL1 = 8
RBF = 8
HID = 64
CUTOFF = 5.0
EPS = 1e-8
INV3 = float(1.0 / np.sqrt(np.float32(3.0)))
APATH = float(1.0 / math.sqrt(MUL0 + MUL1))
NCORE = 8
SUB = 128          # edges per subtile
SPS = 8            # subtiles per supertile
SUPE = SUB * SPS   # 1024 edges per supertile
NPW = 128          # node window per supertile

# P (product/scatter) column layout
C_P12 = 0           # 384: (j16 x [i16 p1 | i8 p2]) unreduced
C_M13 = 384         # 24: m1 path3 (c3,j8) reduced
C_M14 = 408         # 24: m1 path4 (c3,j8) reduced
C_EW = 432          # 1: edge weight (norm channel)
PCOLS = 433

# irrep-norm scale folding: device computes rsqrt(sum of squares); the
# 1/sqrt(mean) = sqrt(16) (s) / sqrt(8) (v) factors are folded into weights
FS = 4.0
FV = float(np.sqrt(8.0))


def _host_prep(x, edge_src, edge_dst, edge_sh, edge_rbf, edge_len,
               w_r1, b_r1, w_r2, b_r2, w_g1, b_g1, w_g2, b_g2,
               Wm_s, Wm_v, Wu_s, Wu_v, Ws_s, Ws_v, res_scale):
    order = np.argsort(edge_dst, kind="stable")
    src_s = edge_src[order]
    dst_s = edge_dst[order]
    sh_s = edge_sh[order]
    rbf_s = edge_rbf[order]
    len_s = edge_len[order]

    deg = np.bincount(edge_dst, minlength=N).astype(np.int64)
    cum = np.concatenate([[0], np.cumsum(deg)])

    bounds = [0]
    for k in range(1, NCORE):
        bounds.append(int(np.searchsorted(cum, k * E // NCORE)))
    bounds.append(N)

    cores = []
    for k in range(NCORE):
        n0, n1 = bounds[k], bounds[k + 1]
        sups = []  # (node_base, estart, ecnt)
        nb = n0
        while nb < n1:
            nn = nb
            cnt = 0
            while nn < n1 and nn - nb < NPW and cnt + deg[nn] <= SUPE:
                cnt += int(deg[nn])
                nn += 1
            sups.append((nb, int(cum[nb]), cnt))
            nb = nn
        cores.append((n0, n1, sups))

    nsup = max(len(c[2]) for c in cores)

    # ---- host-transformed weights (shared across cores) ----
    w1p = np.zeros((9, 128), np.float32)
    w1p[:8, :64] = w_r1
    w1p[:8, 64:] = w_g1
    w1p[8, :64] = b_r1
    w1p[8, 64:] = b_g1

    # w2e [128, 578]: rows 0:64 = w_r2 (reordered cols), rows 64:128 zero
    # except gate col. cols: 0:384 interleaved (j16 x [i16 p1 | i8 p2]),
    # 384:512 p3 (j8,i16), 512:576 p4 (j8,i8), 576 gate, 577 pad
    # block scales fold the 1/sqrt(mean)-vs-rsqrt(sum) factors: paths
    # contracting normalized s get FS, normalized v get FV
    w2e = np.zeros((128, 578), np.float32)
    wsrc = w_r2.astype(np.float32)  # [64, 576]
    # p1: our col j*24+i <- ref col i*16+j (i16, j16)
    jj, ii = np.meshgrid(np.arange(16), np.arange(16), indexing="ij")
    w2e[:64, (jj * 24 + ii).ravel()] = FS * wsrc[:, (ii * 16 + jj).ravel()]
    # p2: our col j*24+16+i <- ref col 256+i*16+j (i8, j16)
    jj, ii = np.meshgrid(np.arange(16), np.arange(8), indexing="ij")
    w2e[:64, (jj * 24 + 16 + ii).ravel()] = FV * wsrc[:, (256 + ii * 16 + jj).ravel()]
    # p3: our col 384+j*16+i <- ref col 384+i*8+j (i16, j8)
    jj, ii = np.meshgrid(np.arange(8), np.arange(16), indexing="ij")
    w2e[:64, (384 + jj * 16 + ii).ravel()] = FS * wsrc[:, (384 + ii * 8 + jj).ravel()]
    # p4: our col 512+j*8+i <- ref col 512+i*8+j (i8, j8)
    jj, ii = np.meshgrid(np.arange(8), np.arange(8), indexing="ij")
    w2e[:64, (512 + jj * 8 + ii).ravel()] = FV * wsrc[:, (512 + ii * 8 + jj).ravel()]
    w2e[64:128, 576] = w_g2[:, 0]

    # b_r2 row, same column order and scales (only used when b_r2 != 0)
    br2e = np.zeros((1, 578), np.float32)
    bsrc = b_r2.astype(np.float32)
    jj, ii = np.meshgrid(np.arange(16), np.arange(16), indexing="ij")
    br2e[0, (jj * 24 + ii).ravel()] = FS * bsrc[(ii * 16 + jj).ravel()]
    jj, ii = np.meshgrid(np.arange(16), np.arange(8), indexing="ij")
    br2e[0, (jj * 24 + 16 + ii).ravel()] = FV * bsrc[(256 + ii * 16 + jj).ravel()]
    jj, ii = np.meshgrid(np.arange(8), np.arange(16), indexing="ij")
    br2e[0, (384 + jj * 16 + ii).ravel()] = FS * bsrc[(384 + ii * 8 + jj).ravel()]
    jj, ii = np.meshgrid(np.arange(8), np.arange(8), indexing="ij")
    br2e[0, (512 + jj * 8 + ii).ravel()] = FV * bsrc[(512 + ii * 8 + jj).ravel()]
    use_bias = bool(np.any(b_r2 != 0.0))

    s0 = 1.0 / math.sqrt(MUL0)
    s1 = 1.0 / math.sqrt(MUL1)
    wms = (Wm_s * s0).astype(np.float32)                      # [16,24]
    wmv = np.zeros((24, 24), np.float32)
    wuv = np.zeros((24, 24), np.float32)
    wsv = np.zeros((24, 24), np.float32)
    for c in range(3):
        for j in range(8):
            for j2 in range(8):
                wmv[c * 8 + j, c * 8 + j2] = Wm_v[j, j2] * s1
                wuv[c * 8 + j, j2 * 3 + c] = Wu_v[j, j2] * s1
                wsv[j * 3 + c, j2 * 3 + c] = Ws_v[j, j2] * s1 * FV
    wus = (Wu_s * s0).astype(np.float32)
    wss = (Ws_s * s0 * FS).astype(np.float32)
    rep = np.zeros((8, 24), np.float32)
    for c in range(3):
        for j in range(8):
            rep[j, c * 8 + j] = 1.0

    shared = dict(
        w1p=w1p.astype(BF), w2e=w2e.astype(BF), br2e=br2e.astype(BF),
        wms=wms.astype(BF), wmv=wmv.astype(BF), rep=rep.astype(BF),
        wus=wus.astype(BF), wss=wss.astype(BF),
        wuv=wuv.astype(BF), wsv=wsv.astype(BF))

    in_maps = []
    metas = []
    for k in range(NCORE):
        n0, n1, sups = cores[k]
        ns = len(sups)
        idx = np.full((nsup, SUPE), -1, np.int64)
        base_arr = np.full((nsup,), n1, np.int64)
        span_arr = np.zeros((nsup,), np.int64)
        for si, (nb, es, cnt) in enumerate(sups):
            idx[si, :cnt] = np.arange(es, es + cnt)
            base_arr[si] = nb
            span_arr[si] = min(NPW, n1 - nb)
        mask = idx >= 0
        ic = np.clip(idx, 0, E - 1)

        feat = x[src_s[ic]]                                    # [nsup,SUPE,40]
        shp = sh_s[ic].astype(np.float32)
        lenp = len_s[ic].astype(np.float32)
        cw = 0.5 * (np.cos(np.pi * lenp / CUTOFF) + 1.0) * (lenp < CUTOFF)
        cwh = np.where(mask, 0.5 * cw, 0.0).astype(np.float32)  # [nsup,SUPE]
        rbfp = np.where(mask[..., None], rbf_s[ic], 0.0).astype(np.float32)
        dstl = np.where(mask, dst_s[ic] - base_arr[:, None], 0).astype(np.int64)

        # scal cols: sh0*APATH, sh1*APATH (3), sh1*APATH*INV3 (3), cwh
        scal = np.concatenate(
            [APATH * shp[..., 0:1], APATH * shp[..., 1:4],
             (APATH * INV3) * shp[..., 1:4], cwh[..., None]],
            axis=-1).astype(np.float32)                         # [nsup,SUPE,8]

        # swizzle [nsup, SUPE, F] -> [nsup, 128, SPS, F]
        def sw(a, dt):
            f = a.shape[-1]
            return np.ascontiguousarray(
                a.reshape(nsup, SPS, SUB, f).transpose(0, 2, 1, 3)).astype(dt)

        rbft = np.concatenate(
            [rbfp.reshape(nsup * 2, 512, 8).transpose(0, 2, 1),
             np.ones((nsup * 2, 1, 512), np.float32)], axis=1)  # [2nsup,9,512]

        # one-hot scatter matrices [nsup, SPS, SUB, NPW] -> [nsup,128,SPS*128]
        sel = np.zeros((nsup, SPS, SUB, NPW), np.float32)
        si_i, e_i = np.nonzero(mask)
        t_i = e_i // SUB
        p_i = e_i % SUB
        sel[si_i, t_i, p_i, dstl[si_i, e_i]] = 1.0
        sel = np.ascontiguousarray(
            sel.transpose(0, 2, 1, 3).reshape(nsup, SUB, SPS * NPW)).astype(BF)

        nodes = np.clip(base_arr[:, None] + np.arange(NPW)[None, :], 0, N - 1)
        xown = x[nodes].astype(np.float32)                      # [nsup,128,40]

        m = dict(shared)
        m.update(feat=sw(feat, BF), scal=sw(scal, np.float32),
                 rbft=np.ascontiguousarray(rbft).astype(BF), sel=sel,
                 xown=np.ascontiguousarray(xown))
        in_maps.append(m)
        metas.append((n0, n1, base_arr, span_arr, ns))

    return in_maps, metas, nsup, float(b_g2[0]), float(res_scale), use_bias


def _newton_rsqrt(nc, y, r, rh, w, msq):
    """y = 1/sqrt(msq) (all args APs of equal shape; r/rh/w scratch).
    msq is a sum of >=1 squared N(0,1) draws (roughly [1, 64]);
    r = 1/msq in ~[0.015, 1]; y = sqrt(r) by Heron from y0 = r + 0.25."""
    nc.vector.reciprocal(out=r, in_=msq)
    nc.vector.tensor_scalar_mul(out=rh, in0=r, scalar1=0.5)
    nc.vector.tensor_scalar(out=y, in0=r, scalar1=0.25, scalar2=None,
                            op0=OP.add)
    for _ in range(3):
        nc.vector.reciprocal(out=w, in_=y)
        nc.vector.tensor_tensor(out=w, in0=w, in1=rh, op=OP.mult)
        nc.vector.scalar_tensor_tensor(out=y, in0=y, scalar=0.5,
                                       in1=w, op0=OP.mult, op1=OP.add)


def build_program(nsup, bg2, res, use_bias):
    import concourse.bacc as bacc
    nc = bacc.Bacc("TRN2", target_bir_lowering=False, debug=False,
                   num_devices=NCORE)

    feat_d = nc.dram_tensor("feat", [nsup, 128, SPS, 40], BF16, kind="ExternalInput")
    scal_d = nc.dram_tensor("scal", [nsup, 128, SPS, 8], F32, kind="ExternalInput")
    rbft_d = nc.dram_tensor("rbft", [nsup * 2, 9, 512], BF16, kind="ExternalInput")
    sel_d = nc.dram_tensor("sel", [nsup, 128, SPS * 128], BF16, kind="ExternalInput")
    xown_d = nc.dram_tensor("xown", [nsup, 128, 40], F32, kind="ExternalInput")
    w1p_d = nc.dram_tensor("w1p", [9, 128], BF16, kind="ExternalInput")
    w2e_d = nc.dram_tensor("w2e", [128, 578], BF16, kind="ExternalInput")
    br2e_d = nc.dram_tensor("br2e", [1, 578], BF16, kind="ExternalInput")
    wms_d = nc.dram_tensor("wms", [16, 24], BF16, kind="ExternalInput")
    wmv_d = nc.dram_tensor("wmv", [24, 24], BF16, kind="ExternalInput")
    rep_d = nc.dram_tensor("rep", [8, 24], BF16, kind="ExternalInput")
    wus_d = nc.dram_tensor("wus", [16, 16], BF16, kind="ExternalInput")
    wss_d = nc.dram_tensor("wss", [16, 16], BF16, kind="ExternalInput")
    wuv_d = nc.dram_tensor("wuv", [24, 24], BF16, kind="ExternalInput")
    wsv_d = nc.dram_tensor("wsv", [24, 24], BF16, kind="ExternalInput")
    out_d = nc.dram_tensor("out", [nsup, 128, 40], F32, kind="ExternalOutput")

    with tile.TileContext(nc) as tc:
        with (
            tc.tile_pool(name="const", bufs=1) as cp,
            tc.tile_pool(name="io", bufs=3) as iop,
            tc.tile_pool(name="mid", bufs=2) as mp,
            tc.tile_pool(name="pp", bufs=2) as ppp,
            tc.tile_pool(name="nd", bufs=2) as ndp,
            tc.tile_pool(name="psh", bufs=2, space="PSUM") as psH,
            tc.tile_pool(name="psw0", bufs=2, space="PSUM") as psW0,
            tc.tile_pool(name="psw1", bufs=1, space="PSUM") as psW1,
            tc.tile_pool(name="psa", bufs=1, space="PSUM") as psA,
        ):
            w1p = cp.tile([9, 128], BF16, tag="w1p")
            w2e = cp.tile([128, 578], BF16, tag="w2e")
            br2e = cp.tile([1, 578], BF16, tag="br2e")
            wms = cp.tile([16, 24], BF16, tag="wms")
            wmv = cp.tile([24, 24], BF16, tag="wmv")
            rep = cp.tile([8, 24], BF16, tag="rep")
            wus = cp.tile([16, 16], BF16, tag="wus")
            wss = cp.tile([16, 16], BF16, tag="wss")
            wuv = cp.tile([24, 24], BF16, tag="wuv")
            wsv = cp.tile([24, 24], BF16, tag="wsv")
            ident = cp.tile([128, 128], F32, tag="ident")
            for t, d in [(w1p, w1p_d), (w2e, w2e_d), (br2e, br2e_d),
                         (wms, wms_d), (wmv, wmv_d), (rep, rep_d),
                         (wus, wus_d), (wss, wss_d), (wuv, wuv_d),
                         (wsv, wsv_d)]:
                nc.sync.dma_start(out=t[:], in_=d[:])
            make_identity(nc, ident[:])
            cbg2h = cp.tile([128, 1], F32, tag="cbg2h")
            nc.gpsimd.memset(cbg2h[:], 0.5 * bg2)
            onesr = cp.tile([1, 128], BF16, tag="onesr")
            nc.gpsimd.memset(onesr[:], 1.0)

            for s in range(nsup):
                feats = iop.tile([128, SPS, 40], BF16, tag="feat")
                scals = iop.tile([128, SPS, 8], F32, tag="scal")
                selt = iop.tile([128, SPS, 128], BF16, tag="sel")
                xo = iop.tile([128, 40], F32, tag="xo")
                nc.sync.dma_start(out=feats[:], in_=feat_d[s])
                nc.sync.dma_start(out=scals[:], in_=scal_d[s])
                nc.sync.dma_start(out=selt[:], in_=sel_d[s])
                nc.sync.dma_start(out=xo[:], in_=xown_d[s])

                # ---- joint RMS factors (raw sums of squares; mean-scales are
                # folded into w2e/wss/wsv on host) ----
                sq = mp.tile([128, SPS, 40], F32, tag="sq")
                nc.gpsimd.tensor_tensor(out=sq[:], in0=feats[:], in1=feats[:],
                                        op=OP.mult)
                xsq = mp.tile([128, 40], F32, tag="xsq")
                nc.gpsimd.tensor_tensor(out=xsq[:], in0=xo[:], in1=xo[:],
                                        op=OP.mult)
                # rows: 0 edge-s, 1 edge-v, 2 node ([s, v] in cols 0:2)
                ms = mp.tile([128, 3, SPS], F32, tag="ms")
                nc.vector.memset(ms[:, 2, 2:SPS], 1.0)
                nc.vector.reduce_sum(out=ms[:, 0, :], in_=sq[:, :, 0:16],
                                     axis=mybir.AxisListType.X)
                nc.vector.reduce_sum(out=ms[:, 1, :], in_=sq[:, :, 16:40],
                                     axis=mybir.AxisListType.X)
                nc.vector.reduce_sum(out=ms[:, 2, 0:1], in_=xsq[:, None, 0:16],
                                     axis=mybir.AxisListType.X)
                nc.vector.reduce_sum(out=ms[:, 2, 1:2], in_=xsq[:, None, 16:40],
                                     axis=mybir.AxisListType.X)
                inv = mp.tile([128, 3, SPS], F32, tag="inv")
                nr = mp.tile([128, 3, SPS], F32, tag="nr")
                nrh = mp.tile([128, 3, SPS], F32, tag="nrh")
                nw = mp.tile([128, 3, SPS], F32, tag="nw")
                _newton_rsqrt(nc, inv[:], nr[:], nrh[:], nw[:], ms[:])
                # inv rows: 0 = edge-s, 1 = edge-v, 2 = [node-s, node-v, ...]

                st = mp.tile([128, SPS, 16], BF16, tag="st")
                vt = mp.tile([128, SPS, 24], BF16, tag="vt")
                nc.gpsimd.tensor_tensor(
                    out=st[:], in0=feats[:, :, 0:16],
                    in1=inv[:, 0, :, None].to_broadcast([128, SPS, 16]),
                    op=OP.mult)
                nc.gpsimd.tensor_tensor(
                    out=vt[:], in0=feats[:, :, 16:40],
                    in1=inv[:, 1, :, None].to_broadcast([128, SPS, 24]),
                    op=OP.mult)

                # ---- radial MLP hidden for both groups ----
                hsil = []
                for g in range(2):
                    rbft = iop.tile([9, 512], BF16, tag="rbft")
                    nc.sync.dma_start(out=rbft[:], in_=rbft_d[s * 2 + g])
                    hp = psH.tile([128, 512], F32, tag="h")
                    nc.tensor.matmul(out=hp[:], lhsT=w1p[:], rhs=rbft[:],
                                     start=True, stop=True)
                    hs = mp.tile([128, 512], BF16, tag=f"hs{g}")
                    nc.scalar.activation(out=hs[:], in_=hp[:], func=AF.Silu)
                    hsil.append(hs)

                # supertile-wide chain/product tiles
                gw8 = mp.tile([128, SPS], F32, tag="gw8")
                o4 = mp.tile([128, SPS], BF16, tag="o4")
                o3cs = mp.tile([128, SPS, 6], BF16, tag="o3cs")
                i4 = mp.tile([128, SPS], BF16, tag="i4")
                g12 = mp.tile([128, SPS, 24], BF16, tag="g12")
                g4 = mp.tile([128, SPS, 24], BF16, tag="g4")
                u3 = mp.tile([128, SPS, 8], BF16, tag="u3")
                a2 = ppp.tile([128, SPS, 8, 3], BF16, tag="a2")
                t3d = ppp.tile([128, SPS, 8, 16], BF16, tag="t3")
                t4d = ppp.tile([128, SPS, 3, 64], BF16, tag="t4")
                P = ppp.tile([128, SPS, PCOLS], BF16, tag="P")

                agg = psA.tile([128, PCOLS], F32, tag="agg")
                for g in range(2):
                    sl4 = slice(g * 4, g * 4 + 4)
                    # gate+p4 matmuls first so the scalar chain overlaps the
                    # big weight matmuls that follow
                    pw1 = psW1.tile([128, 4, 66], F32, tag="pw1")
                    for tl in range(4):
                        lhs = hsil[g][:, tl * 128:(tl + 1) * 128]
                        if use_bias:
                            nc.tensor.matmul(out=pw1[:, tl, :], lhsT=onesr[:],
                                             rhs=br2e[:, 512:578],
                                             start=True, stop=False)
                        nc.tensor.matmul(out=pw1[:, tl, :], lhsT=lhs,
                                         rhs=w2e[:, 512:578],
                                         start=not use_bias, stop=True)

                    # ---- per-edge scalar chain (gpsimd; group batch) ----
                    nc.scalar.activation(out=gw8[:, sl4], in_=pw1[:, :, 64],
                                         func=AF.Tanh, scale=0.5, bias=cbg2h[:])
                    # ew = (tanh+1)*cwh, written straight into P's norm col
                    nc.vector.scalar_tensor_tensor(
                        out=P[:, sl4, C_EW], in0=gw8[:, sl4], scalar=1.0,
                        in1=scals[:, sl4, 7], op0=OP.add, op1=OP.mult)
                    ew = P[:, sl4, C_EW]
                    nc.gpsimd.tensor_tensor(out=o4[:, sl4], in0=ew,
                                            in1=scals[:, sl4, 0], op=OP.mult)
                    nc.gpsimd.tensor_tensor(
                        out=o3cs[:, sl4, :], in0=scals[:, sl4, 1:7],
                        in1=ew[:, :, None].to_broadcast([128, 4, 6]), op=OP.mult)
                    nc.gpsimd.tensor_tensor(out=i4[:, sl4], in0=o4[:, sl4],
                                            in1=inv[:, 0, sl4], op=OP.mult)
                    nc.gpsimd.tensor_tensor(
                        out=g12[:, sl4, 0:16], in0=feats[:, sl4, 0:16],
                        in1=i4[:, sl4, None].to_broadcast([128, 4, 16]),
                        op=OP.mult)
                    nc.gpsimd.tensor_tensor(
                        out=g4[:, sl4, :], in0=vt[:, sl4, :],
                        in1=o4[:, sl4, None].to_broadcast([128, 4, 24]),
                        op=OP.mult)
                    nc.gpsimd.tensor_tensor(
                        out=a2[:, sl4],
                        in0=vt[:, sl4, :].rearrange("p s (i c) -> p s i c", c=3),
                        in1=o3cs[:, sl4, None, 3:6].to_broadcast([128, 4, 8, 3]),
                        op=OP.mult)
                    with nc.allow_low_precision(reason="3-term bf16 sum"):
                        nc.vector.reduce_sum(out=g12[:, sl4, 16:24],
                                             in_=a2[:, sl4],
                                             axis=mybir.AxisListType.X)

                    # ---- weight matmuls in double-buffered pairs, with the
                    # products for each pair issued as soon as it lands ----
                    for k in range(2):
                        sl2 = slice(g * 4 + k * 2, g * 4 + k * 2 + 2)
                        pw0 = psW0.tile([128, 2, 512], F32, tag="pw0")
                        for tl2 in range(2):
                            tl = k * 2 + tl2
                            lhs = hsil[g][:, tl * 128:(tl + 1) * 128]
                            if use_bias:
                                nc.tensor.matmul(out=pw0[:, tl2, :],
                                                 lhsT=onesr[:],
                                                 rhs=br2e[:, 0:512],
                                                 start=True, stop=False)
                            nc.tensor.matmul(out=pw0[:, tl2, :], lhsT=lhs,
                                             rhs=w2e[:, 0:512],
                                             start=not use_bias, stop=True)
                        nc.vector.tensor_tensor(
                            out=P[:, sl2, 0:384].rearrange(
                                "p s (j i) -> p s j i", i=24),
                            in0=pw0[:, :, 0:384].rearrange(
                                "p s (j i) -> p s j i", i=24),
                            in1=g12[:, sl2, None, :].to_broadcast(
                                [128, 2, 16, 24]),
                            op=OP.mult)
                        nc.vector.tensor_tensor(
                            out=t3d[:, sl2],
                            in0=pw0[:, :, 384:512].rearrange(
                                "p s (j i) -> p s j i", i=16),
                            in1=st[:, sl2, None, :].to_broadcast(
                                [128, 2, 8, 16]),
                            op=OP.mult)
                    with nc.allow_low_precision(reason="16-term bf16 sum"):
                        nc.vector.reduce_sum(out=u3[:, sl4], in_=t3d[:, sl4],
                                             axis=mybir.AxisListType.X)
                    g4r = g4[:, sl4, :].rearrange("p s (i c) -> p s i c", c=3)
                    for c in range(3):
                        nc.vector.tensor_tensor(
                            out=t4d[:, sl4, c, :].rearrange(
                                "p s (j i) -> p s j i", i=8),
                            in0=pw1[:, :, 0:64].rearrange(
                                "p s (j i) -> p s j i", i=8),
                            in1=g4r[:, :, :, c][:, :, None, :].to_broadcast(
                                [128, 4, 8, 8]),
                            op=OP.mult)

                # ---- path3/4 outputs (supertile batch) + scatter ----
                nc.vector.tensor_tensor(
                    out=P[:, :, C_M13:C_M13 + 24].rearrange(
                        "p s (c j) -> p s c j", j=8),
                    in0=u3[:, :, None, :].to_broadcast([128, SPS, 3, 8]),
                    in1=o3cs[:, :, 0:3, None].to_broadcast([128, SPS, 3, 8]),
                    op=OP.mult)
                with nc.allow_low_precision(reason="8-term bf16 sum"):
                    nc.vector.reduce_sum(
                        out=P[:, :, C_M14:C_M14 + 24],
                        in_=t4d[:].rearrange("p s c (j i) -> p s (c j) i", i=8),
                        axis=mybir.AxisListType.X)
                for t in range(SPS):
                    nc.tensor.matmul(out=agg[:], lhsT=selt[:, t, :],
                                     rhs=P[:, t, :],
                                     start=(t == 0), stop=(t == SPS - 1))

                # ---- node phase ----
                m0 = ndp.tile([128, 16], F32, tag="m0")
                nc.vector.reduce_sum(
                    out=m0[:],
                    in_=agg[:, 0:384].rearrange("p (j i) -> p j i", i=24),
                    axis=mybir.AxisListType.X)
                v1 = ndp.tile([128, 24], F32, tag="v1")
                nc.vector.reduce_sum(
                    out=v1[:],
                    in_=agg[:, C_M13:C_M13 + 48].rearrange(
                        "p (a b) -> p b a", b=24),
                    axis=mybir.AxisListType.X)
                nrm = ndp.tile([128, 1], F32, tag="nrm")
                nc.vector.tensor_scalar_max(out=nrm[:], in0=agg[:, C_EW, None],
                                            scalar1=EPS)
                rinv = ndp.tile([128, 1], F32, tag="rinv")
                nc.vector.reciprocal(out=rinv[:], in_=nrm[:])

                cat_s = ndp.tile([128, 32], F32, tag="cat_s")
                cat_v = ndp.tile([128, 48], F32, tag="cat_v")
                nc.gpsimd.tensor_tensor(
                    out=cat_s[:, 0:16], in0=m0[:],
                    in1=rinv[:].to_broadcast([128, 16]), op=OP.mult)
                nc.gpsimd.tensor_tensor(
                    out=cat_v[:, 0:24], in0=v1[:],
                    in1=rinv[:].to_broadcast([128, 24]), op=OP.mult)
                nc.gpsimd.tensor_tensor(
                    out=cat_s[:, 16:32], in0=xo[:, 0:16],
                    in1=inv[:, 2, 0:1].to_broadcast([128, 16]), op=OP.mult)
                nc.gpsimd.tensor_tensor(
                    out=cat_v[:, 24:48], in0=xo[:, 16:40],
                    in1=inv[:, 2, 1:2].to_broadcast([128, 24]), op=OP.mult)

                def tposed(src_ap, rows, tag):
                    tp = psH.tile([rows, 128], F32, tag="h")
                    dst = ndp.tile([rows, 128], BF16, tag=tag)
                    nc.tensor.transpose(out=tp[:], in_=src_ap, identity=ident[:])
                    nc.scalar.copy(out=dst[:], in_=tp[:])
                    return dst

                aggT_s = tposed(cat_s[:, 0:16], 16, "aTs")
                xnT_s = tposed(cat_s[:, 16:32], 16, "xnTs")
                aggT_v = tposed(cat_v[:, 0:24], 24, "aTv")
                xnT_v = tposed(cat_v[:, 24:48], 24, "xnTv")

                scp = psH.tile([16, 128], F32, tag="h")
                nc.tensor.matmul(out=scp[:], lhsT=wms[:, 0:16], rhs=aggT_s[:],
                                 start=True, stop=True)
                scalT = ndp.tile([16, 128], BF16, tag="scalT")
                nc.scalar.activation(out=scalT[:], in_=scp[:], func=AF.Silu)
                gcp = psH.tile([8, 128], F32, tag="h")
                nc.tensor.matmul(out=gcp[:], lhsT=wms[:, 16:24], rhs=aggT_s[:],
                                 start=True, stop=True)
                gT = ndp.tile([8, 128], BF16, tag="gT")
                nc.scalar.activation(out=gT[:], in_=gcp[:], func=AF.Tanh,
                                     scale=0.5)
                nc.vector.tensor_scalar(out=gT[:], in0=gT[:], scalar1=0.5,
                                        scalar2=0.5, op0=OP.mult, op1=OP.add)

                vvp = psH.tile([24, 128], F32, tag="h")
                nc.tensor.matmul(out=vvp[:], lhsT=wmv[:], rhs=aggT_v[:],
                                 start=True, stop=True)
                grp = psH.tile([24, 128], F32, tag="h")
                nc.tensor.matmul(out=grp[:], lhsT=rep[:], rhs=gT[:],
                                 start=True, stop=True)
                vvc = ndp.tile([24, 128], BF16, tag="vvc")
                nc.scalar.copy(out=vvc[:], in_=vvp[:])
                vgT = ndp.tile([24, 128], BF16, tag="vgT")
                nc.vector.tensor_tensor(out=vgT[:], in0=vvc[:], in1=grp[:],
                                        op=OP.mult)

                osp = psH.tile([16, 128], F32, tag="h")
                nc.tensor.matmul(out=osp[:], lhsT=wus[:], rhs=scalT[:],
                                 start=True, stop=False)
                nc.tensor.matmul(out=osp[:], lhsT=wss[:], rhs=xnT_s[:],
                                 start=False, stop=True)
                ovp = psH.tile([24, 128], F32, tag="h")
                nc.tensor.matmul(out=ovp[:], lhsT=wuv[:], rhs=vgT[:],
                                 start=True, stop=False)
                nc.tensor.matmul(out=ovp[:], lhsT=wsv[:], rhs=xnT_v[:],
                                 start=False, stop=True)

                fTs = ndp.tile([16, 128], F32, tag="fTs")
                nc.vector.tensor_scalar_mul(out=fTs[:], in0=osp[:], scalar1=res)
                fTv = ndp.tile([24, 128], F32, tag="fTv")
                nc.vector.tensor_scalar_mul(out=fTv[:], in0=ovp[:], scalar1=res)
                fps = psH.tile([128, 16], F32, tag="h")
                nc.tensor.transpose(out=fps[:], in_=fTs[:],
                                    identity=ident[0:16, 0:16])
                fpv = psH.tile([128, 24], F32, tag="h")
                nc.tensor.transpose(out=fpv[:], in_=fTv[:],
                                    identity=ident[0:24, 0:24])
                outt = ndp.tile([128, 40], F32, tag="outt")
                nc.vector.tensor_tensor(out=outt[:, 0:16], in0=xo[:, 0:16],
                                        in1=fps[:], op=OP.add)
                nc.vector.tensor_tensor(out=outt[:, 16:40], in0=xo[:, 16:40],
                                        in1=fpv[:], op=OP.add)
                nc.sync.dma_start(out=out_d[s], in_=outt[:])

    nc.compile()
    return nc


_CACHE = {}


def kernel(**inputs):
    in_maps, metas, nsup, bg2, res, use_bias = _host_prep(**inputs)
    key = (nsup, bg2, res, use_bias)
    if key not in _CACHE:
        _CACHE[key] = build_program(nsup, bg2, res, use_bias)
    nc = _CACHE[key]
    r = run_bass_kernel_spmd(nc, in_maps, list(range(NCORE)))
    out = np.zeros((N, 40), np.float32)
    for k in range(NCORE):
        n0, n1, base_arr, span_arr, ns = metas[k]
        ob = r.results[k]["out"]
        for si in range(ns):
            sp = int(span_arr[si])
            if sp > 0:
                b = int(base_arr[si])
                out[b:b + sp] = ob[si, :sp]
    return out


# revision 33
# speedup vs baseline: 2.4561x; 1.0460x over previous
"""Trainium2 Bass kernel for nn_EquivariantInteractionBlock.

Strategy (edge/graph parallel, 8 cores):
- Host: sort edges by dst; split into 8 node-aligned contiguous ranges with
  ~E/8 edges each. Per core, pack edges into supertiles: <=1024 edges
  covering a window of <=128 consecutive dst nodes. Host gathers raw x rows
  by edge_src, precomputes the cosine cutoff, builds one-hot scatter
  matrices, and pre-swizzles everything into DMA-friendly bf16 layouts.
- Device per supertile (all matmuls bf16, fp32 PSUM accumulate):
  * radial MLP hidden: h = silu(rbf@W1) via one matmul + one silu per
    512-edge group (msg+gate hidden together, feature-major)
  * per-edge TP weights + gate logit: per 128-edge subtile one stationary
    load (h slice) and two matmuls streaming 512+66 weight columns
  * sigmoid via tanh (same ACT table set as silu -> no table reloads),
    rsqrt for RMS norms via DVE Newton iteration
  * tensor-product products on VectorE, i-reductions for paths 1/2 ride
    the scatter matmul as extra columns
  * scatter-add via host-built one-hot selection matrices (bf16 matmul)
  * node phase: normalize, two packed PE transposes, small accumulating
    matmuls for msg/update/self linears, residual in fp32
- Each core owns a disjoint node range: no collectives; host concatenates
  per-core output rows.
"""

import math
import numpy as np
import ml_dtypes

import concourse.bass as bass
import concourse.mybir as mybir
import concourse.tile as tile
from concourse.bass_utils import run_bass_kernel_spmd
from concourse.masks import make_identity

F32 = mybir.dt.float32
BF16 = mybir.dt.bfloat16
AF = mybir.ActivationFunctionType
OP = mybir.AluOpType
BF = ml_dtypes.bfloat16

N = 50000
E = 400000
MUL0 = 16
MUL1 = 8
RBF = 8
HID = 64
CUTOFF = 5.0
EPS = 1e-8
INV3 = float(1.0 / np.sqrt(np.float32(3.0)))
APATH = float(1.0 / math.sqrt(MUL0 + MUL1))
NCORE = 8
SUB = 128          # edges per subtile
SPS = 8            # subtiles per supertile
SUPE = SUB * SPS   # 1024 edges per supertile
NPW = 128          # node window per supertile

# P (product/scatter) column layout
C_P12 = 0           # 384: (j16 x [i16 p1 | i8 p2]) unreduced
C_M13 = 384         # 24: m1 path3 (c3,j8) reduced
C_M14 = 408         # 24: m1 path4 (c3,j8) reduced
C_EW = 432          # 1: edge weight (norm channel)
PCOLS = 433

# irrep-norm scale folding: device computes rsqrt(sum of squares); the
# 1/sqrt(mean) = sqrt(16) (s) / sqrt(8) (v) factors are folded into weights
FS = 4.0
FV = float(np.sqrt(8.0))


def _host_prep(x, edge_src, edge_dst, edge_sh, edge_rbf, edge_len,
               w_r1, b_r1, w_r2, b_r2, w_g1, b_g1, w_g2, b_g2,
               Wm_s, Wm_v, Wu_s, Wu_v, Ws_s, Ws_v, res_scale):
    order = np.argsort(edge_dst, kind="stable")
    src_s = edge_src[order]
    dst_s = edge_dst[order]
    sh_s = edge_sh[order]
    rbf_s = edge_rbf[order]
    len_s = edge_len[order]

    deg = np.bincount(edge_dst, minlength=N).astype(np.int64)
    cum = np.concatenate([[0], np.cumsum(deg)])

    bounds = [0]
    for k in range(1, NCORE):
        bounds.append(int(np.searchsorted(cum, k * E // NCORE)))
    bounds.append(N)

    cores = []
    for k in range(NCORE):
        n0, n1 = bounds[k], bounds[k + 1]
        sups = []  # (node_base, estart, ecnt)
        nb = n0
        while nb < n1:
            nn = nb
            cnt = 0
            while nn < n1 and nn - nb < NPW and cnt + deg[nn] <= SUPE:
                cnt += int(deg[nn])
                nn += 1
            sups.append((nb, int(cum[nb]), cnt))
            nb = nn
        cores.append((n0, n1, sups))

    nsup = max(len(c[2]) for c in cores)

    # ---- host-transformed weights (shared across cores) ----
    w1p = np.zeros((9, 128), np.float32)
    w1p[:8, :64] = w_r1
    w1p[:8, 64:] = w_g1
    w1p[8, :64] = b_r1
    w1p[8, 64:] = b_g1

    # w2e [128, 578]: rows 0:64 = w_r2 (reordered cols), rows 64:128 zero
    # except gate col. cols: 0:384 interleaved (j16 x [i16 p1 | i8 p2]),
    # 384:512 p3 (j8,i16), 512:576 p4 (j8,i8), 576 gate, 577 pad
    # block scales fold the 1/sqrt(mean)-vs-rsqrt(sum) factors: paths
    # contracting normalized s get FS, normalized v get FV
    w2e = np.zeros((128, 578), np.float32)
    wsrc = w_r2.astype(np.float32)  # [64, 576]
    # p1: our col j*24+i <- ref col i*16+j (i16, j16)
    jj, ii = np.meshgrid(np.arange(16), np.arange(16), indexing="ij")
    w2e[:64, (jj * 24 + ii).ravel()] = FS * wsrc[:, (ii * 16 + jj).ravel()]
    # p2: our col j*24+16+i <- ref col 256+i*16+j (i8, j16)
    jj, ii = np.meshgrid(np.arange(16), np.arange(8), indexing="ij")
    w2e[:64, (jj * 24 + 16 + ii).ravel()] = FV * wsrc[:, (256 + ii * 16 + jj).ravel()]
    # p3: our col 384+j*16+i <- ref col 384+i*8+j (i16, j8)
    jj, ii = np.meshgrid(np.arange(8), np.arange(16), indexing="ij")
    w2e[:64, (384 + jj * 16 + ii).ravel()] = FS * wsrc[:, (384 + ii * 8 + jj).ravel()]
    # p4: our col 512+j*8+i <- ref col 512+i*8+j (i8, j8)
    jj, ii = np.meshgrid(np.arange(8), np.arange(8), indexing="ij")
    w2e[:64, (512 + jj * 8 + ii).ravel()] = FV * wsrc[:, (512 + ii * 8 + jj).ravel()]
    w2e[64:128, 576] = w_g2[:, 0]

    # b_r2 row, same column order and scales (only used when b_r2 != 0)
    br2e = np.zeros((1, 578), np.float32)
    bsrc = b_r2.astype(np.float32)
    jj, ii = np.meshgrid(np.arange(16), np.arange(16), indexing="ij")
    br2e[0, (jj * 24 + ii).ravel()] = FS * bsrc[(ii * 16 + jj).ravel()]
    jj, ii = np.meshgrid(np.arange(16), np.arange(8), indexing="ij")
    br2e[0, (jj * 24 + 16 + ii).ravel()] = FV * bsrc[(256 + ii * 16 + jj).ravel()]
    jj, ii = np.meshgrid(np.arange(8), np.arange(16), indexing="ij")
    br2e[0, (384 + jj * 16 + ii).ravel()] = FS * bsrc[(384 + ii * 8 + jj).ravel()]
    jj, ii = np.meshgrid(np.arange(8), np.arange(8), indexing="ij")
    br2e[0, (512 + jj * 8 + ii).ravel()] = FV * bsrc[(512 + ii * 8 + jj).ravel()]
    use_bias = bool(np.any(b_r2 != 0.0))

    s0 = 1.0 / math.sqrt(MUL0)
    s1 = 1.0 / math.sqrt(MUL1)
    wms = (Wm_s * s0).astype(np.float32)                      # [16,24]
    wmv = np.zeros((24, 24), np.float32)
    wuv = np.zeros((24, 24), np.float32)
    wsv = np.zeros((24, 24), np.float32)
    for c in range(3):
        for j in range(8):
            for j2 in range(8):
                wmv[c * 8 + j, c * 8 + j2] = Wm_v[j, j2] * s1
                wuv[c * 8 + j, j2 * 3 + c] = Wu_v[j, j2] * s1
                wsv[j * 3 + c, j2 * 3 + c] = Ws_v[j, j2] * s1 * FV
    wus = (Wu_s * s0).astype(np.float32)
    wss = (Ws_s * s0 * FS).astype(np.float32)
    rep = np.zeros((8, 24), np.float32)
    for c in range(3):
        for j in range(8):
            rep[j, c * 8 + j] = 1.0

    shared = dict(
        w1p=w1p.astype(BF), w2e=w2e.astype(BF), br2e=br2e.astype(BF),
        wms=wms.astype(BF), wmv=wmv.astype(BF), rep=rep.astype(BF),
        wus=wus.astype(BF), wss=wss.astype(BF),
        wuv=wuv.astype(BF), wsv=wsv.astype(BF))

    in_maps = []
    metas = []
    for k in range(NCORE):
        n0, n1, sups = cores[k]
        ns = len(sups)
        idx = np.full((nsup, SUPE), -1, np.int64)
        base_arr = np.full((nsup,), n1, np.int64)
        span_arr = np.zeros((nsup,), np.int64)
        for si, (nb, es, cnt) in enumerate(sups):
            idx[si, :cnt] = np.arange(es, es + cnt)
            base_arr[si] = nb
            span_arr[si] = min(NPW, n1 - nb)
        mask = idx >= 0
        ic = np.clip(idx, 0, E - 1)

        feat = x[src_s[ic]]                                    # [nsup,SUPE,40]
        shp = sh_s[ic].astype(np.float32)
        lenp = len_s[ic].astype(np.float32)
        cw = 0.5 * (np.cos(np.pi * lenp / CUTOFF) + 1.0) * (lenp < CUTOFF)
        cwh = np.where(mask, 0.5 * cw, 0.0).astype(np.float32)  # [nsup,SUPE]
        rbfp = np.where(mask[..., None], rbf_s[ic], 0.0).astype(np.float32)
        dstl = np.where(mask, dst_s[ic] - base_arr[:, None], 0).astype(np.int64)

        # scal cols: sh0*APATH, sh1*APATH (3), sh1*APATH*INV3 (3), cwh
        scal = np.concatenate(
            [APATH * shp[..., 0:1], APATH * shp[..., 1:4],
             (APATH * INV3) * shp[..., 1:4], cwh[..., None]],
            axis=-1).astype(np.float32)                         # [nsup,SUPE,8]

        # swizzle [nsup, SUPE, F] -> [nsup, 128, SPS, F]
        def sw(a, dt):
            f = a.shape[-1]
            return np.ascontiguousarray(
                a.reshape(nsup, SPS, SUB, f).transpose(0, 2, 1, 3)).astype(dt)

        rbft = np.concatenate(
            [rbfp.reshape(nsup * 2, 512, 8).transpose(0, 2, 1),
             np.ones((nsup * 2, 1, 512), np.float32)], axis=1)  # [2nsup,9,512]

        # one-hot scatter matrices [nsup, SPS, SUB, NPW] -> [nsup,128,SPS*128]
        sel = np.zeros((nsup, SPS, SUB, NPW), np.float32)
        si_i, e_i = np.nonzero(mask)
        t_i = e_i // SUB
        p_i = e_i % SUB
        sel[si_i, t_i, p_i, dstl[si_i, e_i]] = 1.0
        sel = np.ascontiguousarray(
            sel.transpose(0, 2, 1, 3).reshape(nsup, SUB, SPS * NPW)).astype(BF)

        nodes = np.clip(base_arr[:, None] + np.arange(NPW)[None, :], 0, N - 1)
        xown = x[nodes].astype(np.float32)                      # [nsup,128,40]

        m = dict(shared)
        m.update(feat=sw(feat, BF), scal=sw(scal, np.float32),
                 rbft=np.ascontiguousarray(rbft).astype(BF), sel=sel,
                 xown=np.ascontiguousarray(xown))
        in_maps.append(m)
        metas.append((n0, n1, base_arr, span_arr, ns))

    return in_maps, metas, nsup, float(b_g2[0]), float(res_scale), use_bias


def _newton_rsqrt(nc, y, r, rh, w, msq):
    """y = 1/sqrt(msq) (all args APs of equal shape; r/rh/w scratch).
    msq is a sum of >=1 squared N(0,1) draws (roughly [1, 64]);
    r = 1/msq in ~[0.015, 1]; y = sqrt(r) by Heron from y0 = r + 0.25."""
    nc.vector.reciprocal(out=r, in_=msq)
    nc.vector.tensor_scalar_mul(out=rh, in0=r, scalar1=0.5)
    nc.vector.tensor_scalar(out=y, in0=r, scalar1=0.25, scalar2=None,
                            op0=OP.add)
    for _ in range(2):
        nc.vector.reciprocal(out=w, in_=y)
        nc.vector.tensor_tensor(out=w, in0=w, in1=rh, op=OP.mult)
        nc.vector.scalar_tensor_tensor(out=y, in0=y, scalar=0.5,
                                       in1=w, op0=OP.mult, op1=OP.add)


def build_program(nsup, bg2, res, use_bias):
    import concourse.bacc as bacc
    nc = bacc.Bacc("TRN2", target_bir_lowering=False, debug=False,
                   num_devices=NCORE)

    feat_d = nc.dram_tensor("feat", [nsup, 128, SPS, 40], BF16, kind="ExternalInput")
    scal_d = nc.dram_tensor("scal", [nsup, 128, SPS, 8], F32, kind="ExternalInput")
    rbft_d = nc.dram_tensor("rbft", [nsup * 2, 9, 512], BF16, kind="ExternalInput")
    sel_d = nc.dram_tensor("sel", [nsup, 128, SPS * 128], BF16, kind="ExternalInput")
    xown_d = nc.dram_tensor("xown", [nsup, 128, 40], F32, kind="ExternalInput")
    w1p_d = nc.dram_tensor("w1p", [9, 128], BF16, kind="ExternalInput")
    w2e_d = nc.dram_tensor("w2e", [128, 578], BF16, kind="ExternalInput")
    br2e_d = nc.dram_tensor("br2e", [1, 578], BF16, kind="ExternalInput")
    wms_d = nc.dram_tensor("wms", [16, 24], BF16, kind="ExternalInput")
    wmv_d = nc.dram_tensor("wmv", [24, 24], BF16, kind="ExternalInput")
    rep_d = nc.dram_tensor("rep", [8, 24], BF16, kind="ExternalInput")
    wus_d = nc.dram_tensor("wus", [16, 16], BF16, kind="ExternalInput")
    wss_d = nc.dram_tensor("wss", [16, 16], BF16, kind="ExternalInput")
    wuv_d = nc.dram_tensor("wuv", [24, 24], BF16, kind="ExternalInput")
    wsv_d = nc.dram_tensor("wsv", [24, 24], BF16, kind="ExternalInput")
    out_d = nc.dram_tensor("out", [nsup, 128, 40], F32, kind="ExternalOutput")

    with tile.TileContext(nc) as tc:
        with (
            tc.tile_pool(name="const", bufs=1) as cp,
            tc.tile_pool(name="io", bufs=3) as iop,
            tc.tile_pool(name="mid", bufs=2) as mp,
            tc.tile_pool(name="pp", bufs=2) as ppp,
            tc.tile_pool(name="nd", bufs=2) as ndp,
            tc.tile_pool(name="psh", bufs=2, space="PSUM") as psH,
            tc.tile_pool(name="psw0", bufs=2, space="PSUM") as psW0,
            tc.tile_pool(name="psw1", bufs=1, space="PSUM") as psW1,
            tc.tile_pool(name="psa", bufs=1, space="PSUM") as psA,
        ):
            w1p = cp.tile([9, 128], BF16, tag="w1p")
            w2e = cp.tile([128, 578], BF16, tag="w2e")
            br2e = cp.tile([1, 578], BF16, tag="br2e")
            wms = cp.tile([16, 24], BF16, tag="wms")
            wmv = cp.tile([24, 24], BF16, tag="wmv")
            rep = cp.tile([8, 24], BF16, tag="rep")
            wus = cp.tile([16, 16], BF16, tag="wus")
            wss = cp.tile([16, 16], BF16, tag="wss")
            wuv = cp.tile([24, 24], BF16, tag="wuv")
            wsv = cp.tile([24, 24], BF16, tag="wsv")
            ident = cp.tile([128, 128], F32, tag="ident")
            for t, d in [(w1p, w1p_d), (w2e, w2e_d), (br2e, br2e_d),
                         (wms, wms_d), (wmv, wmv_d), (rep, rep_d),
                         (wus, wus_d), (wss, wss_d), (wuv, wuv_d),
                         (wsv, wsv_d)]:
                nc.sync.dma_start(out=t[:], in_=d[:])
            make_identity(nc, ident[:])
            cbg2h = cp.tile([128, 1], F32, tag="cbg2h")
            nc.gpsimd.memset(cbg2h[:], 0.5 * bg2)
            onesr = cp.tile([1, 128], BF16, tag="onesr")
            nc.gpsimd.memset(onesr[:], 1.0)

            for s in range(nsup):
                feats = iop.tile([128, SPS, 40], BF16, tag="feat")
                scals = iop.tile([128, SPS, 8], F32, tag="scal")
                selt = iop.tile([128, SPS, 128], BF16, tag="sel")
                xo = iop.tile([128, 40], F32, tag="xo")
                nc.sync.dma_start(out=feats[:], in_=feat_d[s])
                nc.sync.dma_start(out=scals[:], in_=scal_d[s])
                nc.sync.dma_start(out=selt[:], in_=sel_d[s])
                nc.sync.dma_start(out=xo[:], in_=xown_d[s])

                # ---- joint RMS factors (raw sums of squares; mean-scales are
                # folded into w2e/wss/wsv on host) ----
                sq = mp.tile([128, SPS, 40], F32, tag="sq")
                nc.gpsimd.tensor_tensor(out=sq[:], in0=feats[:], in1=feats[:],
                                        op=OP.mult)
                xsq = mp.tile([128, 40], F32, tag="xsq")
                nc.gpsimd.tensor_tensor(out=xsq[:], in0=xo[:], in1=xo[:],
                                        op=OP.mult)
                # rows: 0 edge-s, 1 edge-v, 2 node ([s, v] in cols 0:2)
                ms = mp.tile([128, 3, SPS], F32, tag="ms")
                nc.vector.memset(ms[:, 2, 2:SPS], 1.0)
                nc.vector.reduce_sum(out=ms[:, 0, :], in_=sq[:, :, 0:16],
                                     axis=mybir.AxisListType.X)
                nc.vector.reduce_sum(out=ms[:, 1, :], in_=sq[:, :, 16:40],
                                     axis=mybir.AxisListType.X)
                nc.vector.reduce_sum(out=ms[:, 2, 0:1], in_=xsq[:, None, 0:16],
                                     axis=mybir.AxisListType.X)
                nc.vector.reduce_sum(out=ms[:, 2, 1:2], in_=xsq[:, None, 16:40],
                                     axis=mybir.AxisListType.X)
                inv = mp.tile([128, 3, SPS], F32, tag="inv")
                nr = mp.tile([128, 3, SPS], F32, tag="nr")
                nrh = mp.tile([128, 3, SPS], F32, tag="nrh")
                nw = mp.tile([128, 3, SPS], F32, tag="nw")
                _newton_rsqrt(nc, inv[:], nr[:], nrh[:], nw[:], ms[:])
                # inv rows: 0 = edge-s, 1 = edge-v, 2 = [node-s, node-v, ...]

                st = mp.tile([128, SPS, 16], BF16, tag="st")
                vt = mp.tile([128, SPS, 24], BF16, tag="vt")
                nc.gpsimd.tensor_tensor(
                    out=st[:], in0=feats[:, :, 0:16],
                    in1=inv[:, 0, :, None].to_broadcast([128, SPS, 16]),
                    op=OP.mult)
                nc.gpsimd.tensor_tensor(
                    out=vt[:], in0=feats[:, :, 16:40],
                    in1=inv[:, 1, :, None].to_broadcast([128, SPS, 24]),
                    op=OP.mult)

                # ---- radial MLP hidden for both groups ----
                hsil = []
                for g in range(2):
                    rbft = iop.tile([9, 512], BF16, tag="rbft")
                    nc.sync.dma_start(out=rbft[:], in_=rbft_d[s * 2 + g])
                    hp = psH.tile([128, 512], F32, tag="h")
                    nc.tensor.matmul(out=hp[:], lhsT=w1p[:], rhs=rbft[:],
                                     start=True, stop=True)
                    hs = mp.tile([128, 512], BF16, tag=f"hs{g}")
                    nc.scalar.activation(out=hs[:], in_=hp[:], func=AF.Silu)
                    hsil.append(hs)

                # supertile-wide chain/product tiles
                gw8 = mp.tile([128, SPS], F32, tag="gw8")
                o4 = mp.tile([128, SPS], BF16, tag="o4")
                o3cs = mp.tile([128, SPS, 6], BF16, tag="o3cs")
                i4 = mp.tile([128, SPS], BF16, tag="i4")
                g12 = mp.tile([128, SPS, 24], BF16, tag="g12")
                g4 = mp.tile([128, SPS, 24], BF16, tag="g4")
                u3 = mp.tile([128, SPS, 8], BF16, tag="u3")
                a2 = ppp.tile([128, SPS, 8, 3], BF16, tag="a2")
                t3d = ppp.tile([128, SPS, 8, 16], BF16, tag="t3")
                t4d = ppp.tile([128, SPS, 3, 64], BF16, tag="t4")
                P = ppp.tile([128, SPS, PCOLS], BF16, tag="P")

                agg = psA.tile([128, PCOLS], F32, tag="agg")
                for g in range(2):
                    sl4 = slice(g * 4, g * 4 + 4)
                    # gate+p4 matmuls first so the scalar chain overlaps the
                    # big weight matmuls that follow
                    pw1 = psW1.tile([128, 4, 66], F32, tag="pw1")
                    for tl in range(4):
                        lhs = hsil[g][:, tl * 128:(tl + 1) * 128]
                        if use_bias:
                            nc.tensor.matmul(out=pw1[:, tl, :], lhsT=onesr[:],
                                             rhs=br2e[:, 512:578],
                                             start=True, stop=False)
                        nc.tensor.matmul(out=pw1[:, tl, :], lhsT=lhs,
                                         rhs=w2e[:, 512:578],
                                         start=not use_bias, stop=True)

                    # ---- per-edge scalar chain (gpsimd; group batch) ----
                    nc.scalar.activation(out=gw8[:, sl4], in_=pw1[:, :, 64],
                                         func=AF.Tanh, scale=0.5, bias=cbg2h[:])
                    # ew = (tanh+1)*cwh, written straight into P's norm col
                    nc.vector.scalar_tensor_tensor(
                        out=P[:, sl4, C_EW], in0=gw8[:, sl4], scalar=1.0,
                        in1=scals[:, sl4, 7], op0=OP.add, op1=OP.mult)
                    ew = P[:, sl4, C_EW]
                    nc.gpsimd.tensor_tensor(out=o4[:, sl4], in0=ew,
                                            in1=scals[:, sl4, 0], op=OP.mult)
                    nc.gpsimd.tensor_tensor(
                        out=o3cs[:, sl4, :], in0=scals[:, sl4, 1:7],
                        in1=ew[:, :, None].to_broadcast([128, 4, 6]), op=OP.mult)
                    nc.gpsimd.tensor_tensor(out=i4[:, sl4], in0=o4[:, sl4],
                                            in1=inv[:, 0, sl4], op=OP.mult)
                    nc.gpsimd.tensor_tensor(
                        out=g12[:, sl4, 0:16], in0=feats[:, sl4, 0:16],
                        in1=i4[:, sl4, None].to_broadcast([128, 4, 16]),
                        op=OP.mult)
                    nc.gpsimd.tensor_tensor(
                        out=g4[:, sl4, :], in0=vt[:, sl4, :],
                        in1=o4[:, sl4, None].to_broadcast([128, 4, 24]),
                        op=OP.mult)
                    nc.gpsimd.tensor_tensor(
                        out=a2[:, sl4],
                        in0=vt[:, sl4, :].rearrange("p s (i c) -> p s i c", c=3),
                        in1=o3cs[:, sl4, None, 3:6].to_broadcast([128, 4, 8, 3]),
                        op=OP.mult)
                    with nc.allow_low_precision(reason="3-term bf16 sum"):
                        nc.vector.reduce_sum(out=g12[:, sl4, 16:24],
                                             in_=a2[:, sl4],
                                             axis=mybir.AxisListType.X)

                    # ---- weight matmuls in double-buffered pairs, with the
                    # products for each pair issued as soon as it lands ----
                    for k in range(2):
                        sl2 = slice(g * 4 + k * 2, g * 4 + k * 2 + 2)
                        pw0 = psW0.tile([128, 2, 512], F32, tag="pw0")
                        for tl2 in range(2):
                            tl = k * 2 + tl2
                            lhs = hsil[g][:, tl * 128:(tl + 1) * 128]
                            if use_bias:
                                nc.tensor.matmul(out=pw0[:, tl2, :],
                                                 lhsT=onesr[:],
                                                 rhs=br2e[:, 0:512],
                                                 start=True, stop=False)
                            nc.tensor.matmul(out=pw0[:, tl2, :], lhsT=lhs,
                                             rhs=w2e[:, 0:512],
                                             start=not use_bias, stop=True)
                        nc.vector.tensor_tensor(
                            out=P[:, sl2, 0:384].rearrange(
                                "p s (j i) -> p s j i", i=24),
                            in0=pw0[:, :, 0:384].rearrange(
                                "p s (j i) -> p s j i", i=24),
                            in1=g12[:, sl2, None, :].to_broadcast(
                                [128, 2, 16, 24]),
                            op=OP.mult)
                        nc.vector.tensor_tensor(
                            out=t3d[:, sl2],
                            in0=pw0[:, :, 384:512].rearrange(
                                "p s (j i) -> p s j i", i=16),
                            in1=st[:, sl2, None, :].to_broadcast(
                                [128, 2, 8, 16]),
                            op=OP.mult)
                    with nc.allow_low_precision(reason="16-term bf16 sum"):
                        nc.vector.reduce_sum(out=u3[:, sl4], in_=t3d[:, sl4],
                                             axis=mybir.AxisListType.X)
                    # stage p4 block to SBUF (ACT) so t4 runs in fast DVE mode
                    p4s = mp.tile([128, 4, 64], BF16, tag="p4s")
                    nc.scalar.copy(out=p4s[:], in_=pw1[:, :, 0:64])
                    g4r = g4[:, sl4, :].rearrange("p s (i c) -> p s i c", c=3)
                    for c in range(3):
                        nc.vector.tensor_tensor(
                            out=t4d[:, sl4, c, :].rearrange(
                                "p s (j i) -> p s j i", i=8),
                            in0=p4s[:].rearrange("p s (j i) -> p s j i", i=8),
                            in1=g4r[:, :, :, c][:, :, None, :].to_broadcast(
                                [128, 4, 8, 8]),
                            op=OP.mult)
                    nc.vector.tensor_tensor(
                        out=P[:, sl4, C_M13:C_M13 + 24].rearrange(
                            "p s (c j) -> p s c j", j=8),
                        in0=u3[:, sl4, None, :].to_broadcast([128, 4, 3, 8]),
                        in1=o3cs[:, sl4, 0:3, None].to_broadcast([128, 4, 3, 8]),
                        op=OP.mult)
                    with nc.allow_low_precision(reason="8-term bf16 sum"):
                        nc.vector.reduce_sum(
                            out=P[:, sl4, C_M14:C_M14 + 24],
                            in_=t4d[:, sl4].rearrange(
                                "p s c (j i) -> p s (c j) i", i=8),
                            axis=mybir.AxisListType.X)
                    # scatter this group's subtiles while the next group's
                    # matmuls proceed
                    for tl in range(4):
                        t = g * 4 + tl
                        nc.tensor.matmul(out=agg[:], lhsT=selt[:, t, :],
                                         rhs=P[:, t, :],
                                         start=(t == 0), stop=(t == SPS - 1))

                # ---- node phase ----
                m0 = ndp.tile([128, 16], F32, tag="m0")
                nc.vector.reduce_sum(
                    out=m0[:],
                    in_=agg[:, 0:384].rearrange("p (j i) -> p j i", i=24),
                    axis=mybir.AxisListType.X)
                v1 = ndp.tile([128, 24], F32, tag="v1")
                nc.vector.reduce_sum(
                    out=v1[:],
                    in_=agg[:, C_M13:C_M13 + 48].rearrange(
                        "p (a b) -> p b a", b=24),
                    axis=mybir.AxisListType.X)
                nrm = ndp.tile([128, 1], F32, tag="nrm")
                nc.vector.tensor_scalar_max(out=nrm[:], in0=agg[:, C_EW, None],
                                            scalar1=EPS)
                rinv = ndp.tile([128, 1], F32, tag="rinv")
                nc.vector.reciprocal(out=rinv[:], in_=nrm[:])

                cat_s = ndp.tile([128, 32], F32, tag="cat_s")
                cat_v = ndp.tile([128, 48], F32, tag="cat_v")
                nc.gpsimd.tensor_tensor(
                    out=cat_s[:, 0:16], in0=m0[:],
                    in1=rinv[:].to_broadcast([128, 16]), op=OP.mult)
                nc.gpsimd.tensor_tensor(
                    out=cat_v[:, 0:24], in0=v1[:],
                    in1=rinv[:].to_broadcast([128, 24]), op=OP.mult)
                nc.gpsimd.tensor_tensor(
                    out=cat_s[:, 16:32], in0=xo[:, 0:16],
                    in1=inv[:, 2, 0:1].to_broadcast([128, 16]), op=OP.mult)
                nc.gpsimd.tensor_tensor(
                    out=cat_v[:, 24:48], in0=xo[:, 16:40],
                    in1=inv[:, 2, 1:2].to_broadcast([128, 24]), op=OP.mult)

                def tposed(src_ap, rows, tag):
                    tp = psH.tile([rows, 128], F32, tag="h")
                    dst = ndp.tile([rows, 128], BF16, tag=tag)
                    nc.tensor.transpose(out=tp[:], in_=src_ap, identity=ident[:])
                    nc.scalar.copy(out=dst[:], in_=tp[:])
                    return dst

                aggT_s = tposed(cat_s[:, 0:16], 16, "aTs")
                xnT_s = tposed(cat_s[:, 16:32], 16, "xnTs")
                aggT_v = tposed(cat_v[:, 0:24], 24, "aTv")
                xnT_v = tposed(cat_v[:, 24:48], 24, "xnTv")

                scp = psH.tile([16, 128], F32, tag="h")
                nc.tensor.matmul(out=scp[:], lhsT=wms[:, 0:16], rhs=aggT_s[:],
                                 start=True, stop=True)
                scalT = ndp.tile([16, 128], BF16, tag="scalT")
                nc.scalar.activation(out=scalT[:], in_=scp[:], func=AF.Silu)
                gcp = psH.tile([8, 128], F32, tag="h")
                nc.tensor.matmul(out=gcp[:], lhsT=wms[:, 16:24], rhs=aggT_s[:],
                                 start=True, stop=True)
                gT = ndp.tile([8, 128], BF16, tag="gT")
                nc.scalar.activation(out=gT[:], in_=gcp[:], func=AF.Tanh,
                                     scale=0.5)
                nc.vector.tensor_scalar(out=gT[:], in0=gT[:], scalar1=0.5,
                                        scalar2=0.5, op0=OP.mult, op1=OP.add)

                vvp = psH.tile([24, 128], F32, tag="h")
                nc.tensor.matmul(out=vvp[:], lhsT=wmv[:], rhs=aggT_v[:],
                                 start=True, stop=True)
                grp = psH.tile([24, 128], F32, tag="h")
                nc.tensor.matmul(out=grp[:], lhsT=rep[:], rhs=gT[:],
                                 start=True, stop=True)
                vvc = ndp.tile([24, 128], BF16, tag="vvc")
                nc.scalar.copy(out=vvc[:], in_=vvp[:])
                vgT = ndp.tile([24, 128], BF16, tag="vgT")
                nc.vector.tensor_tensor(out=vgT[:], in0=vvc[:], in1=grp[:],
                                        op=OP.mult)

                osp = psH.tile([16, 128], F32, tag="h")
                nc.tensor.matmul(out=osp[:], lhsT=wus[:], rhs=scalT[:],
                                 start=True, stop=False)
                nc.tensor.matmul(out=osp[:], lhsT=wss[:], rhs=xnT_s[:],
                                 start=False, stop=True)
                ovp = psH.tile([24, 128], F32, tag="h")
                nc.tensor.matmul(out=ovp[:], lhsT=wuv[:], rhs=vgT[:],
                                 start=True, stop=False)
                nc.tensor.matmul(out=ovp[:], lhsT=wsv[:], rhs=xnT_v[:],
                                 start=False, stop=True)

                fTs = ndp.tile([16, 128], F32, tag="fTs")
                nc.vector.tensor_scalar_mul(out=fTs[:], in0=osp[:], scalar1=res)
                fTv = ndp.tile([24, 128], F32, tag="fTv")
                nc.vector.tensor_scalar_mul(out=fTv[:], in0=ovp[:], scalar1=res)
                fps = psH.tile([128, 16], F32, tag="h")
                nc.tensor.transpose(out=fps[:], in_=fTs[:],
                                    identity=ident[0:16, 0:16])
                fpv = psH.tile([128, 24], F32, tag="h")
                nc.tensor.transpose(out=fpv[:], in_=fTv[:],
                                    identity=ident[0:24, 0:24])
                outt = ndp.tile([128, 40], F32, tag="outt")
                nc.vector.tensor_tensor(out=outt[:, 0:16], in0=xo[:, 0:16],
                                        in1=fps[:], op=OP.add)
                nc.vector.tensor_tensor(out=outt[:, 16:40], in0=xo[:, 16:40],
                                        in1=fpv[:], op=OP.add)
                nc.sync.dma_start(out=out_d[s], in_=outt[:])

    nc.compile()
    return nc


_CACHE = {}


def kernel(**inputs):
    in_maps, metas, nsup, bg2, res, use_bias = _host_prep(**inputs)
    key = (nsup, bg2, res, use_bias)
    if key not in _CACHE:
        _CACHE[key] = build_program(nsup, bg2, res, use_bias)
    nc = _CACHE[key]
    r = run_bass_kernel_spmd(nc, in_maps, list(range(NCORE)))
    out = np.zeros((N, 40), np.float32)
    for k in range(NCORE):
        n0, n1, base_arr, span_arr, ns = metas[k]
        ob = r.results[k]["out"]
        for si in range(ns):
            sp = int(span_arr[si])
            if sp > 0:
                b = int(base_arr[si])
                out[b:b + sp] = ob[si, :sp]
    return out


# revision 35
# speedup vs baseline: 2.5433x; 1.0355x over previous
"""Trainium2 Bass kernel for nn_EquivariantInteractionBlock.

Strategy (edge/graph parallel, 8 cores):
- Host: sort edges by dst; split into 8 node-aligned contiguous ranges with
  ~E/8 edges each. Per core, pack edges into supertiles: <=1024 edges
  covering a window of <=128 consecutive dst nodes. Host gathers raw x rows
  by edge_src, precomputes the cosine cutoff, builds one-hot scatter
  matrices, and pre-swizzles everything into DMA-friendly bf16 layouts.
- Device per supertile (all matmuls bf16, fp32 PSUM accumulate):
  * radial MLP hidden: h = silu(rbf@W1) via one matmul + one silu per
    512-edge group (msg+gate hidden together, feature-major)
  * per-edge TP weights + gate logit: per 128-edge subtile one stationary
    load (h slice) and two matmuls streaming 512+66 weight columns
  * sigmoid via tanh (same ACT table set as silu -> no table reloads),
    rsqrt for RMS norms via DVE Newton iteration
  * tensor-product products on VectorE, i-reductions for paths 1/2 ride
    the scatter matmul as extra columns
  * scatter-add via host-built one-hot selection matrices (bf16 matmul)
  * node phase: normalize, two packed PE transposes, small accumulating
    matmuls for msg/update/self linears, residual in fp32
- Each core owns a disjoint node range: no collectives; host concatenates
  per-core output rows.
"""

import math
import numpy as np
import ml_dtypes

import concourse.bass as bass
import concourse.mybir as mybir
import concourse.tile as tile
from concourse.bass_utils import run_bass_kernel_spmd
from concourse.masks import make_identity

F32 = mybir.dt.float32
BF16 = mybir.dt.bfloat16
AF = mybir.ActivationFunctionType
OP = mybir.AluOpType
BF = ml_dtypes.bfloat16

N = 50000
E = 400000
MUL0 = 16
MUL1 = 8
RBF = 8
HID = 64
CUTOFF = 5.0
EPS = 1e-8
INV3 = float(1.0 / np.sqrt(np.float32(3.0)))
APATH = float(1.0 / math.sqrt(MUL0 + MUL1))
NCORE = 8
SUB = 128          # edges per subtile
SPS = 8            # subtiles per supertile
SUPE = SUB * SPS   # 1024 edges per supertile
NPW = 128          # node window per supertile

# P (product/scatter) column layout
C_P12 = 0           # 384: (j16 x [i16 p1 | i8 p2]) unreduced
C_M13 = 384         # 24: m1 path3 (c3,j8) reduced
C_M14 = 408         # 24: m1 path4 (c3,j8) reduced
C_EW = 432          # 1: edge weight (norm channel)
PCOLS = 433

# irrep-norm scale folding: device computes rsqrt(sum of squares); the
# 1/sqrt(mean) = sqrt(16) (s) / sqrt(8) (v) factors are folded into weights
FS = 4.0
FV = float(np.sqrt(8.0))


def _host_prep(x, edge_src, edge_dst, edge_sh, edge_rbf, edge_len,
               w_r1, b_r1, w_r2, b_r2, w_g1, b_g1, w_g2, b_g2,
               Wm_s, Wm_v, Wu_s, Wu_v, Ws_s, Ws_v, res_scale):
    order = np.argsort(edge_dst, kind="stable")
    src_s = edge_src[order]
    dst_s = edge_dst[order]
    sh_s = edge_sh[order]
    rbf_s = edge_rbf[order]
    len_s = edge_len[order]

    deg = np.bincount(edge_dst, minlength=N).astype(np.int64)
    cum = np.concatenate([[0], np.cumsum(deg)])

    bounds = [0]
    for k in range(1, NCORE):
        bounds.append(int(np.searchsorted(cum, k * E // NCORE)))
    bounds.append(N)

    cores = []
    for k in range(NCORE):
        n0, n1 = bounds[k], bounds[k + 1]
        sups = []  # (node_base, estart, ecnt)
        nb = n0
        while nb < n1:
            nn = nb
            cnt = 0
            while nn < n1 and nn - nb < NPW and cnt + deg[nn] <= SUPE:
                cnt += int(deg[nn])
                nn += 1
            sups.append((nb, int(cum[nb]), cnt))
            nb = nn
        cores.append((n0, n1, sups))

    nsup = max(len(c[2]) for c in cores)

    # ---- host-transformed weights (shared across cores) ----
    w1p = np.zeros((9, 128), np.float32)
    w1p[:8, :64] = w_r1
    w1p[:8, 64:] = w_g1
    w1p[8, :64] = b_r1
    w1p[8, 64:] = b_g1

    # w2e [128, 578]: rows 0:64 = w_r2 (reordered cols), rows 64:128 zero
    # except gate col. cols: 0:384 interleaved (j16 x [i16 p1 | i8 p2]),
    # 384:512 p3 (j8,i16), 512:576 p4 (j8,i8), 576 gate, 577 pad
    # block scales fold the 1/sqrt(mean)-vs-rsqrt(sum) factors: paths
    # contracting normalized s get FS, normalized v get FV
    w2e = np.zeros((128, 578), np.float32)
    wsrc = w_r2.astype(np.float32)  # [64, 576]
    # p1: our col j*24+i <- ref col i*16+j (i16, j16)
    jj, ii = np.meshgrid(np.arange(16), np.arange(16), indexing="ij")
    w2e[:64, (jj * 24 + ii).ravel()] = FS * wsrc[:, (ii * 16 + jj).ravel()]
    # p2: our col j*24+16+i <- ref col 256+i*16+j (i8, j16)
    jj, ii = np.meshgrid(np.arange(16), np.arange(8), indexing="ij")
    w2e[:64, (jj * 24 + 16 + ii).ravel()] = FV * wsrc[:, (256 + ii * 16 + jj).ravel()]
    # p3: our col 384+j*16+i <- ref col 384+i*8+j (i16, j8)
    jj, ii = np.meshgrid(np.arange(8), np.arange(16), indexing="ij")
    w2e[:64, (384 + jj * 16 + ii).ravel()] = FS * wsrc[:, (384 + ii * 8 + jj).ravel()]
    # p4: our col 512+j*8+i <- ref col 512+i*8+j (i8, j8)
    jj, ii = np.meshgrid(np.arange(8), np.arange(8), indexing="ij")
    w2e[:64, (512 + jj * 8 + ii).ravel()] = FV * wsrc[:, (512 + ii * 8 + jj).ravel()]
    w2e[64:128, 576] = w_g2[:, 0]

    # b_r2 row, same column order and scales (only used when b_r2 != 0)
    br2e = np.zeros((1, 578), np.float32)
    bsrc = b_r2.astype(np.float32)
    jj, ii = np.meshgrid(np.arange(16), np.arange(16), indexing="ij")
    br2e[0, (jj * 24 + ii).ravel()] = FS * bsrc[(ii * 16 + jj).ravel()]
    jj, ii = np.meshgrid(np.arange(16), np.arange(8), indexing="ij")
    br2e[0, (jj * 24 + 16 + ii).ravel()] = FV * bsrc[(256 + ii * 16 + jj).ravel()]
    jj, ii = np.meshgrid(np.arange(8), np.arange(16), indexing="ij")
    br2e[0, (384 + jj * 16 + ii).ravel()] = FS * bsrc[(384 + ii * 8 + jj).ravel()]
    jj, ii = np.meshgrid(np.arange(8), np.arange(8), indexing="ij")
    br2e[0, (512 + jj * 8 + ii).ravel()] = FV * bsrc[(512 + ii * 8 + jj).ravel()]
    use_bias = bool(np.any(b_r2 != 0.0))

    s0 = 1.0 / math.sqrt(MUL0)
    s1 = 1.0 / math.sqrt(MUL1)
    wms = (Wm_s * s0).astype(np.float32)                      # [16,24]
    wmv = np.zeros((24, 24), np.float32)
    wuv = np.zeros((24, 24), np.float32)
    wsv = np.zeros((24, 24), np.float32)
    for c in range(3):
        for j in range(8):
            for j2 in range(8):
                wmv[c * 8 + j, c * 8 + j2] = Wm_v[j, j2] * s1
                wuv[c * 8 + j, j2 * 3 + c] = Wu_v[j, j2] * s1
                wsv[j * 3 + c, j2 * 3 + c] = Ws_v[j, j2] * s1 * FV
    wus = (Wu_s * s0).astype(np.float32)
    wss = (Ws_s * s0 * FS).astype(np.float32)
    rep = np.zeros((8, 24), np.float32)
    for c in range(3):
        for j in range(8):
            rep[j, c * 8 + j] = 1.0

    shared = dict(
        w1p=w1p.astype(BF), w2e=w2e.astype(BF), br2e=br2e.astype(BF),
        wms=wms.astype(BF), wmv=wmv.astype(BF), rep=rep.astype(BF),
        wus=wus.astype(BF), wss=wss.astype(BF),
        wuv=wuv.astype(BF), wsv=wsv.astype(BF))

    in_maps = []
    metas = []
    for k in range(NCORE):
        n0, n1, sups = cores[k]
        ns = len(sups)
        idx = np.full((nsup, SUPE), -1, np.int64)
        base_arr = np.full((nsup,), n1, np.int64)
        span_arr = np.zeros((nsup,), np.int64)
        for si, (nb, es, cnt) in enumerate(sups):
            idx[si, :cnt] = np.arange(es, es + cnt)
            base_arr[si] = nb
            span_arr[si] = min(NPW, n1 - nb)
        mask = idx >= 0
        ic = np.clip(idx, 0, E - 1)

        feat = x[src_s[ic]]                                    # [nsup,SUPE,40]
        shp = sh_s[ic].astype(np.float32)
        lenp = len_s[ic].astype(np.float32)
        cw = 0.5 * (np.cos(np.pi * lenp / CUTOFF) + 1.0) * (lenp < CUTOFF)
        cwh = np.where(mask, 0.5 * cw, 0.0).astype(np.float32)  # [nsup,SUPE]
        rbfp = np.where(mask[..., None], rbf_s[ic], 0.0).astype(np.float32)
        dstl = np.where(mask, dst_s[ic] - base_arr[:, None], 0).astype(np.int64)

        # scal cols: sh0*APATH, sh1*APATH (3), sh1*APATH*INV3 (3), cwh
        scal = np.concatenate(
            [APATH * shp[..., 0:1], APATH * shp[..., 1:4],
             (APATH * INV3) * shp[..., 1:4], cwh[..., None]],
            axis=-1).astype(np.float32)                         # [nsup,SUPE,8]

        # swizzle [nsup, SUPE, F] -> [nsup, 128, SPS, F]
        def sw(a, dt):
            f = a.shape[-1]
            return np.ascontiguousarray(
                a.reshape(nsup, SPS, SUB, f).transpose(0, 2, 1, 3)).astype(dt)

        rbft = np.concatenate(
            [rbfp.reshape(nsup * 2, 512, 8).transpose(0, 2, 1),
             np.ones((nsup * 2, 1, 512), np.float32)], axis=1)  # [2nsup,9,512]

        # one-hot scatter matrices [nsup, SPS, SUB, NPW] -> [nsup,128,SPS*128]
        sel = np.zeros((nsup, SPS, SUB, NPW), np.float32)
        si_i, e_i = np.nonzero(mask)
        t_i = e_i // SUB
        p_i = e_i % SUB
        sel[si_i, t_i, p_i, dstl[si_i, e_i]] = 1.0
        sel = np.ascontiguousarray(
            sel.transpose(0, 2, 1, 3).reshape(nsup, SUB, SPS * NPW)).astype(BF)

        nodes = np.clip(base_arr[:, None] + np.arange(NPW)[None, :], 0, N - 1)
        xown = x[nodes].astype(np.float32)                      # [nsup,128,40]

        m = dict(shared)
        m.update(feat=sw(feat, BF), scal=sw(scal, np.float32),
                 rbft=np.ascontiguousarray(rbft).astype(BF), sel=sel,
                 xown=np.ascontiguousarray(xown))
        in_maps.append(m)
        metas.append((n0, n1, base_arr, span_arr, ns))

    return in_maps, metas, nsup, float(b_g2[0]), float(res_scale), use_bias


def _newton_rsqrt(nc, y, r, rh, w, msq):
    """y = 1/sqrt(msq) (all args APs of equal shape; r/rh/w scratch).
    msq is a sum of >=1 squared N(0,1) draws (roughly [1, 64]);
    r = 1/msq in ~[0.015, 1]; y = sqrt(r) by Heron from y0 = r + 0.25."""
    nc.vector.reciprocal(out=r, in_=msq)
    nc.vector.tensor_scalar_mul(out=rh, in0=r, scalar1=0.5)
    nc.vector.tensor_scalar(out=y, in0=r, scalar1=0.25, scalar2=None,
                            op0=OP.add)
    for _ in range(2):
        nc.vector.reciprocal(out=w, in_=y)
        nc.vector.tensor_tensor(out=w, in0=w, in1=rh, op=OP.mult)
        nc.vector.scalar_tensor_tensor(out=y, in0=y, scalar=0.5,
                                       in1=w, op0=OP.mult, op1=OP.add)


def build_program(nsup, bg2, res, use_bias):
    import concourse.bacc as bacc
    nc = bacc.Bacc("TRN2", target_bir_lowering=False, debug=False,
                   num_devices=NCORE)

    feat_d = nc.dram_tensor("feat", [nsup, 128, SPS, 40], BF16, kind="ExternalInput")
    scal_d = nc.dram_tensor("scal", [nsup, 128, SPS, 8], F32, kind="ExternalInput")
    rbft_d = nc.dram_tensor("rbft", [nsup * 2, 9, 512], BF16, kind="ExternalInput")
    sel_d = nc.dram_tensor("sel", [nsup, 128, SPS * 128], BF16, kind="ExternalInput")
    xown_d = nc.dram_tensor("xown", [nsup, 128, 40], F32, kind="ExternalInput")
    w1p_d = nc.dram_tensor("w1p", [9, 128], BF16, kind="ExternalInput")
    w2e_d = nc.dram_tensor("w2e", [128, 578], BF16, kind="ExternalInput")
    br2e_d = nc.dram_tensor("br2e", [1, 578], BF16, kind="ExternalInput")
    wms_d = nc.dram_tensor("wms", [16, 24], BF16, kind="ExternalInput")
    wmv_d = nc.dram_tensor("wmv", [24, 24], BF16, kind="ExternalInput")
    rep_d = nc.dram_tensor("rep", [8, 24], BF16, kind="ExternalInput")
    wus_d = nc.dram_tensor("wus", [16, 16], BF16, kind="ExternalInput")
    wss_d = nc.dram_tensor("wss", [16, 16], BF16, kind="ExternalInput")
    wuv_d = nc.dram_tensor("wuv", [24, 24], BF16, kind="ExternalInput")
    wsv_d = nc.dram_tensor("wsv", [24, 24], BF16, kind="ExternalInput")
    out_d = nc.dram_tensor("out", [nsup, 128, 40], F32, kind="ExternalOutput")

    with tile.TileContext(nc) as tc:
        with (
            tc.tile_pool(name="const", bufs=1) as cp,
            tc.tile_pool(name="io", bufs=3) as iop,
            tc.tile_pool(name="mid", bufs=3) as mp,
            tc.tile_pool(name="pp", bufs=3) as ppp,
            tc.tile_pool(name="nd", bufs=2) as ndp,
            tc.tile_pool(name="psh", bufs=2, space="PSUM") as psH,
            tc.tile_pool(name="psw0", bufs=2, space="PSUM") as psW0,
            tc.tile_pool(name="psw1", bufs=1, space="PSUM") as psW1,
            tc.tile_pool(name="psa", bufs=1, space="PSUM") as psA,
        ):
            w1p = cp.tile([9, 128], BF16, tag="w1p")
            w2e = cp.tile([128, 578], BF16, tag="w2e")
            br2e = cp.tile([1, 578], BF16, tag="br2e")
            wms = cp.tile([16, 24], BF16, tag="wms")
            wmv = cp.tile([24, 24], BF16, tag="wmv")
            rep = cp.tile([8, 24], BF16, tag="rep")
            wus = cp.tile([16, 16], BF16, tag="wus")
            wss = cp.tile([16, 16], BF16, tag="wss")
            wuv = cp.tile([24, 24], BF16, tag="wuv")
            wsv = cp.tile([24, 24], BF16, tag="wsv")
            ident = cp.tile([128, 128], F32, tag="ident")
            for t, d in [(w1p, w1p_d), (w2e, w2e_d), (br2e, br2e_d),
                         (wms, wms_d), (wmv, wmv_d), (rep, rep_d),
                         (wus, wus_d), (wss, wss_d), (wuv, wuv_d),
                         (wsv, wsv_d)]:
                nc.sync.dma_start(out=t[:], in_=d[:])
            make_identity(nc, ident[:])
            cbg2h = cp.tile([128, 1], F32, tag="cbg2h")
            nc.gpsimd.memset(cbg2h[:], 0.5 * bg2)
            onesr = cp.tile([1, 128], BF16, tag="onesr")
            nc.gpsimd.memset(onesr[:], 1.0)

            for s in range(nsup):
                feats = iop.tile([128, SPS, 40], BF16, tag="feat")
                scals = iop.tile([128, SPS, 8], F32, tag="scal")
                selt = iop.tile([128, SPS, 128], BF16, tag="sel")
                xo = iop.tile([128, 40], F32, tag="xo")
                nc.sync.dma_start(out=feats[:], in_=feat_d[s])
                nc.sync.dma_start(out=scals[:], in_=scal_d[s])
                nc.sync.dma_start(out=selt[:], in_=sel_d[s])
                nc.sync.dma_start(out=xo[:], in_=xown_d[s])

                # ---- joint RMS factors (raw sums of squares; mean-scales are
                # folded into w2e/wss/wsv on host) ----
                sq = mp.tile([128, SPS, 40], F32, tag="sq")
                nc.gpsimd.tensor_tensor(out=sq[:], in0=feats[:], in1=feats[:],
                                        op=OP.mult)
                xsq = mp.tile([128, 40], F32, tag="xsq")
                nc.gpsimd.tensor_tensor(out=xsq[:], in0=xo[:], in1=xo[:],
                                        op=OP.mult)
                # rows: 0 edge-s, 1 edge-v, 2 node ([s, v] in cols 0:2)
                ms = mp.tile([128, 3, SPS], F32, tag="ms")
                nc.vector.memset(ms[:, 2, 2:SPS], 1.0)
                nc.vector.reduce_sum(out=ms[:, 0, :], in_=sq[:, :, 0:16],
                                     axis=mybir.AxisListType.X)
                nc.vector.reduce_sum(out=ms[:, 1, :], in_=sq[:, :, 16:40],
                                     axis=mybir.AxisListType.X)
                nc.vector.reduce_sum(out=ms[:, 2, 0:1], in_=xsq[:, None, 0:16],
                                     axis=mybir.AxisListType.X)
                nc.vector.reduce_sum(out=ms[:, 2, 1:2], in_=xsq[:, None, 16:40],
                                     axis=mybir.AxisListType.X)
                inv = mp.tile([128, 3, SPS], F32, tag="inv")
                nr = mp.tile([128, 3, SPS], F32, tag="nr")
                nrh = mp.tile([128, 3, SPS], F32, tag="nrh")
                nw = mp.tile([128, 3, SPS], F32, tag="nw")
                _newton_rsqrt(nc, inv[:], nr[:], nrh[:], nw[:], ms[:])
                # inv rows: 0 = edge-s, 1 = edge-v, 2 = [node-s, node-v, ...]

                st = mp.tile([128, SPS, 16], BF16, tag="st")
                vt = mp.tile([128, SPS, 24], BF16, tag="vt")
                nc.gpsimd.tensor_tensor(
                    out=st[:], in0=feats[:, :, 0:16],
                    in1=inv[:, 0, :, None].to_broadcast([128, SPS, 16]),
                    op=OP.mult)
                nc.gpsimd.tensor_tensor(
                    out=vt[:], in0=feats[:, :, 16:40],
                    in1=inv[:, 1, :, None].to_broadcast([128, SPS, 24]),
                    op=OP.mult)

                # ---- radial MLP hidden for both groups ----
                hsil = []
                for g in range(2):
                    rbft = iop.tile([9, 512], BF16, tag="rbft")
                    nc.sync.dma_start(out=rbft[:], in_=rbft_d[s * 2 + g])
                    hp = psH.tile([128, 512], F32, tag="h")
                    nc.tensor.matmul(out=hp[:], lhsT=w1p[:], rhs=rbft[:],
                                     start=True, stop=True)
                    hs = mp.tile([128, 512], BF16, tag=f"hs{g}")
                    nc.scalar.activation(out=hs[:], in_=hp[:], func=AF.Silu)
                    hsil.append(hs)

                # supertile-wide chain/product tiles
                gw8 = mp.tile([128, SPS], F32, tag="gw8")
                o4 = mp.tile([128, SPS], BF16, tag="o4")
                o3cs = mp.tile([128, SPS, 6], BF16, tag="o3cs")
                i4 = mp.tile([128, SPS], BF16, tag="i4")
                g12 = mp.tile([128, SPS, 24], BF16, tag="g12")
                g4 = mp.tile([128, SPS, 24], BF16, tag="g4")
                u3 = mp.tile([128, SPS, 8], BF16, tag="u3")
                a2 = ppp.tile([128, SPS, 8, 3], BF16, tag="a2")
                t3d = ppp.tile([128, SPS, 8, 16], BF16, tag="t3")
                t4d = ppp.tile([128, SPS, 3, 64], BF16, tag="t4")
                P = ppp.tile([128, SPS, PCOLS], BF16, tag="P")

                agg = psA.tile([128, PCOLS], F32, tag="agg")
                for g in range(2):
                    sl4 = slice(g * 4, g * 4 + 4)
                    # gate+p4 matmuls first so the scalar chain overlaps the
                    # big weight matmuls that follow
                    pw1 = psW1.tile([128, 4, 66], F32, tag="pw1")
                    for tl in range(4):
                        lhs = hsil[g][:, tl * 128:(tl + 1) * 128]
                        if use_bias:
                            nc.tensor.matmul(out=pw1[:, tl, :], lhsT=onesr[:],
                                             rhs=br2e[:, 512:578],
                                             start=True, stop=False)
                        nc.tensor.matmul(out=pw1[:, tl, :], lhsT=lhs,
                                         rhs=w2e[:, 512:578],
                                         start=not use_bias, stop=True)

                    # ---- per-edge scalar chain (gpsimd; group batch) ----
                    nc.scalar.activation(out=gw8[:, sl4], in_=pw1[:, :, 64],
                                         func=AF.Tanh, scale=0.5, bias=cbg2h[:])
                    # ew = (tanh+1)*cwh, written straight into P's norm col
                    nc.vector.scalar_tensor_tensor(
                        out=P[:, sl4, C_EW], in0=gw8[:, sl4], scalar=1.0,
                        in1=scals[:, sl4, 7], op0=OP.add, op1=OP.mult)
                    ew = P[:, sl4, C_EW]
                    nc.gpsimd.tensor_tensor(out=o4[:, sl4], in0=ew,
                                            in1=scals[:, sl4, 0], op=OP.mult)
                    nc.gpsimd.tensor_tensor(
                        out=o3cs[:, sl4, :], in0=scals[:, sl4, 1:7],
                        in1=ew[:, :, None].to_broadcast([128, 4, 6]), op=OP.mult)
                    nc.gpsimd.tensor_tensor(out=i4[:, sl4], in0=o4[:, sl4],
                                            in1=inv[:, 0, sl4], op=OP.mult)
                    nc.gpsimd.tensor_tensor(
                        out=g12[:, sl4, 0:16], in0=feats[:, sl4, 0:16],
                        in1=i4[:, sl4, None].to_broadcast([128, 4, 16]),
                        op=OP.mult)
                    nc.gpsimd.tensor_tensor(
                        out=g4[:, sl4, :], in0=vt[:, sl4, :],
                        in1=o4[:, sl4, None].to_broadcast([128, 4, 24]),
                        op=OP.mult)
                    nc.gpsimd.tensor_tensor(
                        out=a2[:, sl4],
                        in0=vt[:, sl4, :].rearrange("p s (i c) -> p s i c", c=3),
                        in1=o3cs[:, sl4, None, 3:6].to_broadcast([128, 4, 8, 3]),
                        op=OP.mult)
                    with nc.allow_low_precision(reason="3-term bf16 sum"):
                        nc.vector.reduce_sum(out=g12[:, sl4, 16:24],
                                             in_=a2[:, sl4],
                                             axis=mybir.AxisListType.X)

                    # ---- weight matmuls in double-buffered pairs, with the
                    # products for each pair issued as soon as it lands ----
                    for k in range(2):
                        sl2 = slice(g * 4 + k * 2, g * 4 + k * 2 + 2)
                        pw0 = psW0.tile([128, 2, 512], F32, tag="pw0")
                        for tl2 in range(2):
                            tl = k * 2 + tl2
                            lhs = hsil[g][:, tl * 128:(tl + 1) * 128]
                            if use_bias:
                                nc.tensor.matmul(out=pw0[:, tl2, :],
                                                 lhsT=onesr[:],
                                                 rhs=br2e[:, 0:512],
                                                 start=True, stop=False)
                            nc.tensor.matmul(out=pw0[:, tl2, :], lhsT=lhs,
                                             rhs=w2e[:, 0:512],
                                             start=not use_bias, stop=True)
                        nc.vector.tensor_tensor(
                            out=P[:, sl2, 0:384].rearrange(
                                "p s (j i) -> p s j i", i=24),
                            in0=pw0[:, :, 0:384].rearrange(
                                "p s (j i) -> p s j i", i=24),
                            in1=g12[:, sl2, None, :].to_broadcast(
                                [128, 2, 16, 24]),
                            op=OP.mult)
                        nc.vector.tensor_tensor(
                            out=t3d[:, sl2],
                            in0=pw0[:, :, 384:512].rearrange(
                                "p s (j i) -> p s j i", i=16),
                            in1=st[:, sl2, None, :].to_broadcast(
                                [128, 2, 8, 16]),
                            op=OP.mult)
                    with nc.allow_low_precision(reason="16-term bf16 sum"):
                        nc.vector.reduce_sum(out=u3[:, sl4], in_=t3d[:, sl4],
                                             axis=mybir.AxisListType.X)
                    # stage p4 block to SBUF (ACT) so t4 runs in fast DVE mode
                    p4s = mp.tile([128, 4, 64], BF16, tag="p4s")
                    nc.scalar.copy(out=p4s[:], in_=pw1[:, :, 0:64])
                    g4r = g4[:, sl4, :].rearrange("p s (i c) -> p s i c", c=3)
                    for c in range(3):
                        nc.gpsimd.tensor_tensor(
                            out=t4d[:, sl4, c, :].rearrange(
                                "p s (j i) -> p s j i", i=8),
                            in0=p4s[:].rearrange("p s (j i) -> p s j i", i=8),
                            in1=g4r[:, :, :, c][:, :, None, :].to_broadcast(
                                [128, 4, 8, 8]),
                            op=OP.mult)
                    nc.gpsimd.tensor_tensor(
                        out=P[:, sl4, C_M13:C_M13 + 24].rearrange(
                            "p s (c j) -> p s c j", j=8),
                        in0=u3[:, sl4, None, :].to_broadcast([128, 4, 3, 8]),
                        in1=o3cs[:, sl4, 0:3, None].to_broadcast([128, 4, 3, 8]),
                        op=OP.mult)
                    with nc.allow_low_precision(reason="8-term bf16 sum"):
                        nc.vector.reduce_sum(
                            out=P[:, sl4, C_M14:C_M14 + 24],
                            in_=t4d[:, sl4].rearrange(
                                "p s c (j i) -> p s (c j) i", i=8),
                            axis=mybir.AxisListType.X)
                    # scatter this group's subtiles while the next group's
                    # matmuls proceed
                    for tl in range(4):
                        t = g * 4 + tl
                        nc.tensor.matmul(out=agg[:], lhsT=selt[:, t, :],
                                         rhs=P[:, t, :],
                                         start=(t == 0), stop=(t == SPS - 1))

                # ---- node phase ----
                m0 = ndp.tile([128, 16], F32, tag="m0")
                nc.vector.reduce_sum(
                    out=m0[:],
                    in_=agg[:, 0:384].rearrange("p (j i) -> p j i", i=24),
                    axis=mybir.AxisListType.X)
                v1 = ndp.tile([128, 24], F32, tag="v1")
                nc.vector.reduce_sum(
                    out=v1[:],
                    in_=agg[:, C_M13:C_M13 + 48].rearrange(
                        "p (a b) -> p b a", b=24),
                    axis=mybir.AxisListType.X)
                nrm = ndp.tile([128, 1], F32, tag="nrm")
                nc.vector.tensor_scalar_max(out=nrm[:], in0=agg[:, C_EW, None],
                                            scalar1=EPS)
                rinv = ndp.tile([128, 1], F32, tag="rinv")
                nc.vector.reciprocal(out=rinv[:], in_=nrm[:])

                cat_s = ndp.tile([128, 32], F32, tag="cat_s")
                cat_v = ndp.tile([128, 48], F32, tag="cat_v")
                nc.gpsimd.tensor_tensor(
                    out=cat_s[:, 0:16], in0=m0[:],
                    in1=rinv[:].to_broadcast([128, 16]), op=OP.mult)
                nc.gpsimd.tensor_tensor(
                    out=cat_v[:, 0:24], in0=v1[:],
                    in1=rinv[:].to_broadcast([128, 24]), op=OP.mult)
                nc.gpsimd.tensor_tensor(
                    out=cat_s[:, 16:32], in0=xo[:, 0:16],
                    in1=inv[:, 2, 0:1].to_broadcast([128, 16]), op=OP.mult)
                nc.gpsimd.tensor_tensor(
                    out=cat_v[:, 24:48], in0=xo[:, 16:40],
                    in1=inv[:, 2, 1:2].to_broadcast([128, 24]), op=OP.mult)

                def tposed(src_ap, rows, tag):
                    tp = psH.tile([rows, 128], F32, tag="h")
                    dst = ndp.tile([rows, 128], BF16, tag=tag)
                    nc.tensor.transpose(out=tp[:], in_=src_ap, identity=ident[:])
                    nc.scalar.copy(out=dst[:], in_=tp[:])
                    return dst

                aggT_s = tposed(cat_s[:, 0:16], 16, "aTs")
                xnT_s = tposed(cat_s[:, 16:32], 16, "xnTs")
                aggT_v = tposed(cat_v[:, 0:24], 24, "aTv")
                xnT_v = tposed(cat_v[:, 24:48], 24, "xnTv")

                scp = psH.tile([16, 128], F32, tag="h")
                nc.tensor.matmul(out=scp[:], lhsT=wms[:, 0:16], rhs=aggT_s[:],
                                 start=True, stop=True)
                scalT = ndp.tile([16, 128], BF16, tag="scalT")
                nc.scalar.activation(out=scalT[:], in_=scp[:], func=AF.Silu)
                gcp = psH.tile([8, 128], F32, tag="h")
                nc.tensor.matmul(out=gcp[:], lhsT=wms[:, 16:24], rhs=aggT_s[:],
                                 start=True, stop=True)
                gT = ndp.tile([8, 128], BF16, tag="gT")
                nc.scalar.activation(out=gT[:], in_=gcp[:], func=AF.Tanh,
                                     scale=0.5)
                nc.vector.tensor_scalar(out=gT[:], in0=gT[:], scalar1=0.5,
                                        scalar2=0.5, op0=OP.mult, op1=OP.add)

                vvp = psH.tile([24, 128], F32, tag="h")
                nc.tensor.matmul(out=vvp[:], lhsT=wmv[:], rhs=aggT_v[:],
                                 start=True, stop=True)
                grp = psH.tile([24, 128], F32, tag="h")
                nc.tensor.matmul(out=grp[:], lhsT=rep[:], rhs=gT[:],
                                 start=True, stop=True)
                vvc = ndp.tile([24, 128], BF16, tag="vvc")
                nc.scalar.copy(out=vvc[:], in_=vvp[:])
                vgT = ndp.tile([24, 128], BF16, tag="vgT")
                nc.vector.tensor_tensor(out=vgT[:], in0=vvc[:], in1=grp[:],
                                        op=OP.mult)

                osp = psH.tile([16, 128], F32, tag="h")
                nc.tensor.matmul(out=osp[:], lhsT=wus[:], rhs=scalT[:],
                                 start=True, stop=False)
                nc.tensor.matmul(out=osp[:], lhsT=wss[:], rhs=xnT_s[:],
                                 start=False, stop=True)
                ovp = psH.tile([24, 128], F32, tag="h")
                nc.tensor.matmul(out=ovp[:], lhsT=wuv[:], rhs=vgT[:],
                                 start=True, stop=False)
                nc.tensor.matmul(out=ovp[:], lhsT=wsv[:], rhs=xnT_v[:],
                                 start=False, stop=True)

                fTs = ndp.tile([16, 128], F32, tag="fTs")
                nc.vector.tensor_scalar_mul(out=fTs[:], in0=osp[:], scalar1=res)
                fTv = ndp.tile([24, 128], F32, tag="fTv")
                nc.vector.tensor_scalar_mul(out=fTv[:], in0=ovp[:], scalar1=res)
                fps = psH.tile([128, 16], F32, tag="h")
                nc.tensor.transpose(out=fps[:], in_=fTs[:],
                                    identity=ident[0:16, 0:16])
                fpv = psH.tile([128, 24], F32, tag="h")
                nc.tensor.transpose(out=fpv[:], in_=fTv[:],
                                    identity=ident[0:24, 0:24])
                outt = ndp.tile([128, 40], F32, tag="outt")
                nc.vector.tensor_tensor(out=outt[:, 0:16], in0=xo[:, 0:16],
                                        in1=fps[:], op=OP.add)
                nc.vector.tensor_tensor(out=outt[:, 16:40], in0=xo[:, 16:40],
                                        in1=fpv[:], op=OP.add)
                nc.sync.dma_start(out=out_d[s], in_=outt[:])

    nc.compile()
    return nc


_CACHE = {}


def kernel(**inputs):
    in_maps, metas, nsup, bg2, res, use_bias = _host_prep(**inputs)
    key = (nsup, bg2, res, use_bias)
    if key not in _CACHE:
        _CACHE[key] = build_program(nsup, bg2, res, use_bias)
    nc = _CACHE[key]
    r = run_bass_kernel_spmd(nc, in_maps, list(range(NCORE)))
    out = np.zeros((N, 40), np.float32)
    for k in range(NCORE):
        n0, n1, base_arr, span_arr, ns = metas[k]
        ob = r.results[k]["out"]
        for si in range(ns):
            sp = int(span_arr[si])
            if sp > 0:
                b = int(base_arr[si])
                out[b:b + sp] = ob[si, :sp]
    return out
